# revision 4
# baseline (speedup 1.0000x reference)
"""GAT encoder (PyG GATConv-style, single head) for Trainium2, 8 NeuronCores.

v3: segment-sum as PE matmuls with on-device selection masks.

  Phase 1 (proj): node-partitioned x @ [W | W@att_src | W@att_dst] -> per
  node h (32, bf16) and logits a_s, a_d (bf16).  No activations.

  Host (pure indexing): destinations degree-sorted per core against a
  COMMON degree profile (elementwise max across cores, +0.6% slots) so all
  8 cores share one program.  Edge slots laid dst-major into 128-slot
  windows; each window owns D=13 psum columns (col 0 = carry for a dst
  straddling from the previous window, straddler sits in col D-1; carries
  never cross a 39-window psum bank).

  Phase 2 (agg) per core, DMA-bound at ~17MB (h-stream 14.1MB @360B/ns):
    num  = exp(max(a, 0.2a)), a = a_s[src]+a_d[dst]      (DVE + ACT exp)
    W    = num * (z == j) built as D per-plane tensor_scalar/tensor_tensor
           ops (is_equal planes split DVE/Pool, mults DVE at 2x; D-major
           layout keeps every op densely packed -> 2x mode)
    per window w: two matmuls (stationary ldweights is free in the cost
    model; Matmult cost = out free size only):
      psum[0:32, wD:(w+1)D]  = he_w.T @ W_w    (h gathered per edge slot)
      psum[32:33, wD:(w+1)D] = ones.T @ W_w    (den row, aligned tile pos)
    per bank: ACT-copy psum -> acc_sb bf16; Pool merges window carries.
    PE re-matmul [33,128] acc tiles against [identity | bias-row] ->
    transposed dst-major psum tiles with den*bias folded in
    ((acc + den*bias)/den == acc/den + bias), then per-15-tile rounds:
    DVE rcp(den) + multiply, ACT sigmoid, batched out DMA.

  Host unshards via the rank->column map.  bf16 streams, f32 psum
  accumulate; rel err ~9e-3 vs the 2e-2 gate (fp8 h measured 2.9e-2).
"""
import os
import sys

for _p in ('/opt/trn_rl_repo',):
    if _p not in sys.path and os.path.isdir(_p):
        sys.path.insert(0, _p)

import numpy as np
import ml_dtypes

import concourse.mybir as mybir
import concourse.tile as tile
from concourse import bacc
from concourse.bass_utils import run_bass_kernel_spmd

F32 = mybir.dt.float32
BF16 = mybir.dt.bfloat16
NPBF16 = ml_dtypes.bfloat16

N_CORES = 8
PSUM_CHUNK = 15          # proj: 15*34 = 510 <= 512 f32 per PSUM bank
CW = 34                  # projected width: 32 h + a_s + a_d
D = 13                   # psum columns per 128-slot window
BANK_WINS = 39           # windows per psum bank (39*13 = 507 <= 512)
TP_TILES = 15            # [128,33] bf16 tiles per epilogue round, stride 34
HE_CHUNK_BANKS = 2       # he DMA granularity (banks per DMA)
LAG = int(os.environ.get("GAT_LAG", "0"))    # transpose lag (banks); 0 = all at end
ACCB = int(os.environ.get("GAT_ACCB", "6"))  # psum banks for accumulation
EVAC_DVE = int(os.environ.get("GAT_EVACDVE", "0"))  # every Nth evac on DVE (0=ACT only)
WSPLIT = int(os.environ.get("GAT_WSPLIT", "0"))  # Pool planes (0 = j%2 eq split, mults DVE)
EQPOOL = int(os.environ.get("GAT_EQPOOL", "0"))  # eq planes on Pool (only if WSPLIT=0; 0 = j%2)
ROUNDEND = int(os.environ.get("GAT_ROUNDEND", "0"))  # 1 = epilogue rounds after loop
BIASFOLD = int(os.environ.get("GAT_BIASFOLD", "1"))  # 1 = bias via transpose matmul
OUT_EVERY = int(os.environ.get("GAT_OUTEVERY", "3"))  # rounds per out DMA
WARM = int(os.environ.get("GAT_WARM", "0"))  # fine-grained W warmup chunks

LAST_RESULTS = None
_NC_CACHE = {}
_LAST_NCS = ()


def sim_exec_time_ns():
    """Sum of TimelineSim estimates for the programs run by kernel()."""
    from concourse.timeline_sim import TimelineSim
    return int(sum(TimelineSim(nc, trace=False).simulate()
                   for nc in _LAST_NCS))


# ---------------------------------------------------------------- planning
def _profile_plan(prof):
    """Pack the common degree profile into 128-slot windows.

    Protocol: per window, col 0 is reserved for a carry (continuation of the
    previous window's straddling dst), new dsts take cols 1..D-2, and a dst
    that straddles into the next window takes col D-1 (its continuation gets
    col 0 there).  The last window of each 42-window bank is padded so no
    carry crosses a psum bank.

    Returns (nb, rank_of_slot[nb*128], col_of_slot[nb*128],
    final_col[n_ranks] (global col id w*D+col)).
    """
    n = len(prof)
    rank_of_slot = []
    col_of_slot = []
    final_col = np.zeros(n, np.int64)
    w = 0          # current window index
    p = 0          # next free slot in window
    newd = 0       # new dsts started in this window

    def close_window():
        nonlocal w, p, newd
        pad = 128 - p
        rank_of_slot.extend([-1] * pad)
        col_of_slot.extend([0] * pad)
        w += 1
        p = 0
        newd = 0

    for r in range(n):
        d = int(prof[r])
        assert d >= 1
        while True:
            if p >= 128:
                close_window()
                continue
            if newd >= D - 2:
                close_window()
                continue
            space = 128 - p
            straddle = d > space
            if straddle and (w % BANK_WINS) == BANK_WINS - 1:
                # no carry across banks: pad and start in next bank
                close_window()
                continue
            break
        newd += 1
        if straddle:
            rank_of_slot.extend([r] * space)
            col_of_slot.extend([D - 1] * space)
            p = 128
            close_window()
            rest = d - space
            assert rest <= 128
            rank_of_slot.extend([r] * rest)
            col_of_slot.extend([0] * rest)
            p = rest
            final_col[r] = w * D + 0
        else:
            col = newd  # 1..D-2
            rank_of_slot.extend([r] * d)
            col_of_slot.extend([col] * d)
            p += d
            final_col[r] = w * D + col
    if p > 0:
        close_window()
    # round out to full banks
    while w % BANK_WINS != 0:
        close_window()
    nb = w
    return (nb, np.array(rank_of_slot, np.int64),
            np.array(col_of_slot, np.int64), final_col)


def _plan(dst, N, n_cores):
    """Common profile + per-core degree-sorted dst orders and edge lists."""
    Nc = N // n_cores
    assert Nc * n_cores == N
    degs = np.zeros((n_cores, Nc), np.int64)
    cores = []
    for c in range(n_cores):
        sel = (dst >= c * Nc) & (dst < (c + 1) * Nc)
        idx = np.nonzero(sel)[0]
        d_c = dst[idx] - c * Nc
        order = np.argsort(d_c, kind='stable')
        eidx_sorted = idx[order]            # edge ids grouped by local dst
        counts = np.bincount(d_c, minlength=Nc).astype(np.int64)
        offsets = np.zeros(Nc + 1, np.int64)
        np.cumsum(counts, out=offsets[1:])
        perm = np.argsort(-counts, kind='stable')   # rank -> local dst
        degs[c] = counts[perm]
        cores.append((eidx_sorted, counts, offsets, perm))
    prof = degs.max(axis=0)
    assert prof[-1] >= 1 and prof[0] <= 128
    nb, rank_of_slot, col_of_slot, final_col = _profile_plan(prof)
    return Nc, prof, nb, rank_of_slot, col_of_slot, final_col, cores


def _core_slots(core_plan, prof, rank_of_slot, n_edges, Nc, c):
    """Per-core (src_of_slot, dst_of_slot) with -1 for padding slots."""
    eidx_sorted, counts, offsets, perm = core_plan
    nslots = len(rank_of_slot)
    src_of_slot = np.full(nslots, -1, np.int64)
    dst_of_slot = np.full(nslots, -1, np.int64)
    # slot positions per rank, in slot order
    pos = np.nonzero(rank_of_slot >= 0)[0]
    rk = rank_of_slot[pos]
    # index of each slot within its rank (0..prof[r]-1), slots of a rank
    # appear in increasing slot order
    order = np.argsort(rk, kind='stable')
    within = np.zeros(len(rk), np.int64)
    cum = np.zeros(len(prof) + 1, np.int64)
    np.cumsum(prof, out=cum[1:])
    within[order] = np.arange(len(rk)) - cum[rk[order]]
    ldst = perm[rk]                        # local dst of each real slot
    k = within
    valid = k < counts[ldst]
    epos = offsets[ldst[valid]] + k[valid]
    src_of_slot[pos[valid]] = -2           # placeholder, filled below
    sv = np.full(len(rk), -1, np.int64)
    sv[valid] = eidx_sorted[epos]
    src_of_slot[pos] = sv                  # edge id per slot (-1 pad)
    dst_of_slot[pos[valid]] = ldst[valid] + c * Nc
    return src_of_slot, dst_of_slot


# ---------------------------------------------------------------- phase 1
def _build_proj(nch):
    nc = bacc.Bacc("TRN2", target_bir_lowering=False, debug=False,
                   num_devices=N_CORES)
    xt = nc.dram_tensor("xt", [128, nch * 128], BF16, kind="ExternalInput").ap()
    wext = nc.dram_tensor("wext", [128, CW], BF16, kind="ExternalInput").ap()
    h_out = nc.dram_tensor("h_out", [128, nch * 32], BF16,
                           kind="ExternalOutput").ap()
    a_out = nc.dram_tensor("a_out", [128, nch * 2], BF16,
                           kind="ExternalOutput").ap()
    N_IN_DMA = 4
    with tile.TileContext(nc) as tc:
        with (
            tc.tile_pool(name="const", bufs=1) as cpool,
            tc.tile_pool(name="ps", bufs=8, space="PSUM") as pspool,
        ):
            wsb = cpool.tile([128, CW], BF16)
            xc = cpool.tile([128, nch * 128], BF16)
            qs = [0]
            left = nch
            for frac in (0.33, 0.33, 0.24, 0.10):
                qs.append(min(nch, qs[-1] + max(1, int(round(nch * frac)))))
            qs[-1] = nch
            for i, (k, k1) in enumerate(zip(qs[:-1], qs[1:])):
                if k1 > k:
                    nc.sync.dma_start(xc[:, k * 128:k1 * 128],
                                      xt[:, k * 128:k1 * 128])
                if i == 0:
                    nc.sync.dma_start(wsb[:], wext[:])
            hsb = cpool.tile([128, nch * 32], BF16)
            asd = cpool.tile([128, nch * 2], BF16)
            n_chunks = -(-nch // PSUM_CHUNK)
            marks = [(n_chunks * 5) // 8, (n_chunks * 7) // 8, n_chunks]
            flush = [0] + sorted(set(min(m * PSUM_CHUNK, nch) for m in marks))
            b0 = 0
            while b0 < nch:
                cn = min(PSUM_CHUNK, nch - b0)
                ps = pspool.tile([128, PSUM_CHUNK * CW], F32, tag="ps")
                for j in range(b0, b0 + cn):
                    nc.tensor.matmul(
                        ps[:, (j - b0) * CW:(j - b0 + 1) * CW],
                        xc[:, j * 128:(j + 1) * 128],
                        wsb[:], start=True, stop=True)
                psv = ps[:, :cn * CW].rearrange("p (s f) -> p s f", f=CW)
                if (b0 // PSUM_CHUNK) % 2 == 0:
                    nc.scalar.activation(
                        hsb[:, b0 * 32:(b0 + cn) * 32]
                        .rearrange("p (s c) -> p s c", c=32),
                        psv[:, :, 0:32],
                        mybir.ActivationFunctionType.Copy)
                else:
                    nc.vector.tensor_copy(
                        out=hsb[:, b0 * 32:(b0 + cn) * 32]
                        .rearrange("p (s c) -> p s c", c=32),
                        in_=psv[:, :, 0:32])
                nc.vector.tensor_copy(
                    out=asd[:, b0 * 2:(b0 + cn) * 2]
                    .rearrange("p (s c) -> p s c", c=2),
                    in_=psv[:, :, 32:34])
                b1 = b0 + cn
                # flush h_out at staged boundaries (earlier pieces bigger)
                for lo, hi in zip(flush[:-1], flush[1:]):
                    if b0 < hi <= b1:
                        if hi == nch:
                            nc.sync.dma_start(a_out[:], asd[:])
                        nc.scalar.dma_start(h_out[:, lo * 32:hi * 32],
                                            hsb[:, lo * 32:hi * 32])
                b0 = b1
    nc.compile()
    return nc


# ---------------------------------------------------------------- phase 2
def _build_agg(nb):
    n_banks = nb // BANK_WINS
    ncols = nb * D
    ntp = -(-ncols // 128)                # transpose tiles
    nc = bacc.Bacc("TRN2", target_bir_lowering=False, debug=False,
                   num_devices=N_CORES)
    he = nc.dram_tensor("he", [128, nb * 32], BF16, kind="ExternalInput").ap()
    a_st = nc.dram_tensor("a_st", [128, 2 * nb], BF16,
                          kind="ExternalInput").ap()
    z_st = nc.dram_tensor("z_st", [128, nb], BF16, kind="ExternalInput").ap()
    consts = nc.dram_tensor("consts", [128, D + 33 + 32], BF16,
                            kind="ExternalInput").ap()
    out = nc.dram_tensor("out", [128, ntp * 32], BF16,
                         kind="ExternalOutput").ap()
    # epilogue round boundaries (tiles); last rounds smaller for the tail
    bounds = list(range(0, ntp, TP_TILES))
    if len(bounds) >= 2 and ntp - bounds[-1] > 6:
        bounds = bounds[:-1] + [ntp - 12, ntp - 6]
    elif ntp > 6:
        bounds = bounds[:-1] + [max(0, ntp - 6)]
    bounds = sorted(set(b for b in bounds if b < ntp))
    with tile.TileContext(nc) as tc:
        with (
            tc.tile_pool(name="const", bufs=1) as cpool,
            tc.tile_pool(name="hec", bufs=6) as hepool,
            tc.tile_pool(name="acc", bufs=ACCB, space="PSUM") as accpool,
            tc.tile_pool(name="tp", bufs=8 - ACCB, space="PSUM") as tppool,
        ):
            # ---- constants + small streams (sync queue: ordered first)
            cst = cpool.tile([128, D + 33 + 32], BF16)
            nc.sync.dma_start(cst[:], consts[:])
            ident = cst[:, D:D + 33]      # identity in partitions 0..32
            bias_sb = cst[:, D + 33:D + 33 + 32]
            ones_sb = cpool.tile([128, 1], BF16)
            nc.gpsimd.memset(ones_sb[:], 1.0)
            ac = cpool.tile([128, 2 * nb], BF16)
            nc.sync.dma_start(ac[:], a_st[:])
            zc = cpool.tile([128, nb], BF16)
            nc.sync.dma_start(zc[:], z_st[:])
            # ---- num = exp(max(a, 0.2a))  [128, nb]
            num = cpool.tile([128, nb], BF16)
            wk = cpool.tile([128, nb], BF16)
            nc.vector.tensor_tensor(out=wk[:], in0=ac[:, 0:nb],
                                    in1=ac[:, nb:2 * nb],
                                    op=mybir.AluOpType.add)
            nc.vector.tensor_scalar(out=num[:], in0=wk[:], scalar1=0.2,
                                    scalar2=None, op0=mybir.AluOpType.mult)
            nc.vector.tensor_tensor(out=wk[:], in0=wk[:], in1=num[:],
                                    op=mybir.AluOpType.max)
            nc.scalar.activation(num[:], wk[:],
                                 mybir.ActivationFunctionType.Exp, scale=1.0)
            # ---- W[p, j, b] = num[p, b] * (z[p, b] == j), D-major
            wsel = cpool.tile([128, D * nb], BF16)
            w3 = wsel[:].rearrange("p (d b) -> p d b", b=nb)
            NCHUNK = 6
            cb = -(-nb // NCHUNK)
            wstate = {"done": 0, "warm": WARM}

            def emit_w_chunk():
                s0 = wstate["done"]
                if s0 >= nb:
                    return
                if wstate["warm"] > 0:
                    wstate["warm"] -= 1
                    s1 = min(s0 + BANK_WINS, nb)
                else:
                    s1 = min(s0 + cb, nb)
                for j in range(D):
                    if WSPLIT:
                        eng = nc.gpsimd if j >= D - WSPLIT else nc.vector
                        eng.tensor_scalar(
                            out=w3[:, j, s0:s1], in0=zc[:, s0:s1],
                            scalar1=float(j), scalar2=None,
                            op0=mybir.AluOpType.is_equal)
                        eng.tensor_tensor(
                            out=w3[:, j, s0:s1], in0=w3[:, j, s0:s1],
                            in1=num[:, s0:s1], op=mybir.AluOpType.mult)
                    else:
                        if EQPOOL:
                            eng = nc.gpsimd if j < EQPOOL else nc.vector
                        else:
                            eng = nc.vector if j % 2 == 0 else nc.gpsimd
                        eng.tensor_scalar(
                            out=w3[:, j, s0:s1], in0=zc[:, s0:s1],
                            scalar1=float(j), scalar2=None,
                            op0=mybir.AluOpType.is_equal)
                        nc.vector.tensor_tensor(
                            out=w3[:, j, s0:s1], in0=w3[:, j, s0:s1],
                            in1=num[:, s0:s1], op=mybir.AluOpType.mult)
                wstate["done"] = s1

            for _ in range(4):
                emit_w_chunk()
            # ---- streaming accumulate + interleaved epilogue
            acc_sb = cpool.tile([128, ntp * 128], BF16)
            if ntp * 128 > ncols:
                nc.gpsimd.memset(acc_sb[0:33, ncols:ntp * 128], 0.0)
            out_sb = cpool.tile([128, ntp * 32], BF16)
            rcp = cpool.tile([128, ntp], BF16)
            state = {"tile": 0, "round": 0, "odma": []}

            def emit_transposes(bank_done):
                """Emit transposes fully covered by merged banks <= bank_done."""
                max_t = min(ntp, ((bank_done + 1) * BANK_WINS * D) // 128)
                if bank_done >= n_banks - 1:
                    max_t = ntp
                while state["tile"] < max_t:
                    t = state["tile"]
                    r = state["round"]
                    t0 = bounds[r]
                    if r not in state["tps"]:
                        if BIASFOLD:
                            tp_r = tppool.tile([128, TP_TILES * 33], F32,
                                               tag="tp")
                        else:
                            tp_r = tppool.tile([128, TP_TILES * 34], BF16,
                                               tag="tp")
                        state["tps"][r] = tp_r
                    tp = state["tps"][r]
                    if BIASFOLD:
                        # regular matmul against [identity | bias row]:
                        # transposed acc with den*bias folded in
                        # ((acc + den*bias)*rcp == acc*rcp + bias)
                        nc.tensor.matmul(
                            tp[:, (t - t0) * 33:(t - t0) * 33 + 33],
                            acc_sb[0:33, t * 128:(t + 1) * 128],
                            ident[0:33, 0:33],
                            start=True, stop=True)
                    else:
                        nc.tensor.transpose(
                            tp[:, (t - t0) * 34:(t - t0) * 34 + 33],
                            acc_sb[0:33, t * 128:(t + 1) * 128],
                            ident[0:33, 0:33])
                    state["tile"] = t + 1
                    t1 = bounds[r + 1] if r + 1 < len(bounds) else ntp
                    if t + 1 == t1:
                        if not ROUNDEND:
                            emit_round(r, t0, t1, state["tps"][r])
                        state["round"] = r + 1

            def emit_round(r, t0, t1, tp):
                cw = 33 if BIASFOLD else 34
                tpv = tp[:, :(t1 - t0) * cw] \
                    .rearrange("p (t c) -> p t c", c=cw)
                with nc.allow_low_precision(reason="1/den bf16"):
                    nc.vector.reciprocal(rcp[:, t0:t1], tpv[:, :, 32])
                ov = out_sb[:, t0 * 32:t1 * 32] \
                    .rearrange("p (t c) -> p t c", c=32)
                nc.vector.tensor_tensor(
                    out=ov, in0=tpv[:, :, 0:32],
                    in1=rcp[:, t0:t1].rearrange("p (t o) -> p t o", o=1)
                    .to_broadcast([128, t1 - t0, 32]),
                    op=mybir.AluOpType.mult)
                if not BIASFOLD:
                    nc.vector.tensor_tensor(
                        out=ov, in0=ov,
                        in1=bias_sb.rearrange("p (o c) -> p o c", o=1)
                        .to_broadcast([128, t1 - t0, 32]),
                        op=mybir.AluOpType.add)
                nc.scalar.activation(out_sb[:, t0 * 32:t1 * 32],
                                     out_sb[:, t0 * 32:t1 * 32],
                                     mybir.ActivationFunctionType.Sigmoid)
                state["odma"].append((t0, t1))
                flush = (r % OUT_EVERY == OUT_EVERY - 1
                         or t1 >= ntp)
                if flush:
                    o0 = state["odma"][0][0]
                    o1 = state["odma"][-1][1]
                    state["odma"] = []
                    nc.scalar.dma_start(out[:, o0 * 32:o1 * 32],
                                        out_sb[:, o0 * 32:o1 * 32])

            # tp tiles must be allocated per round; pre-wire creation order
            state["tps"] = {}
            for s0 in range(0, nb, HE_CHUNK_BANKS * BANK_WINS):
                s1 = min(s0 + HE_CHUNK_BANKS * BANK_WINS, nb)
                hc = hepool.tile([128, HE_CHUNK_BANKS * BANK_WINS * 32], BF16,
                                 tag="hec")
                nc.sync.dma_start(hc[:, :(s1 - s0) * 32],
                                  he[:, s0 * 32:s1 * 32])
                # keep the on-device W build ~3 he-chunks ahead of the
                # matmul stream so merges queue promptly behind it
                if wstate["done"] < min(nb, s1 + 3 * HE_CHUNK_BANKS * BANK_WINS):
                    emit_w_chunk()
                for b in range(s0 // BANK_WINS,
                               s0 // BANK_WINS + HE_CHUNK_BANKS):
                    if b >= n_banks:
                        break
                    while wstate["done"] < min(nb, (b + 1) * BANK_WINS):
                        emit_w_chunk()
                    w0 = b * BANK_WINS
                    ap = accpool.tile([128, 512], F32, tag="acc")
                    for w in range(w0, min(w0 + BANK_WINS, nb)):
                        lw = w - s0
                        nc.tensor.matmul(
                            ap[0:32, (w - w0) * D:(w - w0 + 1) * D],
                            hc[:, lw * 32:(lw + 1) * 32],
                            w3[:, :, w],
                            start=True, stop=True)
                        nc.tensor.matmul(
                            ap[32:33, (w - w0) * D:(w - w0 + 1) * D],
                            ones_sb[:], w3[:, :, w],
                            start=True, stop=True)
                    # evacuate bank -> acc_sb (mostly ACT; Copy is in
                    # every act table set so no reloads)
                    if EVAC_DVE and b % EVAC_DVE == EVAC_DVE - 1:
                        nc.vector.tensor_copy(
                            out=acc_sb[0:33, w0 * D:(w0 + BANK_WINS) * D],
                            in_=ap[0:33, 0:BANK_WINS * D])
                    else:
                        nc.scalar.activation(
                            acc_sb[0:33, w0 * D:(w0 + BANK_WINS) * D],
                            ap[0:33, 0:BANK_WINS * D],
                            mybir.ActivationFunctionType.Copy)
                    # merge carries within the bank (Pool, sbuf only)
                    a3o = acc_sb[0:33, w0 * D + D:(w0 + BANK_WINS) * D] \
                        .rearrange("p (b d) -> p b d", d=D)
                    a3i = acc_sb[0:33, w0 * D + D - 1:
                                 (w0 + BANK_WINS) * D - 1] \
                        .rearrange("p (b d) -> p b d", d=D)
                    nc.gpsimd.tensor_tensor(
                        out=a3o[:, :, 0:1], in0=a3o[:, :, 0:1],
                        in1=a3i[:, :, 0:1], op=mybir.AluOpType.add)
                    # interleave transposes/epilogue with a lag so their
                    # evac/merge deps are long resolved by the time in-order
                    # PE.SEQ reaches them
                    if LAG > 0 and b >= LAG:
                        emit_transposes(b - LAG)
            emit_transposes(n_banks - 1)
            assert state["tile"] == ntp and state["round"] == len(bounds)
            if ROUNDEND:
                for r, t0 in enumerate(bounds):
                    t1 = bounds[r + 1] if r + 1 < len(bounds) else ntp
                    emit_round(r, t0, t1, state["tps"][r])
    nc.compile()
    return nc


# ---------------------------------------------------------------- runners
def _run(nc, in_maps, trace):
    if os.environ.get("GAT_SIM"):
        from concourse.bass_interp import CoreSim
        results = []
        for m in in_maps:
            sim = CoreSim(nc, require_finite=False, require_nnan=False)
            for k, v in m.items():
                sim.tensor(k)[:] = v
            sim.simulate()
            outs = {}
            for alloc in nc.m.functions[0].allocations:
                if getattr(alloc, 'kind', None) == "ExternalOutput":
                    name = alloc.memorylocations[0].name
                    outs[name] = np.array(sim.tensor(name))
            results.append(outs)
        class R: pass
        r = R(); r.results = results; r.exec_time_ns = None
        return r
    return run_bass_kernel_spmd(nc, in_maps, core_ids=list(range(N_CORES)),
                                trace=trace)


def kernel(x, edge_index, W, att_src, att_dst, bias):
    global LAST_RESULTS, _LAST_NCS
    x = np.asarray(x, np.float32)
    edge_index = np.asarray(edge_index)
    W = np.asarray(W, np.float32)
    att_src = np.asarray(att_src, np.float32)
    att_dst = np.asarray(att_dst, np.float32)
    bias_np = np.asarray(bias, np.float32)

    N, C_in = x.shape
    C_out = W.shape[1]
    assert C_in == 128 and C_out == 32, (C_in, C_out)
    trace = bool(os.environ.get("GAT_TRACE"))

    loops = np.arange(N, dtype=np.int64)
    src = np.concatenate([edge_index[0].astype(np.int64), loops])
    dst = np.concatenate([edge_index[1].astype(np.int64), loops])

    Nc, prof, nb, rank_of_slot, col_of_slot, final_col, cores = \
        _plan(dst, N, N_CORES)
    nch = -(-Nc // 128)

    # ---- phase 1: project every node once (node-partitioned) ----
    ws = W @ att_src
    wd = W @ att_dst
    wext = np.concatenate([W, ws[:, None], wd[:, None]], 1).astype(NPBF16)
    in1 = []
    for c in range(N_CORES):
        xt = np.zeros((128, nch * 128), NPBF16)
        n0 = c * Nc
        xt[:, :Nc] = x[n0:n0 + Nc].T.astype(NPBF16)
        in1.append({"xt": xt, "wext": wext})

    key1 = ("proj", nch)
    if key1 not in _NC_CACHE:
        _NC_CACHE[key1] = _build_proj(nch)
    nc1 = _NC_CACHE[key1]
    res1 = _run(nc1, in1, trace)

    h_all = np.zeros((N, 32), NPBF16)
    as_all = np.zeros(N, NPBF16)
    ad_all = np.zeros(N, NPBF16)
    for c in range(N_CORES):
        o = res1.results[c]
        h = np.asarray(o["h_out"]).reshape(128, nch, 32) \
            .transpose(1, 0, 2).reshape(nch * 128, 32)[:Nc]
        a = np.asarray(o["a_out"]).reshape(128, nch, 2) \
            .transpose(1, 0, 2).reshape(nch * 128, 2)[:Nc]
        sl = slice(c * Nc, (c + 1) * Nc)
        h_all[sl] = h
        as_all[sl], ad_all[sl] = a[:, 0], a[:, 1]

    # ---- host gather: per-core streams (pure indexing) ----
    h_pad = np.concatenate([h_all, np.zeros((1, 32), NPBF16)], 0)
    as_pad = np.concatenate([as_all, [NPBF16(-300.0)]])
    ad_pad = np.concatenate([ad_all, [NPBF16(-300.0)]])

    z_np = np.broadcast_to(
        col_of_slot.reshape(nb, 128).T.astype(NPBF16), (128, nb)).copy()
    consts = np.zeros((128, D + 33 + 32), NPBF16)
    consts[:, 0:D] = np.arange(D, dtype=np.float32).astype(NPBF16)[None, :]
    idb = np.eye(33, dtype=np.float32)
    idb[32, 0:32] = bias_np          # transpose-matmul folds den*bias in
    consts[0:33, D:D + 33] = idb.astype(NPBF16)
    consts[:, D + 33:] = bias_np.astype(NPBF16)[None, :]

    ncols = nb * D
    ntp = -(-ncols // 128)

    in2 = []
    for c in range(N_CORES):
        src_of_slot, dst_of_slot = _core_slots(
            cores[c], prof, rank_of_slot, len(src), Nc, c)
        sid = np.where(src_of_slot >= 0, src[np.maximum(src_of_slot, 0)], N)
        did = np.where(dst_of_slot >= 0, dst_of_slot, N)
        he = np.empty((128, nb * 32), NPBF16)
        he3 = he.reshape(128, nb, 32)
        hrows = h_pad[sid].reshape(nb, 128, 32)
        he3[:, :, :] = hrows.transpose(1, 0, 2)
        a_st = np.empty((128, 2 * nb), NPBF16)
        a_st[:, 0:nb] = as_pad[sid].reshape(nb, 128).T
        a_st[:, nb:2 * nb] = ad_pad[did].reshape(nb, 128).T
        in2.append({"he": he, "a_st": a_st, "z_st": z_np, "consts": consts})

    key2 = ("agg", nb, LAG, WARM, ACCB, EVAC_DVE, WSPLIT, EQPOOL, ROUNDEND, BIASFOLD, OUT_EVERY)
    if key2 not in _NC_CACHE:
        _NC_CACHE[key2] = _build_agg(nb)
    nc2 = _NC_CACHE[key2]
    res2 = _run(nc2, in2, trace)
    LAST_RESULTS = (res1, res2)
    _LAST_NCS = (nc1, nc2)

    out_full = np.zeros((N, C_out), np.float32)
    fc = final_col                       # rank -> global col id
    tpi = fc // 128                      # transpose tile
    tpp = fc % 128                       # partition within tile
    for c in range(N_CORES):
        o = np.asarray(res2.results[c]["out"]).astype(np.float32)
        o3 = o.reshape(128, ntp, 32)
        vals = o3[tpp, tpi, :]           # [n_ranks, 32]
        perm = cores[c][3]
        out_full[c * Nc + perm] = vals
    return out_full


# revision 7
# speedup vs baseline: 1.0102x; 1.0102x over previous
"""GAT encoder (PyG GATConv-style, single head) for Trainium2, 8 NeuronCores.

v3: segment-sum as PE matmuls with on-device selection masks.

  Phase 1 (proj): node-partitioned x @ [W | W@att_src | W@att_dst] -> per
  node h (32, bf16) and logits a_s, a_d (bf16).  No activations.

  Host (pure indexing): destinations degree-sorted per core against a
  COMMON degree profile (elementwise max across cores, +0.6% slots) so all
  8 cores share one program.  Edge slots laid dst-major into 128-slot
  windows; each window owns D=13 psum columns (col 0 = carry for a dst
  straddling from the previous window, straddler sits in col D-1; carries
  never cross a 39-window psum bank).

  Phase 2 (agg) per core, DMA-bound at ~17MB (h-stream 14.1MB @360B/ns):
    num  = exp(max(a, 0.2a)), a = a_s[src]+a_d[dst]      (DVE + ACT exp)
    W    = num * (z == j) built as D per-plane tensor_scalar/tensor_tensor
           ops (is_equal planes split DVE/Pool, mults DVE at 2x; D-major
           layout keeps every op densely packed -> 2x mode)
    per window w: two matmuls (stationary ldweights is free in the cost
    model; Matmult cost = out free size only):
      psum[0:32, wD:(w+1)D]  = he_w.T @ W_w    (h gathered per edge slot)
      psum[32:33, wD:(w+1)D] = ones.T @ W_w    (den row, aligned tile pos)
    per bank: ACT-copy psum -> acc_sb bf16; Pool merges window carries.
    PE re-matmul [33,128] acc tiles against [identity | bias-row] ->
    transposed dst-major psum tiles with den*bias folded in
    ((acc + den*bias)/den == acc/den + bias), then per-15-tile rounds:
    DVE rcp(den) + multiply, ACT sigmoid, batched out DMA.

  Host unshards via the rank->column map.  bf16 streams, f32 psum
  accumulate; rel err ~9e-3 vs the 2e-2 gate (fp8 h measured 2.9e-2).
"""
import os
import sys

for _p in ('/opt/trn_rl_repo',):
    if _p not in sys.path and os.path.isdir(_p):
        sys.path.insert(0, _p)

import numpy as np
import ml_dtypes

import concourse.mybir as mybir
import concourse.tile as tile
from concourse import bacc
from concourse.bass_utils import run_bass_kernel_spmd

F32 = mybir.dt.float32
BF16 = mybir.dt.bfloat16
NPBF16 = ml_dtypes.bfloat16

N_CORES = 8
PSUM_CHUNK = 15          # proj: 15*34 = 510 <= 512 f32 per PSUM bank
CW = 34                  # projected width: 32 h + a_s + a_d
D = 13                   # psum columns per 128-slot window
BANK_WINS = 39           # windows per psum bank (39*13 = 507 <= 512)
TP_TILES = 15            # [128,33] bf16 tiles per epilogue round, stride 34
HE_CHUNK_BANKS = 2       # he DMA granularity (banks per DMA)
LAG = int(os.environ.get("GAT_LAG", "0"))    # transpose lag (banks); 0 = all at end
ACCB = int(os.environ.get("GAT_ACCB", "6"))  # psum banks for accumulation
EVAC_DVE = int(os.environ.get("GAT_EVACDVE", "0"))  # every Nth evac on DVE (0=ACT only)
WSPLIT = int(os.environ.get("GAT_WSPLIT", "0"))  # Pool planes (0 = j%2 eq split, mults DVE)
EQPOOL = int(os.environ.get("GAT_EQPOOL", "0"))  # eq planes on Pool (only if WSPLIT=0; 0 = j%2)
ROUNDEND = int(os.environ.get("GAT_ROUNDEND", "0"))  # 1 = epilogue rounds after loop
BIASFOLD = int(os.environ.get("GAT_BIASFOLD", "1"))  # 1 = bias via transpose matmul
OUT_EVERY = int(os.environ.get("GAT_OUTEVERY", "3"))  # rounds per out DMA
WARM = int(os.environ.get("GAT_WARM", "0"))  # fine-grained W warmup chunks

LAST_RESULTS = None
_NC_CACHE = {}
_LAST_NCS = ()


def sim_exec_time_ns():
    """Sum of TimelineSim estimates for the programs run by kernel()."""
    from concourse.timeline_sim import TimelineSim
    return int(sum(TimelineSim(nc, trace=False).simulate()
                   for nc in _LAST_NCS))


# ---------------------------------------------------------------- planning
def _profile_plan(prof):
    """Pack the common degree profile into 128-slot windows.

    Protocol: per window, col 0 is reserved for a carry (continuation of the
    previous window's straddling dst), new dsts take cols 1..D-2, and a dst
    that straddles into the next window takes col D-1 (its continuation gets
    col 0 there).  The last window of each 42-window bank is padded so no
    carry crosses a psum bank.

    Returns (nb, rank_of_slot[nb*128], col_of_slot[nb*128],
    final_col[n_ranks] (global col id w*D+col)).
    """
    n = len(prof)
    rank_of_slot = []
    col_of_slot = []
    final_col = np.zeros(n, np.int64)
    w = 0          # current window index
    p = 0          # next free slot in window
    newd = 0       # new dsts started in this window

    def close_window():
        nonlocal w, p, newd
        pad = 128 - p
        rank_of_slot.extend([-1] * pad)
        col_of_slot.extend([0] * pad)
        w += 1
        p = 0
        newd = 0

    for r in range(n):
        d = int(prof[r])
        assert d >= 1
        while True:
            if p >= 128:
                close_window()
                continue
            if newd >= D - 2:
                close_window()
                continue
            space = 128 - p
            straddle = d > space
            if straddle and (w % BANK_WINS) == BANK_WINS - 1:
                # no carry across banks: pad and start in next bank
                close_window()
                continue
            break
        newd += 1
        if straddle:
            rank_of_slot.extend([r] * space)
            col_of_slot.extend([D - 1] * space)
            p = 128
            close_window()
            rest = d - space
            assert rest <= 128
            rank_of_slot.extend([r] * rest)
            col_of_slot.extend([0] * rest)
            p = rest
            final_col[r] = w * D + 0
        else:
            col = newd  # 1..D-2
            rank_of_slot.extend([r] * d)
            col_of_slot.extend([col] * d)
            p += d
            final_col[r] = w * D + col
    if p > 0:
        close_window()
    nb = w
    return (nb, np.array(rank_of_slot, np.int64),
            np.array(col_of_slot, np.int64), final_col)


def _plan(dst, N, n_cores):
    """Common profile + per-core degree-sorted dst orders and edge lists."""
    Nc = N // n_cores
    assert Nc * n_cores == N
    degs = np.zeros((n_cores, Nc), np.int64)
    cores = []
    for c in range(n_cores):
        sel = (dst >= c * Nc) & (dst < (c + 1) * Nc)
        idx = np.nonzero(sel)[0]
        d_c = dst[idx] - c * Nc
        order = np.argsort(d_c, kind='stable')
        eidx_sorted = idx[order]            # edge ids grouped by local dst
        counts = np.bincount(d_c, minlength=Nc).astype(np.int64)
        offsets = np.zeros(Nc + 1, np.int64)
        np.cumsum(counts, out=offsets[1:])
        perm = np.argsort(-counts, kind='stable')   # rank -> local dst
        degs[c] = counts[perm]
        cores.append((eidx_sorted, counts, offsets, perm))
    prof = degs.max(axis=0)
    assert prof[-1] >= 1 and prof[0] <= 128
    nb, rank_of_slot, col_of_slot, final_col = _profile_plan(prof)
    return Nc, prof, nb, rank_of_slot, col_of_slot, final_col, cores


def _core_slots(core_plan, prof, rank_of_slot, n_edges, Nc, c):
    """Per-core (src_of_slot, dst_of_slot) with -1 for padding slots."""
    eidx_sorted, counts, offsets, perm = core_plan
    nslots = len(rank_of_slot)
    src_of_slot = np.full(nslots, -1, np.int64)
    dst_of_slot = np.full(nslots, -1, np.int64)
    # slot positions per rank, in slot order
    pos = np.nonzero(rank_of_slot >= 0)[0]
    rk = rank_of_slot[pos]
    # index of each slot within its rank (0..prof[r]-1), slots of a rank
    # appear in increasing slot order
    order = np.argsort(rk, kind='stable')
    within = np.zeros(len(rk), np.int64)
    cum = np.zeros(len(prof) + 1, np.int64)
    np.cumsum(prof, out=cum[1:])
    within[order] = np.arange(len(rk)) - cum[rk[order]]
    ldst = perm[rk]                        # local dst of each real slot
    k = within
    valid = k < counts[ldst]
    epos = offsets[ldst[valid]] + k[valid]
    src_of_slot[pos[valid]] = -2           # placeholder, filled below
    sv = np.full(len(rk), -1, np.int64)
    sv[valid] = eidx_sorted[epos]
    src_of_slot[pos] = sv                  # edge id per slot (-1 pad)
    dst_of_slot[pos[valid]] = ldst[valid] + c * Nc
    return src_of_slot, dst_of_slot


# ---------------------------------------------------------------- phase 1
def _build_proj(nch):
    nc = bacc.Bacc("TRN2", target_bir_lowering=False, debug=False,
                   num_devices=N_CORES)
    xt = nc.dram_tensor("xt", [128, nch * 128], BF16, kind="ExternalInput").ap()
    wext = nc.dram_tensor("wext", [128, CW], BF16, kind="ExternalInput").ap()
    h_out = nc.dram_tensor("h_out", [128, nch * 32], BF16,
                           kind="ExternalOutput").ap()
    a_out = nc.dram_tensor("a_out", [128, nch * 2], BF16,
                           kind="ExternalOutput").ap()
    N_IN_DMA = 4
    with tile.TileContext(nc) as tc:
        with (
            tc.tile_pool(name="const", bufs=1) as cpool,
            tc.tile_pool(name="ps", bufs=8, space="PSUM") as pspool,
        ):
            wsb = cpool.tile([128, CW], BF16)
            xc = cpool.tile([128, nch * 128], BF16)
            qs = [0]
            left = nch
            for frac in (0.33, 0.33, 0.24, 0.10):
                qs.append(min(nch, qs[-1] + max(1, int(round(nch * frac)))))
            qs[-1] = nch
            for i, (k, k1) in enumerate(zip(qs[:-1], qs[1:])):
                if k1 > k:
                    nc.sync.dma_start(xc[:, k * 128:k1 * 128],
                                      xt[:, k * 128:k1 * 128])
                if i == 0:
                    nc.sync.dma_start(wsb[:], wext[:])
            hsb = cpool.tile([128, nch * 32], BF16)
            asd = cpool.tile([128, nch * 2], BF16)
            n_chunks = -(-nch // PSUM_CHUNK)
            marks = [(n_chunks * 5) // 8, (n_chunks * 7) // 8, n_chunks]
            flush = [0] + sorted(set(min(m * PSUM_CHUNK, nch) for m in marks))
            b0 = 0
            while b0 < nch:
                cn = min(PSUM_CHUNK, nch - b0)
                ps = pspool.tile([128, PSUM_CHUNK * CW], F32, tag="ps")
                for j in range(b0, b0 + cn):
                    nc.tensor.matmul(
                        ps[:, (j - b0) * CW:(j - b0 + 1) * CW],
                        xc[:, j * 128:(j + 1) * 128],
                        wsb[:], start=True, stop=True)
                psv = ps[:, :cn * CW].rearrange("p (s f) -> p s f", f=CW)
                if (b0 // PSUM_CHUNK) % 2 == 0:
                    nc.scalar.activation(
                        hsb[:, b0 * 32:(b0 + cn) * 32]
                        .rearrange("p (s c) -> p s c", c=32),
                        psv[:, :, 0:32],
                        mybir.ActivationFunctionType.Copy)
                else:
                    nc.vector.tensor_copy(
                        out=hsb[:, b0 * 32:(b0 + cn) * 32]
                        .rearrange("p (s c) -> p s c", c=32),
                        in_=psv[:, :, 0:32])
                nc.vector.tensor_copy(
                    out=asd[:, b0 * 2:(b0 + cn) * 2]
                    .rearrange("p (s c) -> p s c", c=2),
                    in_=psv[:, :, 32:34])
                b1 = b0 + cn
                # flush h_out at staged boundaries (earlier pieces bigger)
                for lo, hi in zip(flush[:-1], flush[1:]):
                    if b0 < hi <= b1:
                        if hi == nch:
                            nc.sync.dma_start(a_out[:], asd[:])
                        nc.scalar.dma_start(h_out[:, lo * 32:hi * 32],
                                            hsb[:, lo * 32:hi * 32])
                b0 = b1
    nc.compile()
    return nc


# ---------------------------------------------------------------- phase 2
def _build_agg(nb):
    n_banks = -(-nb // BANK_WINS)
    ncols = nb * D
    ntp = -(-ncols // 128)                # transpose tiles
    nc = bacc.Bacc("TRN2", target_bir_lowering=False, debug=False,
                   num_devices=N_CORES)
    he = nc.dram_tensor("he", [128, nb * 32], BF16, kind="ExternalInput").ap()
    a_st = nc.dram_tensor("a_st", [128, 2 * nb], BF16,
                          kind="ExternalInput").ap()
    z_st = nc.dram_tensor("z_st", [128, nb], BF16, kind="ExternalInput").ap()
    consts = nc.dram_tensor("consts", [128, D + 33 + 32], BF16,
                            kind="ExternalInput").ap()
    out = nc.dram_tensor("out", [128, ntp * 32], BF16,
                         kind="ExternalOutput").ap()
    # epilogue round boundaries (tiles); last rounds smaller for the tail
    bounds = list(range(0, ntp, TP_TILES))
    if len(bounds) >= 2 and ntp - bounds[-1] > 6:
        bounds = bounds[:-1] + [ntp - 12, ntp - 6]
    elif ntp > 6:
        bounds = bounds[:-1] + [max(0, ntp - 6)]
    bounds = sorted(set(b for b in bounds if b < ntp))
    with tile.TileContext(nc) as tc:
        with (
            tc.tile_pool(name="const", bufs=1) as cpool,
            tc.tile_pool(name="hec", bufs=6) as hepool,
            tc.tile_pool(name="acc", bufs=ACCB, space="PSUM") as accpool,
            tc.tile_pool(name="tp", bufs=8 - ACCB, space="PSUM") as tppool,
        ):
            # ---- constants + small streams (sync queue: ordered first)
            cst = cpool.tile([128, D + 33 + 32], BF16)
            nc.sync.dma_start(cst[:], consts[:])
            ident = cst[:, D:D + 33]      # identity in partitions 0..32
            bias_sb = cst[:, D + 33:D + 33 + 32]
            ones_sb = cpool.tile([128, 1], BF16)
            nc.gpsimd.memset(ones_sb[:], 1.0)
            ac = cpool.tile([128, 2 * nb], BF16)
            nc.sync.dma_start(ac[:], a_st[:])
            zc = cpool.tile([128, nb], BF16)
            nc.sync.dma_start(zc[:], z_st[:])
            # ---- num = exp(max(a, 0.2a))  [128, nb]
            num = cpool.tile([128, nb], BF16)
            wk = cpool.tile([128, nb], BF16)
            nc.vector.tensor_tensor(out=wk[:], in0=ac[:, 0:nb],
                                    in1=ac[:, nb:2 * nb],
                                    op=mybir.AluOpType.add)
            nc.vector.tensor_scalar(out=num[:], in0=wk[:], scalar1=0.2,
                                    scalar2=None, op0=mybir.AluOpType.mult)
            nc.vector.tensor_tensor(out=wk[:], in0=wk[:], in1=num[:],
                                    op=mybir.AluOpType.max)
            nc.scalar.activation(num[:], wk[:],
                                 mybir.ActivationFunctionType.Exp, scale=1.0)
            # ---- W[p, j, b] = num[p, b] * (z[p, b] == j), D-major
            wsel = cpool.tile([128, D * nb], BF16)
            w3 = wsel[:].rearrange("p (d b) -> p d b", b=nb)
            NCHUNK = 6
            cb = -(-nb // NCHUNK)
            wstate = {"done": 0, "warm": WARM}

            def emit_w_chunk():
                s0 = wstate["done"]
                if s0 >= nb:
                    return
                if wstate["warm"] > 0:
                    wstate["warm"] -= 1
                    s1 = min(s0 + BANK_WINS, nb)
                else:
                    s1 = min(s0 + cb, nb)
                for j in range(D):
                    if WSPLIT:
                        eng = nc.gpsimd if j >= D - WSPLIT else nc.vector
                        eng.tensor_scalar(
                            out=w3[:, j, s0:s1], in0=zc[:, s0:s1],
                            scalar1=float(j), scalar2=None,
                            op0=mybir.AluOpType.is_equal)
                        eng.tensor_tensor(
                            out=w3[:, j, s0:s1], in0=w3[:, j, s0:s1],
                            in1=num[:, s0:s1], op=mybir.AluOpType.mult)
                    else:
                        if EQPOOL:
                            eng = nc.gpsimd if j < EQPOOL else nc.vector
                        else:
                            eng = nc.vector if j % 2 == 0 else nc.gpsimd
                        eng.tensor_scalar(
                            out=w3[:, j, s0:s1], in0=zc[:, s0:s1],
                            scalar1=float(j), scalar2=None,
                            op0=mybir.AluOpType.is_equal)
                        nc.vector.tensor_tensor(
                            out=w3[:, j, s0:s1], in0=w3[:, j, s0:s1],
                            in1=num[:, s0:s1], op=mybir.AluOpType.mult)
                wstate["done"] = s1

            for _ in range(4):
                emit_w_chunk()
            # ---- streaming accumulate + interleaved epilogue
            acc_sb = cpool.tile([128, ntp * 128], BF16)
            if ntp * 128 > ncols:
                nc.gpsimd.memset(acc_sb[0:33, ncols:ntp * 128], 0.0)
            out_sb = cpool.tile([128, ntp * 32], BF16)
            rcp = cpool.tile([128, ntp], BF16)
            state = {"tile": 0, "round": 0, "odma": []}

            def emit_transposes(bank_done):
                """Emit transposes fully covered by merged banks <= bank_done."""
                max_t = min(ntp, ((bank_done + 1) * BANK_WINS * D) // 128)
                if bank_done >= n_banks - 1:
                    max_t = ntp
                while state["tile"] < max_t:
                    t = state["tile"]
                    r = state["round"]
                    t0 = bounds[r]
                    if r not in state["tps"]:
                        if BIASFOLD:
                            tp_r = tppool.tile([128, TP_TILES * 33], F32,
                                               tag="tp")
                        else:
                            tp_r = tppool.tile([128, TP_TILES * 34], BF16,
                                               tag="tp")
                        state["tps"][r] = tp_r
                    tp = state["tps"][r]
                    if BIASFOLD:
                        # regular matmul against [identity | bias row]:
                        # transposed acc with den*bias folded in
                        # ((acc + den*bias)*rcp == acc*rcp + bias)
                        nc.tensor.matmul(
                            tp[:, (t - t0) * 33:(t - t0) * 33 + 33],
                            acc_sb[0:33, t * 128:(t + 1) * 128],
                            ident[0:33, 0:33],
                            start=True, stop=True)
                    else:
                        nc.tensor.transpose(
                            tp[:, (t - t0) * 34:(t - t0) * 34 + 33],
                            acc_sb[0:33, t * 128:(t + 1) * 128],
                            ident[0:33, 0:33])
                    state["tile"] = t + 1
                    t1 = bounds[r + 1] if r + 1 < len(bounds) else ntp
                    if t + 1 == t1:
                        if not ROUNDEND:
                            emit_round(r, t0, t1, state["tps"][r])
                        state["round"] = r + 1

            def emit_round(r, t0, t1, tp):
                cw = 33 if BIASFOLD else 34
                tpv = tp[:, :(t1 - t0) * cw] \
                    .rearrange("p (t c) -> p t c", c=cw)
                with nc.allow_low_precision(reason="1/den bf16"):
                    nc.vector.reciprocal(rcp[:, t0:t1], tpv[:, :, 32])
                ov = out_sb[:, t0 * 32:t1 * 32] \
                    .rearrange("p (t c) -> p t c", c=32)
                nc.vector.tensor_tensor(
                    out=ov, in0=tpv[:, :, 0:32],
                    in1=rcp[:, t0:t1].rearrange("p (t o) -> p t o", o=1)
                    .to_broadcast([128, t1 - t0, 32]),
                    op=mybir.AluOpType.mult)
                if not BIASFOLD:
                    nc.vector.tensor_tensor(
                        out=ov, in0=ov,
                        in1=bias_sb.rearrange("p (o c) -> p o c", o=1)
                        .to_broadcast([128, t1 - t0, 32]),
                        op=mybir.AluOpType.add)
                nc.scalar.activation(out_sb[:, t0 * 32:t1 * 32],
                                     out_sb[:, t0 * 32:t1 * 32],
                                     mybir.ActivationFunctionType.Sigmoid)
                state["odma"].append((t0, t1))
                flush = (r % OUT_EVERY == OUT_EVERY - 1
                         or t1 >= ntp)
                if flush:
                    o0 = state["odma"][0][0]
                    o1 = state["odma"][-1][1]
                    state["odma"] = []
                    nc.scalar.dma_start(out[:, o0 * 32:o1 * 32],
                                        out_sb[:, o0 * 32:o1 * 32])

            # tp tiles must be allocated per round; pre-wire creation order
            state["tps"] = {}
            for s0 in range(0, nb, HE_CHUNK_BANKS * BANK_WINS):
                s1 = min(s0 + HE_CHUNK_BANKS * BANK_WINS, nb)
                hc = hepool.tile([128, HE_CHUNK_BANKS * BANK_WINS * 32], BF16,
                                 tag="hec")
                nc.sync.dma_start(hc[:, :(s1 - s0) * 32],
                                  he[:, s0 * 32:s1 * 32])
                # keep the on-device W build ~3 he-chunks ahead of the
                # matmul stream so merges queue promptly behind it
                if wstate["done"] < min(nb, s1 + 3 * HE_CHUNK_BANKS * BANK_WINS):
                    emit_w_chunk()
                for b in range(s0 // BANK_WINS,
                               s0 // BANK_WINS + HE_CHUNK_BANKS):
                    if b >= n_banks:
                        break
                    while wstate["done"] < min(nb, (b + 1) * BANK_WINS):
                        emit_w_chunk()
                    w0 = b * BANK_WINS
                    w1 = min(w0 + BANK_WINS, nb)
                    ap = accpool.tile([128, 512], F32, tag="acc")
                    for w in range(w0, min(w0 + BANK_WINS, nb)):
                        lw = w - s0
                        nc.tensor.matmul(
                            ap[0:32, (w - w0) * D:(w - w0 + 1) * D],
                            hc[:, lw * 32:(lw + 1) * 32],
                            w3[:, :, w],
                            start=True, stop=True)
                        nc.tensor.matmul(
                            ap[32:33, (w - w0) * D:(w - w0 + 1) * D],
                            ones_sb[:], w3[:, :, w],
                            start=True, stop=True)
                    # evacuate bank -> acc_sb (mostly ACT; Copy is in
                    # every act table set so no reloads)
                    if EVAC_DVE and b % EVAC_DVE == EVAC_DVE - 1:
                        nc.vector.tensor_copy(
                            out=acc_sb[0:33, w0 * D:w1 * D],
                            in_=ap[0:33, 0:(w1 - w0) * D])
                    else:
                        nc.scalar.activation(
                            acc_sb[0:33, w0 * D:w1 * D],
                            ap[0:33, 0:(w1 - w0) * D],
                            mybir.ActivationFunctionType.Copy)
                    # merge carries within the bank (Pool, sbuf only)
                    if w1 - w0 > 1:
                        a3o = acc_sb[0:33, w0 * D + D:w1 * D] \
                            .rearrange("p (b d) -> p b d", d=D)
                        a3i = acc_sb[0:33, w0 * D + D - 1:w1 * D - 1] \
                            .rearrange("p (b d) -> p b d", d=D)
                        nc.gpsimd.tensor_tensor(
                            out=a3o[:, :, 0:1], in0=a3o[:, :, 0:1],
                            in1=a3i[:, :, 0:1], op=mybir.AluOpType.add)
                    # interleave transposes/epilogue with a lag so their
                    # evac/merge deps are long resolved by the time in-order
                    # PE.SEQ reaches them
                    if LAG > 0 and b >= LAG:
                        emit_transposes(b - LAG)
            emit_transposes(n_banks - 1)
            assert state["tile"] == ntp and state["round"] == len(bounds)
            if ROUNDEND:
                for r, t0 in enumerate(bounds):
                    t1 = bounds[r + 1] if r + 1 < len(bounds) else ntp
                    emit_round(r, t0, t1, state["tps"][r])
    nc.compile()
    return nc


# ---------------------------------------------------------------- runners
def _run(nc, in_maps, trace):
    if os.environ.get("GAT_SIM"):
        from concourse.bass_interp import CoreSim
        results = []
        for m in in_maps:
            sim = CoreSim(nc, require_finite=False, require_nnan=False)
            for k, v in m.items():
                sim.tensor(k)[:] = v
            sim.simulate()
            outs = {}
            for alloc in nc.m.functions[0].allocations:
                if getattr(alloc, 'kind', None) == "ExternalOutput":
                    name = alloc.memorylocations[0].name
                    outs[name] = np.array(sim.tensor(name))
            results.append(outs)
        class R: pass
        r = R(); r.results = results; r.exec_time_ns = None
        return r
    return run_bass_kernel_spmd(nc, in_maps, core_ids=list(range(N_CORES)),
                                trace=trace)


def kernel(x, edge_index, W, att_src, att_dst, bias):
    global LAST_RESULTS, _LAST_NCS
    x = np.asarray(x, np.float32)
    edge_index = np.asarray(edge_index)
    W = np.asarray(W, np.float32)
    att_src = np.asarray(att_src, np.float32)
    att_dst = np.asarray(att_dst, np.float32)
    bias_np = np.asarray(bias, np.float32)

    N, C_in = x.shape
    C_out = W.shape[1]
    assert C_in == 128 and C_out == 32, (C_in, C_out)
    trace = bool(os.environ.get("GAT_TRACE"))

    loops = np.arange(N, dtype=np.int64)
    src = np.concatenate([edge_index[0].astype(np.int64), loops])
    dst = np.concatenate([edge_index[1].astype(np.int64), loops])

    Nc, prof, nb, rank_of_slot, col_of_slot, final_col, cores = \
        _plan(dst, N, N_CORES)
    nch = -(-Nc // 128)

    # ---- phase 1: project every node once (node-partitioned) ----
    ws = W @ att_src
    wd = W @ att_dst
    wext = np.concatenate([W, ws[:, None], wd[:, None]], 1).astype(NPBF16)
    in1 = []
    for c in range(N_CORES):
        xt = np.zeros((128, nch * 128), NPBF16)
        n0 = c * Nc
        xt[:, :Nc] = x[n0:n0 + Nc].T.astype(NPBF16)
        in1.append({"xt": xt, "wext": wext})

    key1 = ("proj", nch)
    if key1 not in _NC_CACHE:
        _NC_CACHE[key1] = _build_proj(nch)
    nc1 = _NC_CACHE[key1]
    res1 = _run(nc1, in1, trace)

    h_all = np.zeros((N, 32), NPBF16)
    as_all = np.zeros(N, NPBF16)
    ad_all = np.zeros(N, NPBF16)
    for c in range(N_CORES):
        o = res1.results[c]
        h = np.asarray(o["h_out"]).reshape(128, nch, 32) \
            .transpose(1, 0, 2).reshape(nch * 128, 32)[:Nc]
        a = np.asarray(o["a_out"]).reshape(128, nch, 2) \
            .transpose(1, 0, 2).reshape(nch * 128, 2)[:Nc]
        sl = slice(c * Nc, (c + 1) * Nc)
        h_all[sl] = h
        as_all[sl], ad_all[sl] = a[:, 0], a[:, 1]

    # ---- host gather: per-core streams (pure indexing) ----
    h_pad = np.concatenate([h_all, np.zeros((1, 32), NPBF16)], 0)
    as_pad = np.concatenate([as_all, [NPBF16(-300.0)]])
    ad_pad = np.concatenate([ad_all, [NPBF16(-300.0)]])

    z_np = np.broadcast_to(
        col_of_slot.reshape(nb, 128).T.astype(NPBF16), (128, nb)).copy()
    consts = np.zeros((128, D + 33 + 32), NPBF16)
    consts[:, 0:D] = np.arange(D, dtype=np.float32).astype(NPBF16)[None, :]
    idb = np.eye(33, dtype=np.float32)
    idb[32, 0:32] = bias_np          # transpose-matmul folds den*bias in
    consts[0:33, D:D + 33] = idb.astype(NPBF16)
    consts[:, D + 33:] = bias_np.astype(NPBF16)[None, :]

    ncols = nb * D
    ntp = -(-ncols // 128)

    in2 = []
    for c in range(N_CORES):
        src_of_slot, dst_of_slot = _core_slots(
            cores[c], prof, rank_of_slot, len(src), Nc, c)
        sid = np.where(src_of_slot >= 0, src[np.maximum(src_of_slot, 0)], N)
        did = np.where(dst_of_slot >= 0, dst_of_slot, N)
        he = np.empty((128, nb * 32), NPBF16)
        he3 = he.reshape(128, nb, 32)
        hrows = h_pad[sid].reshape(nb, 128, 32)
        he3[:, :, :] = hrows.transpose(1, 0, 2)
        a_st = np.empty((128, 2 * nb), NPBF16)
        a_st[:, 0:nb] = as_pad[sid].reshape(nb, 128).T
        a_st[:, nb:2 * nb] = ad_pad[did].reshape(nb, 128).T
        in2.append({"he": he, "a_st": a_st, "z_st": z_np, "consts": consts})

    key2 = ("agg", nb, LAG, WARM, ACCB, EVAC_DVE, WSPLIT, EQPOOL, ROUNDEND, BIASFOLD, OUT_EVERY)
    if key2 not in _NC_CACHE:
        _NC_CACHE[key2] = _build_agg(nb)
    nc2 = _NC_CACHE[key2]
    res2 = _run(nc2, in2, trace)
    LAST_RESULTS = (res1, res2)
    _LAST_NCS = (nc1, nc2)

    out_full = np.zeros((N, C_out), np.float32)
    fc = final_col                       # rank -> global col id
    tpi = fc // 128                      # transpose tile
    tpp = fc % 128                       # partition within tile
    for c in range(N_CORES):
        o = np.asarray(res2.results[c]["out"]).astype(np.float32)
        o3 = o.reshape(128, ntp, 32)
        vals = o3[tpp, tpi, :]           # [n_ranks, 32]
        perm = cores[c][3]
        out_full[c * Nc + perm] = vals
    return out_full


# revision 9
# speedup vs baseline: 1.0146x; 1.0044x over previous
"""GAT encoder (PyG GATConv-style, single head) for Trainium2, 8 NeuronCores.

v3: segment-sum as PE matmuls with on-device selection masks.

  Phase 1 (proj): node-partitioned x @ [W | W@att_src | W@att_dst] -> per
  node h (32, bf16) and logits a_s, a_d (bf16).  No activations.

  Host (pure indexing): destinations degree-sorted per core against a
  COMMON degree profile (elementwise max across cores, +0.6% slots) so all
  8 cores share one program.  Edge slots laid dst-major into 128-slot
  windows; each window owns D=13 psum columns (col 0 = carry for a dst
  straddling from the previous window, straddler sits in col D-1; carries
  never cross a 39-window psum bank).

  Phase 2 (agg) per core, DMA-bound at ~17MB (h-stream 14.1MB @360B/ns):
    num  = exp(max(a, 0.2a)), a = a_s[src]+a_d[dst]      (DVE + ACT exp)
    W    = num * (z == j) built as D per-plane tensor_scalar/tensor_tensor
           ops (is_equal planes split DVE/Pool, mults DVE at 2x; D-major
           layout keeps every op densely packed -> 2x mode)
    per window w: two matmuls (stationary ldweights is free in the cost
    model; Matmult cost = out free size only):
      psum[0:32, wD:(w+1)D]  = he_w.T @ W_w    (h gathered per edge slot)
      psum[32:33, wD:(w+1)D] = ones.T @ W_w    (den row, aligned tile pos)
    per bank: ACT-copy psum -> acc_sb bf16; Pool merges window carries.
    PE re-matmul [33,128] acc tiles against [identity | bias-row] ->
    transposed dst-major psum tiles with den*bias folded in
    ((acc + den*bias)/den == acc/den + bias), then per-15-tile rounds:
    DVE rcp(den) + multiply, ACT sigmoid, batched out DMA.

  Host unshards via the rank->column map.  bf16 streams, f32 psum
  accumulate; rel err ~9e-3 vs the 2e-2 gate (fp8 h measured 2.9e-2).
"""
import os
import sys

for _p in ('/opt/trn_rl_repo',):
    if _p not in sys.path and os.path.isdir(_p):
        sys.path.insert(0, _p)

import numpy as np
import ml_dtypes

import concourse.mybir as mybir
import concourse.tile as tile
from concourse import bacc
from concourse.bass_utils import run_bass_kernel_spmd

F32 = mybir.dt.float32
BF16 = mybir.dt.bfloat16
NPBF16 = ml_dtypes.bfloat16

N_CORES = 8
PSUM_CHUNK = 15          # proj: 15*34 = 510 <= 512 f32 per PSUM bank
CW = 34                  # projected width: 32 h + a_s + a_d
D = 13                   # psum columns per 128-slot window
BANK_WINS = 39           # windows per psum bank (39*13 = 507 <= 512)
TP_TILES = 15            # [128,33] bf16 tiles per epilogue round, stride 34
HE_CHUNK_BANKS = 2       # he DMA granularity (banks per DMA)
LAG = int(os.environ.get("GAT_LAG", "0"))    # transpose lag (banks); 0 = all at end
ACCB = int(os.environ.get("GAT_ACCB", "6"))  # psum banks for accumulation
EVAC_DVE = int(os.environ.get("GAT_EVACDVE", "0"))  # every Nth evac on DVE (0=ACT only)
WSPLIT = int(os.environ.get("GAT_WSPLIT", "0"))  # Pool planes (0 = j%2 eq split, mults DVE)
EQPOOL = int(os.environ.get("GAT_EQPOOL", "0"))  # eq planes on Pool (only if WSPLIT=0; 0 = j%2)
ROUNDEND = int(os.environ.get("GAT_ROUNDEND", "0"))  # 1 = epilogue rounds after loop
BIASFOLD = int(os.environ.get("GAT_BIASFOLD", "1"))  # 1 = bias via transpose matmul
OUT_EVERY = int(os.environ.get("GAT_OUTEVERY", "4"))  # rounds per out DMA
WARM = int(os.environ.get("GAT_WARM", "0"))  # fine-grained W warmup chunks

LAST_RESULTS = None
_NC_CACHE = {}
_LAST_NCS = ()


def sim_exec_time_ns():
    """Sum of TimelineSim estimates for the programs run by kernel()."""
    from concourse.timeline_sim import TimelineSim
    return int(sum(TimelineSim(nc, trace=False).simulate()
                   for nc in _LAST_NCS))


# ---------------------------------------------------------------- planning
def _profile_plan(prof):
    """Pack the common degree profile into 128-slot windows.

    Protocol: per window, col 0 is reserved for a carry (continuation of the
    previous window's straddling dst), new dsts take cols 1..D-2, and a dst
    that straddles into the next window takes col D-1 (its continuation gets
    col 0 there).  The last window of each 42-window bank is padded so no
    carry crosses a psum bank.

    Returns (nb, rank_of_slot[nb*128], col_of_slot[nb*128],
    final_col[n_ranks] (global col id w*D+col)).
    """
    n = len(prof)
    rank_of_slot = []
    col_of_slot = []
    final_col = np.zeros(n, np.int64)
    w = 0          # current window index
    p = 0          # next free slot in window
    newd = 0       # new dsts started in this window

    def close_window():
        nonlocal w, p, newd
        pad = 128 - p
        rank_of_slot.extend([-1] * pad)
        col_of_slot.extend([0] * pad)
        w += 1
        p = 0
        newd = 0

    for r in range(n):
        d = int(prof[r])
        assert d >= 1
        while True:
            if p >= 128:
                close_window()
                continue
            if newd >= D - 2:
                close_window()
                continue
            space = 128 - p
            straddle = d > space
            if straddle and (w % BANK_WINS) == BANK_WINS - 1:
                # no carry across banks: pad and start in next bank
                close_window()
                continue
            break
        newd += 1
        if straddle:
            rank_of_slot.extend([r] * space)
            col_of_slot.extend([D - 1] * space)
            p = 128
            close_window()
            rest = d - space
            assert rest <= 128
            rank_of_slot.extend([r] * rest)
            col_of_slot.extend([0] * rest)
            p = rest
            final_col[r] = w * D + 0
        else:
            col = newd  # 1..D-2
            rank_of_slot.extend([r] * d)
            col_of_slot.extend([col] * d)
            p += d
            final_col[r] = w * D + col
    if p > 0:
        close_window()
    nb = w
    return (nb, np.array(rank_of_slot, np.int64),
            np.array(col_of_slot, np.int64), final_col)


def _plan(dst, N, n_cores):
    """Common profile + per-core degree-sorted dst orders and edge lists."""
    Nc = N // n_cores
    assert Nc * n_cores == N
    degs = np.zeros((n_cores, Nc), np.int64)
    cores = []
    for c in range(n_cores):
        sel = (dst >= c * Nc) & (dst < (c + 1) * Nc)
        idx = np.nonzero(sel)[0]
        d_c = dst[idx] - c * Nc
        order = np.argsort(d_c, kind='stable')
        eidx_sorted = idx[order]            # edge ids grouped by local dst
        counts = np.bincount(d_c, minlength=Nc).astype(np.int64)
        offsets = np.zeros(Nc + 1, np.int64)
        np.cumsum(counts, out=offsets[1:])
        perm = np.argsort(-counts, kind='stable')   # rank -> local dst
        degs[c] = counts[perm]
        cores.append((eidx_sorted, counts, offsets, perm))
    prof = degs.max(axis=0)
    assert prof[-1] >= 1 and prof[0] <= 128
    nb, rank_of_slot, col_of_slot, final_col = _profile_plan(prof)
    return Nc, prof, nb, rank_of_slot, col_of_slot, final_col, cores


def _core_slots(core_plan, prof, rank_of_slot, n_edges, Nc, c):
    """Per-core (src_of_slot, dst_of_slot) with -1 for padding slots."""
    eidx_sorted, counts, offsets, perm = core_plan
    nslots = len(rank_of_slot)
    src_of_slot = np.full(nslots, -1, np.int64)
    dst_of_slot = np.full(nslots, -1, np.int64)
    # slot positions per rank, in slot order
    pos = np.nonzero(rank_of_slot >= 0)[0]
    rk = rank_of_slot[pos]
    # index of each slot within its rank (0..prof[r]-1), slots of a rank
    # appear in increasing slot order
    order = np.argsort(rk, kind='stable')
    within = np.zeros(len(rk), np.int64)
    cum = np.zeros(len(prof) + 1, np.int64)
    np.cumsum(prof, out=cum[1:])
    within[order] = np.arange(len(rk)) - cum[rk[order]]
    ldst = perm[rk]                        # local dst of each real slot
    k = within
    valid = k < counts[ldst]
    epos = offsets[ldst[valid]] + k[valid]
    src_of_slot[pos[valid]] = -2           # placeholder, filled below
    sv = np.full(len(rk), -1, np.int64)
    sv[valid] = eidx_sorted[epos]
    src_of_slot[pos] = sv                  # edge id per slot (-1 pad)
    dst_of_slot[pos[valid]] = ldst[valid] + c * Nc
    return src_of_slot, dst_of_slot


# ---------------------------------------------------------------- phase 1
def _build_proj(nch):
    nc = bacc.Bacc("TRN2", target_bir_lowering=False, debug=False,
                   num_devices=N_CORES)
    xt = nc.dram_tensor("xt", [128, nch * 128], BF16, kind="ExternalInput").ap()
    wext = nc.dram_tensor("wext", [128, CW], BF16, kind="ExternalInput").ap()
    h_out = nc.dram_tensor("h_out", [128, nch * 32], BF16,
                           kind="ExternalOutput").ap()
    a_out = nc.dram_tensor("a_out", [128, nch * 2], BF16,
                           kind="ExternalOutput").ap()
    N_IN_DMA = 4
    with tile.TileContext(nc) as tc:
        with (
            tc.tile_pool(name="const", bufs=1) as cpool,
            tc.tile_pool(name="ps", bufs=8, space="PSUM") as pspool,
        ):
            wsb = cpool.tile([128, CW], BF16)
            xc = cpool.tile([128, nch * 128], BF16)
            qs = [0]
            left = nch
            for frac in (0.33, 0.33, 0.24, 0.10):
                qs.append(min(nch, qs[-1] + max(1, int(round(nch * frac)))))
            qs[-1] = nch
            for i, (k, k1) in enumerate(zip(qs[:-1], qs[1:])):
                if k1 > k:
                    nc.sync.dma_start(xc[:, k * 128:k1 * 128],
                                      xt[:, k * 128:k1 * 128])
                if i == 0:
                    nc.sync.dma_start(wsb[:], wext[:])
            hsb = cpool.tile([128, nch * 32], BF16)
            asd = cpool.tile([128, nch * 2], BF16)
            n_chunks = -(-nch // PSUM_CHUNK)
            marks = [(n_chunks * 4) // 8, (n_chunks * 6) // 8, n_chunks - 1, n_chunks]
            flush = [0] + sorted(set(min(m * PSUM_CHUNK, nch) for m in marks))
            b0 = 0
            while b0 < nch:
                cn = min(PSUM_CHUNK, nch - b0)
                ps = pspool.tile([128, PSUM_CHUNK * CW], F32, tag="ps")
                for j in range(b0, b0 + cn):
                    nc.tensor.matmul(
                        ps[:, (j - b0) * CW:(j - b0 + 1) * CW],
                        xc[:, j * 128:(j + 1) * 128],
                        wsb[:], start=True, stop=True)
                psv = ps[:, :cn * CW].rearrange("p (s f) -> p s f", f=CW)
                if (b0 // PSUM_CHUNK) % 2 == 0:
                    nc.scalar.activation(
                        hsb[:, b0 * 32:(b0 + cn) * 32]
                        .rearrange("p (s c) -> p s c", c=32),
                        psv[:, :, 0:32],
                        mybir.ActivationFunctionType.Copy)
                else:
                    nc.vector.tensor_copy(
                        out=hsb[:, b0 * 32:(b0 + cn) * 32]
                        .rearrange("p (s c) -> p s c", c=32),
                        in_=psv[:, :, 0:32])
                nc.vector.tensor_copy(
                    out=asd[:, b0 * 2:(b0 + cn) * 2]
                    .rearrange("p (s c) -> p s c", c=2),
                    in_=psv[:, :, 32:34])
                b1 = b0 + cn
                # flush h_out at staged boundaries (earlier pieces bigger)
                for lo, hi in zip(flush[:-1], flush[1:]):
                    if b0 < hi <= b1:
                        if hi == nch:
                            nc.sync.dma_start(a_out[:], asd[:])
                        nc.scalar.dma_start(h_out[:, lo * 32:hi * 32],
                                            hsb[:, lo * 32:hi * 32])
                b0 = b1
    nc.compile()
    return nc


# ---------------------------------------------------------------- phase 2
def _build_agg(nb):
    n_banks = -(-nb // BANK_WINS)
    ncols = nb * D
    ntp = -(-ncols // 128)                # transpose tiles
    nc = bacc.Bacc("TRN2", target_bir_lowering=False, debug=False,
                   num_devices=N_CORES)
    he = nc.dram_tensor("he", [128, nb * 32], BF16, kind="ExternalInput").ap()
    a_st = nc.dram_tensor("a_st", [128, 2 * nb], BF16,
                          kind="ExternalInput").ap()
    z_st = nc.dram_tensor("z_st", [128, nb], BF16, kind="ExternalInput").ap()
    consts = nc.dram_tensor("consts", [128, D + 33 + 32], BF16,
                            kind="ExternalInput").ap()
    out = nc.dram_tensor("out", [128, ntp * 32], BF16,
                         kind="ExternalOutput").ap()
    # epilogue round boundaries (tiles); last rounds smaller for the tail
    bounds = list(range(0, ntp, TP_TILES))
    if len(bounds) >= 2 and ntp - bounds[-1] > 6:
        bounds = bounds[:-1] + [ntp - 12, ntp - 6]
    elif ntp > 6:
        bounds = bounds[:-1] + [max(0, ntp - 6)]
    bounds = sorted(set(b for b in bounds if b < ntp))
    with tile.TileContext(nc) as tc:
        with (
            tc.tile_pool(name="const", bufs=1) as cpool,
            tc.tile_pool(name="hec", bufs=6) as hepool,
            tc.tile_pool(name="acc", bufs=ACCB, space="PSUM") as accpool,
            tc.tile_pool(name="tp", bufs=8 - ACCB, space="PSUM") as tppool,
        ):
            # ---- constants + small streams (sync queue: ordered first)
            cst = cpool.tile([128, D + 33 + 32], BF16)
            nc.sync.dma_start(cst[:], consts[:])
            ident = cst[:, D:D + 33]      # identity in partitions 0..32
            bias_sb = cst[:, D + 33:D + 33 + 32]
            ones_sb = cpool.tile([128, 1], BF16)
            nc.gpsimd.memset(ones_sb[:], 1.0)
            ac = cpool.tile([128, 2 * nb], BF16)
            nc.sync.dma_start(ac[:], a_st[:])
            zc = cpool.tile([128, nb], BF16)
            nc.sync.dma_start(zc[:], z_st[:])
            # ---- num = exp(max(a, 0.2a))  [128, nb]
            num = cpool.tile([128, nb], BF16)
            wk = cpool.tile([128, nb], BF16)
            nc.vector.tensor_tensor(out=wk[:], in0=ac[:, 0:nb],
                                    in1=ac[:, nb:2 * nb],
                                    op=mybir.AluOpType.add)
            nc.vector.tensor_scalar(out=num[:], in0=wk[:], scalar1=0.2,
                                    scalar2=None, op0=mybir.AluOpType.mult)
            nc.vector.tensor_tensor(out=wk[:], in0=wk[:], in1=num[:],
                                    op=mybir.AluOpType.max)
            nc.scalar.activation(num[:], wk[:],
                                 mybir.ActivationFunctionType.Exp, scale=1.0)
            # ---- W[p, j, b] = num[p, b] * (z[p, b] == j), D-major
            wsel = cpool.tile([128, D * nb], BF16)
            w3 = wsel[:].rearrange("p (d b) -> p d b", b=nb)
            NCHUNK = 6
            cb = -(-nb // NCHUNK)
            wstate = {"done": 0, "warm": WARM}

            def emit_w_chunk():
                s0 = wstate["done"]
                if s0 >= nb:
                    return
                if wstate["warm"] > 0:
                    wstate["warm"] -= 1
                    s1 = min(s0 + BANK_WINS, nb)
                else:
                    s1 = min(s0 + cb, nb)
                for j in range(D):
                    if WSPLIT:
                        eng = nc.gpsimd if j >= D - WSPLIT else nc.vector
                        eng.tensor_scalar(
                            out=w3[:, j, s0:s1], in0=zc[:, s0:s1],
                            scalar1=float(j), scalar2=None,
                            op0=mybir.AluOpType.is_equal)
                        eng.tensor_tensor(
                            out=w3[:, j, s0:s1], in0=w3[:, j, s0:s1],
                            in1=num[:, s0:s1], op=mybir.AluOpType.mult)
                    else:
                        if EQPOOL:
                            eng = nc.gpsimd if j < EQPOOL else nc.vector
                        else:
                            eng = nc.vector if j % 2 == 0 else nc.gpsimd
                        eng.tensor_scalar(
                            out=w3[:, j, s0:s1], in0=zc[:, s0:s1],
                            scalar1=float(j), scalar2=None,
                            op0=mybir.AluOpType.is_equal)
                        nc.vector.tensor_tensor(
                            out=w3[:, j, s0:s1], in0=w3[:, j, s0:s1],
                            in1=num[:, s0:s1], op=mybir.AluOpType.mult)
                wstate["done"] = s1

            for _ in range(4):
                emit_w_chunk()
            # ---- streaming accumulate + interleaved epilogue
            acc_sb = cpool.tile([128, ntp * 128], BF16)
            if ntp * 128 > ncols:
                nc.gpsimd.memset(acc_sb[0:33, ncols:ntp * 128], 0.0)
            out_sb = cpool.tile([128, ntp * 32], BF16)
            rcp = cpool.tile([128, ntp], BF16)
            state = {"tile": 0, "round": 0, "odma": []}

            def emit_transposes(bank_done):
                """Emit transposes fully covered by merged banks <= bank_done."""
                max_t = min(ntp, ((bank_done + 1) * BANK_WINS * D) // 128)
                if bank_done >= n_banks - 1:
                    max_t = ntp
                while state["tile"] < max_t:
                    t = state["tile"]
                    r = state["round"]
                    t0 = bounds[r]
                    if r not in state["tps"]:
                        if BIASFOLD:
                            tp_r = tppool.tile([128, TP_TILES * 33], F32,
                                               tag="tp")
                        else:
                            tp_r = tppool.tile([128, TP_TILES * 34], BF16,
                                               tag="tp")
                        state["tps"][r] = tp_r
                    tp = state["tps"][r]
                    if BIASFOLD:
                        # regular matmul against [identity | bias row]:
                        # transposed acc with den*bias folded in
                        # ((acc + den*bias)*rcp == acc*rcp + bias)
                        nc.tensor.matmul(
                            tp[:, (t - t0) * 33:(t - t0) * 33 + 33],
                            acc_sb[0:33, t * 128:(t + 1) * 128],
                            ident[0:33, 0:33],
                            start=True, stop=True)
                    else:
                        nc.tensor.transpose(
                            tp[:, (t - t0) * 34:(t - t0) * 34 + 33],
                            acc_sb[0:33, t * 128:(t + 1) * 128],
                            ident[0:33, 0:33])
                    state["tile"] = t + 1
                    t1 = bounds[r + 1] if r + 1 < len(bounds) else ntp
                    if t + 1 == t1:
                        if not ROUNDEND:
                            emit_round(r, t0, t1, state["tps"][r])
                        state["round"] = r + 1

            def emit_round(r, t0, t1, tp):
                cw = 33 if BIASFOLD else 34
                tpv = tp[:, :(t1 - t0) * cw] \
                    .rearrange("p (t c) -> p t c", c=cw)
                with nc.allow_low_precision(reason="1/den bf16"):
                    nc.vector.reciprocal(rcp[:, t0:t1], tpv[:, :, 32])
                ov = out_sb[:, t0 * 32:t1 * 32] \
                    .rearrange("p (t c) -> p t c", c=32)
                nc.vector.tensor_tensor(
                    out=ov, in0=tpv[:, :, 0:32],
                    in1=rcp[:, t0:t1].rearrange("p (t o) -> p t o", o=1)
                    .to_broadcast([128, t1 - t0, 32]),
                    op=mybir.AluOpType.mult)
                if not BIASFOLD:
                    nc.vector.tensor_tensor(
                        out=ov, in0=ov,
                        in1=bias_sb.rearrange("p (o c) -> p o c", o=1)
                        .to_broadcast([128, t1 - t0, 32]),
                        op=mybir.AluOpType.add)
                nc.scalar.activation(out_sb[:, t0 * 32:t1 * 32],
                                     out_sb[:, t0 * 32:t1 * 32],
                                     mybir.ActivationFunctionType.Sigmoid)
                state["odma"].append((t0, t1))
                flush = (r % OUT_EVERY == OUT_EVERY - 1
                         or t1 >= ntp)
                if flush:
                    o0 = state["odma"][0][0]
                    o1 = state["odma"][-1][1]
                    state["odma"] = []
                    nc.scalar.dma_start(out[:, o0 * 32:o1 * 32],
                                        out_sb[:, o0 * 32:o1 * 32])

            # tp tiles must be allocated per round; pre-wire creation order
            state["tps"] = {}
            for s0 in range(0, nb, HE_CHUNK_BANKS * BANK_WINS):
                s1 = min(s0 + HE_CHUNK_BANKS * BANK_WINS, nb)
                hc = hepool.tile([128, HE_CHUNK_BANKS * BANK_WINS * 32], BF16,
                                 tag="hec")
                nc.sync.dma_start(hc[:, :(s1 - s0) * 32],
                                  he[:, s0 * 32:s1 * 32])
                # keep the on-device W build ~3 he-chunks ahead of the
                # matmul stream so merges queue promptly behind it
                if wstate["done"] < min(nb, s1 + 3 * HE_CHUNK_BANKS * BANK_WINS):
                    emit_w_chunk()
                for b in range(s0 // BANK_WINS,
                               s0 // BANK_WINS + HE_CHUNK_BANKS):
                    if b >= n_banks:
                        break
                    while wstate["done"] < min(nb, (b + 1) * BANK_WINS):
                        emit_w_chunk()
                    w0 = b * BANK_WINS
                    w1 = min(w0 + BANK_WINS, nb)
                    ap = accpool.tile([128, 512], F32, tag="acc")
                    for w in range(w0, min(w0 + BANK_WINS, nb)):
                        lw = w - s0
                        nc.tensor.matmul(
                            ap[0:32, (w - w0) * D:(w - w0 + 1) * D],
                            hc[:, lw * 32:(lw + 1) * 32],
                            w3[:, :, w],
                            start=True, stop=True)
                        nc.tensor.matmul(
                            ap[32:33, (w - w0) * D:(w - w0 + 1) * D],
                            ones_sb[:], w3[:, :, w],
                            start=True, stop=True)
                    # evacuate bank -> acc_sb (mostly ACT; Copy is in
                    # every act table set so no reloads)
                    if EVAC_DVE and b % EVAC_DVE == EVAC_DVE - 1:
                        nc.vector.tensor_copy(
                            out=acc_sb[0:33, w0 * D:w1 * D],
                            in_=ap[0:33, 0:(w1 - w0) * D])
                    else:
                        nc.scalar.activation(
                            acc_sb[0:33, w0 * D:w1 * D],
                            ap[0:33, 0:(w1 - w0) * D],
                            mybir.ActivationFunctionType.Copy)
                    # merge carries within the bank (Pool, sbuf only)
                    if w1 - w0 > 1:
                        a3o = acc_sb[0:33, w0 * D + D:w1 * D] \
                            .rearrange("p (b d) -> p b d", d=D)
                        a3i = acc_sb[0:33, w0 * D + D - 1:w1 * D - 1] \
                            .rearrange("p (b d) -> p b d", d=D)
                        nc.gpsimd.tensor_tensor(
                            out=a3o[:, :, 0:1], in0=a3o[:, :, 0:1],
                            in1=a3i[:, :, 0:1], op=mybir.AluOpType.add)
                    # interleave transposes/epilogue with a lag so their
                    # evac/merge deps are long resolved by the time in-order
                    # PE.SEQ reaches them
                    if LAG > 0 and b >= LAG:
                        emit_transposes(b - LAG)
            emit_transposes(n_banks - 1)
            assert state["tile"] == ntp and state["round"] == len(bounds)
            if ROUNDEND:
                for r, t0 in enumerate(bounds):
                    t1 = bounds[r + 1] if r + 1 < len(bounds) else ntp
                    emit_round(r, t0, t1, state["tps"][r])
    nc.compile()
    return nc


# ---------------------------------------------------------------- runners
def _run(nc, in_maps, trace):
    if os.environ.get("GAT_SIM"):
        from concourse.bass_interp import CoreSim
        results = []
        for m in in_maps:
            sim = CoreSim(nc, require_finite=False, require_nnan=False)
            for k, v in m.items():
                sim.tensor(k)[:] = v
            sim.simulate()
            outs = {}
            for alloc in nc.m.functions[0].allocations:
                if getattr(alloc, 'kind', None) == "ExternalOutput":
                    name = alloc.memorylocations[0].name
                    outs[name] = np.array(sim.tensor(name))
            results.append(outs)
        class R: pass
        r = R(); r.results = results; r.exec_time_ns = None
        return r
    return run_bass_kernel_spmd(nc, in_maps, core_ids=list(range(N_CORES)),
                                trace=trace)


def kernel(x, edge_index, W, att_src, att_dst, bias):
    global LAST_RESULTS, _LAST_NCS
    x = np.asarray(x, np.float32)
    edge_index = np.asarray(edge_index)
    W = np.asarray(W, np.float32)
    att_src = np.asarray(att_src, np.float32)
    att_dst = np.asarray(att_dst, np.float32)
    bias_np = np.asarray(bias, np.float32)

    N, C_in = x.shape
    C_out = W.shape[1]
    assert C_in == 128 and C_out == 32, (C_in, C_out)
    trace = bool(os.environ.get("GAT_TRACE"))

    loops = np.arange(N, dtype=np.int64)
    src = np.concatenate([edge_index[0].astype(np.int64), loops])
    dst = np.concatenate([edge_index[1].astype(np.int64), loops])

    Nc, prof, nb, rank_of_slot, col_of_slot, final_col, cores = \
        _plan(dst, N, N_CORES)
    nch = -(-Nc // 128)

    # ---- phase 1: project every node once (node-partitioned) ----
    ws = W @ att_src
    wd = W @ att_dst
    wext = np.concatenate([W, ws[:, None], wd[:, None]], 1).astype(NPBF16)
    in1 = []
    for c in range(N_CORES):
        xt = np.zeros((128, nch * 128), NPBF16)
        n0 = c * Nc
        xt[:, :Nc] = x[n0:n0 + Nc].T.astype(NPBF16)
        in1.append({"xt": xt, "wext": wext})

    key1 = ("proj", nch)
    if key1 not in _NC_CACHE:
        _NC_CACHE[key1] = _build_proj(nch)
    nc1 = _NC_CACHE[key1]
    res1 = _run(nc1, in1, trace)

    h_all = np.zeros((N, 32), NPBF16)
    as_all = np.zeros(N, NPBF16)
    ad_all = np.zeros(N, NPBF16)
    for c in range(N_CORES):
        o = res1.results[c]
        h = np.asarray(o["h_out"]).reshape(128, nch, 32) \
            .transpose(1, 0, 2).reshape(nch * 128, 32)[:Nc]
        a = np.asarray(o["a_out"]).reshape(128, nch, 2) \
            .transpose(1, 0, 2).reshape(nch * 128, 2)[:Nc]
        sl = slice(c * Nc, (c + 1) * Nc)
        h_all[sl] = h
        as_all[sl], ad_all[sl] = a[:, 0], a[:, 1]

    # ---- host gather: per-core streams (pure indexing) ----
    h_pad = np.concatenate([h_all, np.zeros((1, 32), NPBF16)], 0)
    as_pad = np.concatenate([as_all, [NPBF16(-300.0)]])
    ad_pad = np.concatenate([ad_all, [NPBF16(-300.0)]])

    z_np = np.broadcast_to(
        col_of_slot.reshape(nb, 128).T.astype(NPBF16), (128, nb)).copy()
    consts = np.zeros((128, D + 33 + 32), NPBF16)
    consts[:, 0:D] = np.arange(D, dtype=np.float32).astype(NPBF16)[None, :]
    idb = np.eye(33, dtype=np.float32)
    idb[32, 0:32] = bias_np          # transpose-matmul folds den*bias in
    consts[0:33, D:D + 33] = idb.astype(NPBF16)
    consts[:, D + 33:] = bias_np.astype(NPBF16)[None, :]

    ncols = nb * D
    ntp = -(-ncols // 128)

    in2 = []
    for c in range(N_CORES):
        src_of_slot, dst_of_slot = _core_slots(
            cores[c], prof, rank_of_slot, len(src), Nc, c)
        sid = np.where(src_of_slot >= 0, src[np.maximum(src_of_slot, 0)], N)
        did = np.where(dst_of_slot >= 0, dst_of_slot, N)
        he = np.empty((128, nb * 32), NPBF16)
        he3 = he.reshape(128, nb, 32)
        hrows = h_pad[sid].reshape(nb, 128, 32)
        he3[:, :, :] = hrows.transpose(1, 0, 2)
        a_st = np.empty((128, 2 * nb), NPBF16)
        a_st[:, 0:nb] = as_pad[sid].reshape(nb, 128).T
        a_st[:, nb:2 * nb] = ad_pad[did].reshape(nb, 128).T
        in2.append({"he": he, "a_st": a_st, "z_st": z_np, "consts": consts})

    key2 = ("agg", nb, LAG, WARM, ACCB, EVAC_DVE, WSPLIT, EQPOOL, ROUNDEND, BIASFOLD, OUT_EVERY)
    if key2 not in _NC_CACHE:
        _NC_CACHE[key2] = _build_agg(nb)
    nc2 = _NC_CACHE[key2]
    res2 = _run(nc2, in2, trace)
    LAST_RESULTS = (res1, res2)
    _LAST_NCS = (nc1, nc2)

    out_full = np.zeros((N, C_out), np.float32)
    fc = final_col                       # rank -> global col id
    tpi = fc // 128                      # transpose tile
    tpp = fc % 128                       # partition within tile
    for c in range(N_CORES):
        o = np.asarray(res2.results[c]["out"]).astype(np.float32)
        o3 = o.reshape(128, ntp, 32)
        vals = o3[tpp, tpi, :]           # [n_ranks, 32]
        perm = cores[c][3]
        out_full[c * Nc + perm] = vals
    return out_full


# revision 10
# speedup vs baseline: 1.0199x; 1.0052x over previous
"""GAT encoder (PyG GATConv-style, single head) for Trainium2, 8 NeuronCores.

v3: segment-sum as PE matmuls with on-device selection masks.

  Phase 1 (proj): node-partitioned x @ [W | W@att_src | W@att_dst] -> per
  node h (32, bf16) and logits a_s, a_d (bf16).  No activations.

  Host (pure indexing): destinations degree-sorted per core against a
  COMMON degree profile (elementwise max across cores, +0.6% slots) so all
  8 cores share one program.  Edge slots laid dst-major into 128-slot
  windows; each window owns D=13 psum columns (col 0 = carry for a dst
  straddling from the previous window, straddler sits in col D-1; carries
  never cross a 39-window psum bank).

  Phase 2 (agg) per core, DMA-bound at ~17MB (h-stream 14.1MB @360B/ns):
    num  = exp(max(a, 0.2a)), a = a_s[src]+a_d[dst]      (DVE + ACT exp)
    W    = num * (z == j) built as D per-plane tensor_scalar/tensor_tensor
           ops (is_equal planes split DVE/Pool, mults DVE at 2x; D-major
           layout keeps every op densely packed -> 2x mode)
    per window w: two matmuls (stationary ldweights is free in the cost
    model; Matmult cost = out free size only):
      psum[0:32, wD:(w+1)D]  = he_w.T @ W_w    (h gathered per edge slot)
      psum[32:33, wD:(w+1)D] = ones.T @ W_w    (den row, aligned tile pos)
    per bank: ACT-copy psum -> acc_sb bf16; Pool merges window carries.
    PE re-matmul [33,128] acc tiles against [identity | bias-row] ->
    transposed dst-major psum tiles with den*bias folded in
    ((acc + den*bias)/den == acc/den + bias), then per-15-tile rounds:
    DVE rcp(den) + multiply, ACT sigmoid, batched out DMA.

  Host unshards via the rank->column map.  bf16 streams, f32 psum
  accumulate; rel err ~9e-3 vs the 2e-2 gate (fp8 h measured 2.9e-2).
"""
import os
import sys

for _p in ('/opt/trn_rl_repo',):
    if _p not in sys.path and os.path.isdir(_p):
        sys.path.insert(0, _p)

import numpy as np
import ml_dtypes

import concourse.mybir as mybir
import concourse.tile as tile
from concourse import bacc
from concourse.bass_utils import run_bass_kernel_spmd

F32 = mybir.dt.float32
BF16 = mybir.dt.bfloat16
NPBF16 = ml_dtypes.bfloat16

N_CORES = 8
PSUM_CHUNK = 15          # proj: 15*34 = 510 <= 512 f32 per PSUM bank
CW = 34                  # projected width: 32 h + a_s + a_d
D = 13                   # psum columns per 128-slot window
BANK_WINS = 39           # windows per psum bank (39*13 = 507 <= 512)
TP_TILES = 15            # [128,33] bf16 tiles per epilogue round, stride 34
HE_CHUNK_BANKS = 2       # he DMA granularity (banks per DMA)
LAG = int(os.environ.get("GAT_LAG", "0"))    # transpose lag (banks); 0 = all at end
ACCB = int(os.environ.get("GAT_ACCB", "6"))  # psum banks for accumulation
EVAC_DVE = int(os.environ.get("GAT_EVACDVE", "0"))  # every Nth evac on DVE (0=ACT only)
WSPLIT = int(os.environ.get("GAT_WSPLIT", "0"))  # Pool planes (0 = j%2 eq split, mults DVE)
EQPOOL = int(os.environ.get("GAT_EQPOOL", "0"))  # eq planes on Pool (only if WSPLIT=0; 0 = j%2)
ROUNDEND = int(os.environ.get("GAT_ROUNDEND", "0"))  # 1 = epilogue rounds after loop
BIASFOLD = int(os.environ.get("GAT_BIASFOLD", "1"))  # 1 = bias via transpose matmul
OUT_EVERY = int(os.environ.get("GAT_OUTEVERY", "4"))  # rounds per out DMA
WARM = int(os.environ.get("GAT_WARM", "0"))  # fine-grained W warmup chunks
HEBUFS = int(os.environ.get("GAT_HEBUFS", "6"))  # he stream buffers

LAST_RESULTS = None
_NC_CACHE = {}
_LAST_NCS = ()


def sim_exec_time_ns():
    """Sum of TimelineSim estimates for the programs run by kernel()."""
    from concourse.timeline_sim import TimelineSim
    return int(sum(TimelineSim(nc, trace=False).simulate()
                   for nc in _LAST_NCS))


# ---------------------------------------------------------------- planning
def _profile_plan(prof):
    """Pack the common degree profile into 128-slot windows.

    Protocol: per window, col 0 is reserved for a carry (continuation of the
    previous window's straddling dst), new dsts take cols 1..D-2, and a dst
    that straddles into the next window takes col D-1 (its continuation gets
    col 0 there).  The last window of each 42-window bank is padded so no
    carry crosses a psum bank.

    Returns (nb, rank_of_slot[nb*128], col_of_slot[nb*128],
    final_col[n_ranks] (global col id w*D+col)).
    """
    n = len(prof)
    rank_of_slot = []
    col_of_slot = []
    final_col = np.zeros(n, np.int64)
    w = 0          # current window index
    p = 0          # next free slot in window
    newd = 0       # new dsts started in this window

    def close_window():
        nonlocal w, p, newd
        pad = 128 - p
        rank_of_slot.extend([-1] * pad)
        col_of_slot.extend([0] * pad)
        w += 1
        p = 0
        newd = 0

    # visit ranks big/small interleaved so the new-dst cap (D-2 per
    # window) never closes a half-empty window in the small-degree tail
    lo, hi = 0, n - 1
    visit = []
    flip = True
    while lo <= hi:
        if flip:
            visit.append(lo); lo += 1
        else:
            visit.append(hi); hi -= 1
        flip = not flip
    for r in visit:
        d = int(prof[r])
        assert d >= 1
        while True:
            if p >= 128:
                close_window()
                continue
            if newd >= D - 2:
                close_window()
                continue
            space = 128 - p
            straddle = d > space
            if straddle and (w % BANK_WINS) == BANK_WINS - 1:
                # no carry across banks: pad and start in next bank
                close_window()
                continue
            break
        newd += 1
        if straddle:
            rank_of_slot.extend([r] * space)
            col_of_slot.extend([D - 1] * space)
            p = 128
            close_window()
            rest = d - space
            assert rest <= 128
            rank_of_slot.extend([r] * rest)
            col_of_slot.extend([0] * rest)
            p = rest
            final_col[r] = w * D + 0
        else:
            col = newd  # 1..D-2
            rank_of_slot.extend([r] * d)
            col_of_slot.extend([col] * d)
            p += d
            final_col[r] = w * D + col
    if p > 0:
        close_window()
    nb = w
    return (nb, np.array(rank_of_slot, np.int64),
            np.array(col_of_slot, np.int64), final_col)


def _plan(dst, N, n_cores):
    """Common profile + per-core degree-sorted dst orders and edge lists."""
    Nc = N // n_cores
    assert Nc * n_cores == N
    degs = np.zeros((n_cores, Nc), np.int64)
    cores = []
    for c in range(n_cores):
        sel = (dst >= c * Nc) & (dst < (c + 1) * Nc)
        idx = np.nonzero(sel)[0]
        d_c = dst[idx] - c * Nc
        order = np.argsort(d_c, kind='stable')
        eidx_sorted = idx[order]            # edge ids grouped by local dst
        counts = np.bincount(d_c, minlength=Nc).astype(np.int64)
        offsets = np.zeros(Nc + 1, np.int64)
        np.cumsum(counts, out=offsets[1:])
        perm = np.argsort(-counts, kind='stable')   # rank -> local dst
        degs[c] = counts[perm]
        cores.append((eidx_sorted, counts, offsets, perm))
    prof = degs.max(axis=0)
    assert prof[-1] >= 1 and prof[0] <= 128
    nb, rank_of_slot, col_of_slot, final_col = _profile_plan(prof)
    return Nc, prof, nb, rank_of_slot, col_of_slot, final_col, cores


def _core_slots(core_plan, prof, rank_of_slot, n_edges, Nc, c):
    """Per-core (src_of_slot, dst_of_slot) with -1 for padding slots."""
    eidx_sorted, counts, offsets, perm = core_plan
    nslots = len(rank_of_slot)
    src_of_slot = np.full(nslots, -1, np.int64)
    dst_of_slot = np.full(nslots, -1, np.int64)
    # slot positions per rank, in slot order
    pos = np.nonzero(rank_of_slot >= 0)[0]
    rk = rank_of_slot[pos]
    # index of each slot within its rank (0..prof[r]-1), slots of a rank
    # appear in increasing slot order
    order = np.argsort(rk, kind='stable')
    within = np.zeros(len(rk), np.int64)
    cum = np.zeros(len(prof) + 1, np.int64)
    np.cumsum(prof, out=cum[1:])
    within[order] = np.arange(len(rk)) - cum[rk[order]]
    ldst = perm[rk]                        # local dst of each real slot
    k = within
    valid = k < counts[ldst]
    epos = offsets[ldst[valid]] + k[valid]
    src_of_slot[pos[valid]] = -2           # placeholder, filled below
    sv = np.full(len(rk), -1, np.int64)
    sv[valid] = eidx_sorted[epos]
    src_of_slot[pos] = sv                  # edge id per slot (-1 pad)
    dst_of_slot[pos[valid]] = ldst[valid] + c * Nc
    return src_of_slot, dst_of_slot


# ---------------------------------------------------------------- phase 1
def _build_proj(nch):
    nc = bacc.Bacc("TRN2", target_bir_lowering=False, debug=False,
                   num_devices=N_CORES)
    xt = nc.dram_tensor("xt", [128, nch * 128], BF16, kind="ExternalInput").ap()
    wext = nc.dram_tensor("wext", [128, CW], BF16, kind="ExternalInput").ap()
    h_out = nc.dram_tensor("h_out", [128, nch * 32], BF16,
                           kind="ExternalOutput").ap()
    a_out = nc.dram_tensor("a_out", [128, nch * 2], BF16,
                           kind="ExternalOutput").ap()
    N_IN_DMA = 4
    with tile.TileContext(nc) as tc:
        with (
            tc.tile_pool(name="const", bufs=1) as cpool,
            tc.tile_pool(name="ps", bufs=8, space="PSUM") as pspool,
        ):
            wsb = cpool.tile([128, CW], BF16)
            xc = cpool.tile([128, nch * 128], BF16)
            qs = [0]
            left = nch
            for frac in (0.33, 0.33, 0.24, 0.10):
                qs.append(min(nch, qs[-1] + max(1, int(round(nch * frac)))))
            qs[-1] = nch
            for i, (k, k1) in enumerate(zip(qs[:-1], qs[1:])):
                if k1 > k:
                    nc.sync.dma_start(xc[:, k * 128:k1 * 128],
                                      xt[:, k * 128:k1 * 128])
                if i == 0:
                    nc.sync.dma_start(wsb[:], wext[:])
            hsb = cpool.tile([128, nch * 32], BF16)
            asd = cpool.tile([128, nch * 2], BF16)
            n_chunks = -(-nch // PSUM_CHUNK)
            marks = [(n_chunks * 4) // 8, (n_chunks * 6) // 8, n_chunks - 1, n_chunks]
            flush = [0] + sorted(set(min(m * PSUM_CHUNK, nch) for m in marks))
            b0 = 0
            while b0 < nch:
                cn = min(PSUM_CHUNK, nch - b0)
                ps = pspool.tile([128, PSUM_CHUNK * CW], F32, tag="ps")
                for j in range(b0, b0 + cn):
                    nc.tensor.matmul(
                        ps[:, (j - b0) * CW:(j - b0 + 1) * CW],
                        xc[:, j * 128:(j + 1) * 128],
                        wsb[:], start=True, stop=True)
                psv = ps[:, :cn * CW].rearrange("p (s f) -> p s f", f=CW)
                if (b0 // PSUM_CHUNK) % 2 == 0:
                    nc.scalar.activation(
                        hsb[:, b0 * 32:(b0 + cn) * 32]
                        .rearrange("p (s c) -> p s c", c=32),
                        psv[:, :, 0:32],
                        mybir.ActivationFunctionType.Copy)
                else:
                    nc.vector.tensor_copy(
                        out=hsb[:, b0 * 32:(b0 + cn) * 32]
                        .rearrange("p (s c) -> p s c", c=32),
                        in_=psv[:, :, 0:32])
                nc.vector.tensor_copy(
                    out=asd[:, b0 * 2:(b0 + cn) * 2]
                    .rearrange("p (s c) -> p s c", c=2),
                    in_=psv[:, :, 32:34])
                b1 = b0 + cn
                # flush h_out at staged boundaries (earlier pieces bigger)
                for lo, hi in zip(flush[:-1], flush[1:]):
                    if b0 < hi <= b1:
                        if hi == nch:
                            nc.sync.dma_start(a_out[:], asd[:])
                        nc.scalar.dma_start(h_out[:, lo * 32:hi * 32],
                                            hsb[:, lo * 32:hi * 32])
                b0 = b1
    nc.compile()
    return nc


# ---------------------------------------------------------------- phase 2
def _build_agg(nb):
    n_banks = -(-nb // BANK_WINS)
    ncols = nb * D
    ntp = -(-ncols // 128)                # transpose tiles
    nc = bacc.Bacc("TRN2", target_bir_lowering=False, debug=False,
                   num_devices=N_CORES)
    he = nc.dram_tensor("he", [128, nb * 32], BF16, kind="ExternalInput").ap()
    a_st = nc.dram_tensor("a_st", [128, 2 * nb], BF16,
                          kind="ExternalInput").ap()
    z_st = nc.dram_tensor("z_st", [128, nb], BF16, kind="ExternalInput").ap()
    consts = nc.dram_tensor("consts", [128, D + 33 + 32], BF16,
                            kind="ExternalInput").ap()
    out = nc.dram_tensor("out", [128, ntp * 32], BF16,
                         kind="ExternalOutput").ap()
    # epilogue round boundaries (tiles); last rounds smaller for the tail
    bounds = list(range(0, ntp, TP_TILES))
    if len(bounds) >= 2 and ntp - bounds[-1] > 6:
        bounds = bounds[:-1] + [ntp - 12, ntp - 6]
    elif ntp > 6:
        bounds = bounds[:-1] + [max(0, ntp - 6)]
    bounds = sorted(set(b for b in bounds if b < ntp))
    with tile.TileContext(nc) as tc:
        with (
            tc.tile_pool(name="const", bufs=1) as cpool,
            tc.tile_pool(name="hec", bufs=HEBUFS) as hepool,
            tc.tile_pool(name="acc", bufs=ACCB, space="PSUM") as accpool,
            tc.tile_pool(name="tp", bufs=8 - ACCB, space="PSUM") as tppool,
        ):
            # ---- constants + small streams (sync queue: ordered first)
            cst = cpool.tile([128, D + 33 + 32], BF16)
            nc.sync.dma_start(cst[:], consts[:])
            ident = cst[:, D:D + 33]      # identity in partitions 0..32
            bias_sb = cst[:, D + 33:D + 33 + 32]
            ones_sb = cpool.tile([128, 1], BF16)
            nc.gpsimd.memset(ones_sb[:], 1.0)
            ac = cpool.tile([128, 2 * nb], BF16)
            nc.sync.dma_start(ac[:], a_st[:])
            zc = cpool.tile([128, nb], BF16)
            nc.sync.dma_start(zc[:], z_st[:])
            # ---- num = exp(max(a, 0.2a))  [128, nb]
            num = cpool.tile([128, nb], BF16)
            wk = cpool.tile([128, nb], BF16)
            nc.vector.tensor_tensor(out=wk[:], in0=ac[:, 0:nb],
                                    in1=ac[:, nb:2 * nb],
                                    op=mybir.AluOpType.add)
            nc.vector.tensor_scalar(out=num[:], in0=wk[:], scalar1=0.2,
                                    scalar2=None, op0=mybir.AluOpType.mult)
            nc.vector.tensor_tensor(out=wk[:], in0=wk[:], in1=num[:],
                                    op=mybir.AluOpType.max)
            nc.scalar.activation(num[:], wk[:],
                                 mybir.ActivationFunctionType.Exp, scale=1.0)
            # ---- W[p, j, b] = num[p, b] * (z[p, b] == j), D-major
            wsel = cpool.tile([128, D * nb], BF16)
            w3 = wsel[:].rearrange("p (d b) -> p d b", b=nb)
            NCHUNK = 6
            cb = -(-nb // NCHUNK)
            wstate = {"done": 0, "warm": WARM}

            def emit_w_chunk():
                s0 = wstate["done"]
                if s0 >= nb:
                    return
                if wstate["warm"] > 0:
                    wstate["warm"] -= 1
                    s1 = min(s0 + BANK_WINS, nb)
                else:
                    s1 = min(s0 + cb, nb)
                for j in range(D):
                    if WSPLIT:
                        eng = nc.gpsimd if j >= D - WSPLIT else nc.vector
                        eng.tensor_scalar(
                            out=w3[:, j, s0:s1], in0=zc[:, s0:s1],
                            scalar1=float(j), scalar2=None,
                            op0=mybir.AluOpType.is_equal)
                        eng.tensor_tensor(
                            out=w3[:, j, s0:s1], in0=w3[:, j, s0:s1],
                            in1=num[:, s0:s1], op=mybir.AluOpType.mult)
                    else:
                        if EQPOOL:
                            eng = nc.gpsimd if j < EQPOOL else nc.vector
                        else:
                            eng = nc.vector if j % 2 == 0 else nc.gpsimd
                        eng.tensor_scalar(
                            out=w3[:, j, s0:s1], in0=zc[:, s0:s1],
                            scalar1=float(j), scalar2=None,
                            op0=mybir.AluOpType.is_equal)
                        nc.vector.tensor_tensor(
                            out=w3[:, j, s0:s1], in0=w3[:, j, s0:s1],
                            in1=num[:, s0:s1], op=mybir.AluOpType.mult)
                wstate["done"] = s1

            for _ in range(4):
                emit_w_chunk()
            # ---- streaming accumulate + interleaved epilogue
            acc_sb = cpool.tile([128, ntp * 128], BF16)
            if ntp * 128 > ncols:
                nc.gpsimd.memset(acc_sb[0:33, ncols:ntp * 128], 0.0)
            out_sb = cpool.tile([128, ntp * 32], BF16)
            rcp = cpool.tile([128, ntp], BF16)
            state = {"tile": 0, "round": 0, "odma": []}

            def emit_transposes(bank_done):
                """Emit transposes fully covered by merged banks <= bank_done."""
                max_t = min(ntp, ((bank_done + 1) * BANK_WINS * D) // 128)
                if bank_done >= n_banks - 1:
                    max_t = ntp
                while state["tile"] < max_t:
                    t = state["tile"]
                    r = state["round"]
                    t0 = bounds[r]
                    if r not in state["tps"]:
                        if BIASFOLD:
                            tp_r = tppool.tile([128, TP_TILES * 33], F32,
                                               tag="tp")
                        else:
                            tp_r = tppool.tile([128, TP_TILES * 34], BF16,
                                               tag="tp")
                        state["tps"][r] = tp_r
                    tp = state["tps"][r]
                    if BIASFOLD:
                        # regular matmul against [identity | bias row]:
                        # transposed acc with den*bias folded in
                        # ((acc + den*bias)*rcp == acc*rcp + bias)
                        nc.tensor.matmul(
                            tp[:, (t - t0) * 33:(t - t0) * 33 + 33],
                            acc_sb[0:33, t * 128:(t + 1) * 128],
                            ident[0:33, 0:33],
                            start=True, stop=True)
                    else:
                        nc.tensor.transpose(
                            tp[:, (t - t0) * 34:(t - t0) * 34 + 33],
                            acc_sb[0:33, t * 128:(t + 1) * 128],
                            ident[0:33, 0:33])
                    state["tile"] = t + 1
                    t1 = bounds[r + 1] if r + 1 < len(bounds) else ntp
                    if t + 1 == t1:
                        if not ROUNDEND:
                            emit_round(r, t0, t1, state["tps"][r])
                        state["round"] = r + 1

            def emit_round(r, t0, t1, tp):
                cw = 33 if BIASFOLD else 34
                tpv = tp[:, :(t1 - t0) * cw] \
                    .rearrange("p (t c) -> p t c", c=cw)
                with nc.allow_low_precision(reason="1/den bf16"):
                    nc.vector.reciprocal(rcp[:, t0:t1], tpv[:, :, 32])
                ov = out_sb[:, t0 * 32:t1 * 32] \
                    .rearrange("p (t c) -> p t c", c=32)
                nc.vector.tensor_tensor(
                    out=ov, in0=tpv[:, :, 0:32],
                    in1=rcp[:, t0:t1].rearrange("p (t o) -> p t o", o=1)
                    .to_broadcast([128, t1 - t0, 32]),
                    op=mybir.AluOpType.mult)
                if not BIASFOLD:
                    nc.vector.tensor_tensor(
                        out=ov, in0=ov,
                        in1=bias_sb.rearrange("p (o c) -> p o c", o=1)
                        .to_broadcast([128, t1 - t0, 32]),
                        op=mybir.AluOpType.add)
                nc.scalar.activation(out_sb[:, t0 * 32:t1 * 32],
                                     out_sb[:, t0 * 32:t1 * 32],
                                     mybir.ActivationFunctionType.Sigmoid)
                state["odma"].append((t0, t1))
                flush = (r % OUT_EVERY == OUT_EVERY - 1
                         or t1 >= ntp)
                if flush:
                    o0 = state["odma"][0][0]
                    o1 = state["odma"][-1][1]
                    state["odma"] = []
                    nc.scalar.dma_start(out[:, o0 * 32:o1 * 32],
                                        out_sb[:, o0 * 32:o1 * 32])

            # tp tiles must be allocated per round; pre-wire creation order
            state["tps"] = {}
            for s0 in range(0, nb, HE_CHUNK_BANKS * BANK_WINS):
                s1 = min(s0 + HE_CHUNK_BANKS * BANK_WINS, nb)
                hc = hepool.tile([128, HE_CHUNK_BANKS * BANK_WINS * 32], BF16,
                                 tag="hec")
                nc.sync.dma_start(hc[:, :(s1 - s0) * 32],
                                  he[:, s0 * 32:s1 * 32])
                # keep the on-device W build ~3 he-chunks ahead of the
                # matmul stream so merges queue promptly behind it
                if wstate["done"] < min(nb, s1 + 3 * HE_CHUNK_BANKS * BANK_WINS):
                    emit_w_chunk()
                for b in range(s0 // BANK_WINS,
                               s0 // BANK_WINS + HE_CHUNK_BANKS):
                    if b >= n_banks:
                        break
                    while wstate["done"] < min(nb, (b + 1) * BANK_WINS):
                        emit_w_chunk()
                    w0 = b * BANK_WINS
                    w1 = min(w0 + BANK_WINS, nb)
                    ap = accpool.tile([128, 512], F32, tag="acc")
                    for w in range(w0, min(w0 + BANK_WINS, nb)):
                        lw = w - s0
                        nc.tensor.matmul(
                            ap[0:32, (w - w0) * D:(w - w0 + 1) * D],
                            hc[:, lw * 32:(lw + 1) * 32],
                            w3[:, :, w],
                            start=True, stop=True)
                        nc.tensor.matmul(
                            ap[32:33, (w - w0) * D:(w - w0 + 1) * D],
                            ones_sb[:], w3[:, :, w],
                            start=True, stop=True)
                    # evacuate bank -> acc_sb (mostly ACT; Copy is in
                    # every act table set so no reloads)
                    if EVAC_DVE and b % EVAC_DVE == EVAC_DVE - 1:
                        nc.vector.tensor_copy(
                            out=acc_sb[0:33, w0 * D:w1 * D],
                            in_=ap[0:33, 0:(w1 - w0) * D])
                    else:
                        nc.scalar.activation(
                            acc_sb[0:33, w0 * D:w1 * D],
                            ap[0:33, 0:(w1 - w0) * D],
                            mybir.ActivationFunctionType.Copy)
                    # merge carries within the bank (Pool, sbuf only)
                    if w1 - w0 > 1:
                        a3o = acc_sb[0:33, w0 * D + D:w1 * D] \
                            .rearrange("p (b d) -> p b d", d=D)
                        a3i = acc_sb[0:33, w0 * D + D - 1:w1 * D - 1] \
                            .rearrange("p (b d) -> p b d", d=D)
                        nc.gpsimd.tensor_tensor(
                            out=a3o[:, :, 0:1], in0=a3o[:, :, 0:1],
                            in1=a3i[:, :, 0:1], op=mybir.AluOpType.add)
                    # interleave transposes/epilogue with a lag so their
                    # evac/merge deps are long resolved by the time in-order
                    # PE.SEQ reaches them
                    if LAG > 0 and b >= LAG:
                        emit_transposes(b - LAG)
            emit_transposes(n_banks - 1)
            assert state["tile"] == ntp and state["round"] == len(bounds)
            if ROUNDEND:
                for r, t0 in enumerate(bounds):
                    t1 = bounds[r + 1] if r + 1 < len(bounds) else ntp
                    emit_round(r, t0, t1, state["tps"][r])
    nc.compile()
    return nc


# ---------------------------------------------------------------- runners
def _run(nc, in_maps, trace):
    if os.environ.get("GAT_SIM"):
        from concourse.bass_interp import CoreSim
        results = []
        for m in in_maps:
            sim = CoreSim(nc, require_finite=False, require_nnan=False)
            for k, v in m.items():
                sim.tensor(k)[:] = v
            sim.simulate()
            outs = {}
            for alloc in nc.m.functions[0].allocations:
                if getattr(alloc, 'kind', None) == "ExternalOutput":
                    name = alloc.memorylocations[0].name
                    outs[name] = np.array(sim.tensor(name))
            results.append(outs)
        class R: pass
        r = R(); r.results = results; r.exec_time_ns = None
        return r
    return run_bass_kernel_spmd(nc, in_maps, core_ids=list(range(N_CORES)),
                                trace=trace)


def kernel(x, edge_index, W, att_src, att_dst, bias):
    global LAST_RESULTS, _LAST_NCS
    x = np.asarray(x, np.float32)
    edge_index = np.asarray(edge_index)
    W = np.asarray(W, np.float32)
    att_src = np.asarray(att_src, np.float32)
    att_dst = np.asarray(att_dst, np.float32)
    bias_np = np.asarray(bias, np.float32)

    N, C_in = x.shape
    C_out = W.shape[1]
    assert C_in == 128 and C_out == 32, (C_in, C_out)
    trace = bool(os.environ.get("GAT_TRACE"))

    loops = np.arange(N, dtype=np.int64)
    src = np.concatenate([edge_index[0].astype(np.int64), loops])
    dst = np.concatenate([edge_index[1].astype(np.int64), loops])

    Nc, prof, nb, rank_of_slot, col_of_slot, final_col, cores = \
        _plan(dst, N, N_CORES)
    nch = -(-Nc // 128)

    # ---- phase 1: project every node once (node-partitioned) ----
    ws = W @ att_src
    wd = W @ att_dst
    wext = np.concatenate([W, ws[:, None], wd[:, None]], 1).astype(NPBF16)
    in1 = []
    for c in range(N_CORES):
        xt = np.zeros((128, nch * 128), NPBF16)
        n0 = c * Nc
        xt[:, :Nc] = x[n0:n0 + Nc].T.astype(NPBF16)
        in1.append({"xt": xt, "wext": wext})

    key1 = ("proj", nch)
    if key1 not in _NC_CACHE:
        _NC_CACHE[key1] = _build_proj(nch)
    nc1 = _NC_CACHE[key1]
    res1 = _run(nc1, in1, trace)

    h_all = np.zeros((N, 32), NPBF16)
    as_all = np.zeros(N, NPBF16)
    ad_all = np.zeros(N, NPBF16)
    for c in range(N_CORES):
        o = res1.results[c]
        h = np.asarray(o["h_out"]).reshape(128, nch, 32) \
            .transpose(1, 0, 2).reshape(nch * 128, 32)[:Nc]
        a = np.asarray(o["a_out"]).reshape(128, nch, 2) \
            .transpose(1, 0, 2).reshape(nch * 128, 2)[:Nc]
        sl = slice(c * Nc, (c + 1) * Nc)
        h_all[sl] = h
        as_all[sl], ad_all[sl] = a[:, 0], a[:, 1]

    # ---- host gather: per-core streams (pure indexing) ----
    h_pad = np.concatenate([h_all, np.zeros((1, 32), NPBF16)], 0)
    as_pad = np.concatenate([as_all, [NPBF16(-300.0)]])
    ad_pad = np.concatenate([ad_all, [NPBF16(-300.0)]])

    z_np = np.broadcast_to(
        col_of_slot.reshape(nb, 128).T.astype(NPBF16), (128, nb)).copy()
    consts = np.zeros((128, D + 33 + 32), NPBF16)
    consts[:, 0:D] = np.arange(D, dtype=np.float32).astype(NPBF16)[None, :]
    idb = np.eye(33, dtype=np.float32)
    idb[32, 0:32] = bias_np          # transpose-matmul folds den*bias in
    consts[0:33, D:D + 33] = idb.astype(NPBF16)
    consts[:, D + 33:] = bias_np.astype(NPBF16)[None, :]

    ncols = nb * D
    ntp = -(-ncols // 128)

    in2 = []
    for c in range(N_CORES):
        src_of_slot, dst_of_slot = _core_slots(
            cores[c], prof, rank_of_slot, len(src), Nc, c)
        sid = np.where(src_of_slot >= 0, src[np.maximum(src_of_slot, 0)], N)
        did = np.where(dst_of_slot >= 0, dst_of_slot, N)
        he = np.empty((128, nb * 32), NPBF16)
        he3 = he.reshape(128, nb, 32)
        hrows = h_pad[sid].reshape(nb, 128, 32)
        he3[:, :, :] = hrows.transpose(1, 0, 2)
        a_st = np.empty((128, 2 * nb), NPBF16)
        a_st[:, 0:nb] = as_pad[sid].reshape(nb, 128).T
        a_st[:, nb:2 * nb] = ad_pad[did].reshape(nb, 128).T
        in2.append({"he": he, "a_st": a_st, "z_st": z_np, "consts": consts})

    key2 = ("agg", nb, LAG, WARM, ACCB, EVAC_DVE, WSPLIT, EQPOOL, ROUNDEND, BIASFOLD, OUT_EVERY, HEBUFS)
    if key2 not in _NC_CACHE:
        _NC_CACHE[key2] = _build_agg(nb)
    nc2 = _NC_CACHE[key2]
    res2 = _run(nc2, in2, trace)
    LAST_RESULTS = (res1, res2)
    _LAST_NCS = (nc1, nc2)

    out_full = np.zeros((N, C_out), np.float32)
    fc = final_col                       # rank -> global col id
    tpi = fc // 128                      # transpose tile
    tpp = fc % 128                       # partition within tile
    for c in range(N_CORES):
        o = np.asarray(res2.results[c]["out"]).astype(np.float32)
        o3 = o.reshape(128, ntp, 32)
        vals = o3[tpp, tpi, :]           # [n_ranks, 32]
        perm = cores[c][3]
        out_full[c * Nc + perm] = vals
    return out_full


# revision 11
# speedup vs baseline: 1.0207x; 1.0008x over previous
"""GAT encoder (PyG GATConv-style, single head) for Trainium2, 8 NeuronCores.

v3: segment-sum as PE matmuls with on-device selection masks.

  Phase 1 (proj): node-partitioned x @ [W | W@att_src | W@att_dst] -> per
  node h (32, bf16) and logits a_s, a_d (bf16).  No activations.

  Host (pure indexing): destinations degree-sorted per core against a
  COMMON degree profile (elementwise max across cores, +0.6% slots) so all
  8 cores share one program.  Edge slots laid dst-major into 128-slot
  windows; each window owns D=13 psum columns (col 0 = carry for a dst
  straddling from the previous window, straddler sits in col D-1; carries
  never cross a 39-window psum bank).

  Phase 2 (agg) per core, DMA-bound at ~17MB (h-stream 14.1MB @360B/ns):
    num  = exp(max(a, 0.2a)), a = a_s[src]+a_d[dst]      (DVE + ACT exp)
    W    = num * (z == j) built as D per-plane tensor_scalar/tensor_tensor
           ops (is_equal planes split DVE/Pool, mults DVE at 2x; D-major
           layout keeps every op densely packed -> 2x mode)
    per window w: two matmuls (stationary ldweights is free in the cost
    model; Matmult cost = out free size only):
      psum[0:32, wD:(w+1)D]  = he_w.T @ W_w    (h gathered per edge slot)
      psum[32:33, wD:(w+1)D] = ones.T @ W_w    (den row, aligned tile pos)
    per bank: ACT-copy psum -> acc_sb bf16; Pool merges window carries.
    PE re-matmul [33,128] acc tiles against [identity | bias-row] ->
    transposed dst-major psum tiles with den*bias folded in
    ((acc + den*bias)/den == acc/den + bias), then per-15-tile rounds:
    DVE rcp(den) + multiply, ACT sigmoid, batched out DMA.

  Host unshards via the rank->column map.  bf16 streams, f32 psum
  accumulate; rel err ~9e-3 vs the 2e-2 gate (fp8 h measured 2.9e-2).
"""
import os
import sys

for _p in ('/opt/trn_rl_repo',):
    if _p not in sys.path and os.path.isdir(_p):
        sys.path.insert(0, _p)

import numpy as np
import ml_dtypes

import concourse.mybir as mybir
import concourse.tile as tile
from concourse import bacc
from concourse.bass_utils import run_bass_kernel_spmd

F32 = mybir.dt.float32
BF16 = mybir.dt.bfloat16
NPBF16 = ml_dtypes.bfloat16

N_CORES = 8
PSUM_CHUNK = 15          # proj: 15*34 = 510 <= 512 f32 per PSUM bank
CW = 34                  # projected width: 32 h + a_s + a_d
D = 13                   # psum columns per 128-slot window
BANK_WINS = 39           # windows per psum bank (39*13 = 507 <= 512)
TP_TILES = 15            # [128,33] bf16 tiles per epilogue round, stride 34
HE_CHUNK_BANKS = 2       # he DMA granularity (banks per DMA)
LAG = int(os.environ.get("GAT_LAG", "0"))    # transpose lag (banks); 0 = all at end
ACCB = int(os.environ.get("GAT_ACCB", "6"))  # psum banks for accumulation
EVAC_DVE = int(os.environ.get("GAT_EVACDVE", "0"))  # every Nth evac on DVE (0=ACT only)
WSPLIT = int(os.environ.get("GAT_WSPLIT", "0"))  # Pool planes (0 = j%2 eq split, mults DVE)
EQPOOL = int(os.environ.get("GAT_EQPOOL", "0"))  # eq planes on Pool (only if WSPLIT=0; 0 = j%2)
ROUNDEND = int(os.environ.get("GAT_ROUNDEND", "0"))  # 1 = epilogue rounds after loop
BIASFOLD = int(os.environ.get("GAT_BIASFOLD", "1"))  # 1 = bias via transpose matmul
OUT_EVERY = int(os.environ.get("GAT_OUTEVERY", "4"))  # rounds per out DMA
WARM = int(os.environ.get("GAT_WARM", "0"))  # fine-grained W warmup chunks
HEBUFS = int(os.environ.get("GAT_HEBUFS", "6"))  # he stream buffers

LAST_RESULTS = None
_NC_CACHE = {}
_LAST_NCS = ()


def sim_exec_time_ns():
    """Sum of TimelineSim estimates for the programs run by kernel()."""
    from concourse.timeline_sim import TimelineSim
    return int(sum(TimelineSim(nc, trace=False).simulate()
                   for nc in _LAST_NCS))


# ---------------------------------------------------------------- planning
def _profile_plan(prof):
    """Pack the common degree profile into 128-slot windows.

    Protocol: per window, col 0 is reserved for a carry (continuation of the
    previous window's straddling dst), new dsts take cols 1..D-2, and a dst
    that straddles into the next window takes col D-1 (its continuation gets
    col 0 there).  The last window of each 42-window bank is padded so no
    carry crosses a psum bank.

    Returns (nb, rank_of_slot[nb*128], col_of_slot[nb*128],
    final_col[n_ranks] (global col id w*D+col)).
    """
    n = len(prof)
    rank_of_slot = []
    col_of_slot = []
    final_col = np.zeros(n, np.int64)
    w = 0          # current window index
    p = 0          # next free slot in window
    newd = 0       # new dsts started in this window

    def close_window():
        nonlocal w, p, newd
        pad = 128 - p
        rank_of_slot.extend([-1] * pad)
        col_of_slot.extend([0] * pad)
        w += 1
        p = 0
        newd = 0

    # visit ranks big/small interleaved so the new-dst cap (D-2 per
    # window) never closes a half-empty window in the small-degree tail
    lo, hi = 0, n - 1
    visit = []
    flip = True
    while lo <= hi:
        if flip:
            visit.append(lo); lo += 1
        else:
            visit.append(hi); hi -= 1
        flip = not flip
    for r in visit:
        d = int(prof[r])
        assert d >= 1
        while True:
            if p >= 128:
                close_window()
                continue
            if newd >= D - 2:
                close_window()
                continue
            space = 128 - p
            straddle = d > space
            if straddle and (w % BANK_WINS) == BANK_WINS - 1:
                # no carry across banks: pad and start in next bank
                close_window()
                continue
            break
        newd += 1
        if straddle:
            rank_of_slot.extend([r] * space)
            col_of_slot.extend([D - 1] * space)
            p = 128
            close_window()
            rest = d - space
            assert rest <= 128
            rank_of_slot.extend([r] * rest)
            col_of_slot.extend([0] * rest)
            p = rest
            final_col[r] = w * D + 0
        else:
            col = newd  # 1..D-2
            rank_of_slot.extend([r] * d)
            col_of_slot.extend([col] * d)
            p += d
            final_col[r] = w * D + col
    if p > 0:
        close_window()
    nb = w
    return (nb, np.array(rank_of_slot, np.int64),
            np.array(col_of_slot, np.int64), final_col)


def _plan(dst, N, n_cores):
    """Common profile + per-core degree-sorted dst orders and edge lists."""
    Nc = N // n_cores
    assert Nc * n_cores == N
    degs = np.zeros((n_cores, Nc), np.int64)
    cores = []
    for c in range(n_cores):
        sel = (dst >= c * Nc) & (dst < (c + 1) * Nc)
        idx = np.nonzero(sel)[0]
        d_c = dst[idx] - c * Nc
        order = np.argsort(d_c, kind='stable')
        eidx_sorted = idx[order]            # edge ids grouped by local dst
        counts = np.bincount(d_c, minlength=Nc).astype(np.int64)
        offsets = np.zeros(Nc + 1, np.int64)
        np.cumsum(counts, out=offsets[1:])
        perm = np.argsort(-counts, kind='stable')   # rank -> local dst
        degs[c] = counts[perm]
        cores.append((eidx_sorted, counts, offsets, perm))
    prof = degs.max(axis=0)
    assert prof[-1] >= 1 and prof[0] <= 128
    nb, rank_of_slot, col_of_slot, final_col = _profile_plan(prof)
    return Nc, prof, nb, rank_of_slot, col_of_slot, final_col, cores


def _core_slots(core_plan, prof, rank_of_slot, n_edges, Nc, c):
    """Per-core (src_of_slot, dst_of_slot) with -1 for padding slots."""
    eidx_sorted, counts, offsets, perm = core_plan
    nslots = len(rank_of_slot)
    src_of_slot = np.full(nslots, -1, np.int64)
    dst_of_slot = np.full(nslots, -1, np.int64)
    # slot positions per rank, in slot order
    pos = np.nonzero(rank_of_slot >= 0)[0]
    rk = rank_of_slot[pos]
    # index of each slot within its rank (0..prof[r]-1), slots of a rank
    # appear in increasing slot order
    order = np.argsort(rk, kind='stable')
    within = np.zeros(len(rk), np.int64)
    cum = np.zeros(len(prof) + 1, np.int64)
    np.cumsum(prof, out=cum[1:])
    within[order] = np.arange(len(rk)) - cum[rk[order]]
    ldst = perm[rk]                        # local dst of each real slot
    k = within
    valid = k < counts[ldst]
    epos = offsets[ldst[valid]] + k[valid]
    src_of_slot[pos[valid]] = -2           # placeholder, filled below
    sv = np.full(len(rk), -1, np.int64)
    sv[valid] = eidx_sorted[epos]
    src_of_slot[pos] = sv                  # edge id per slot (-1 pad)
    dst_of_slot[pos[valid]] = ldst[valid] + c * Nc
    return src_of_slot, dst_of_slot


# ---------------------------------------------------------------- phase 1
def _build_proj(nch):
    nc = bacc.Bacc("TRN2", target_bir_lowering=False, debug=False,
                   num_devices=N_CORES)
    xt = nc.dram_tensor("xt", [128, nch * 128], BF16, kind="ExternalInput").ap()
    wext = nc.dram_tensor("wext", [128, CW], BF16, kind="ExternalInput").ap()
    h_out = nc.dram_tensor("h_out", [128, nch * 32], BF16,
                           kind="ExternalOutput").ap()
    a_out = nc.dram_tensor("a_out", [128, nch * 2], BF16,
                           kind="ExternalOutput").ap()
    N_IN_DMA = 4
    with tile.TileContext(nc) as tc:
        with (
            tc.tile_pool(name="const", bufs=1) as cpool,
            tc.tile_pool(name="ps", bufs=8, space="PSUM") as pspool,
        ):
            wsb = cpool.tile([128, CW], BF16)
            xc = cpool.tile([128, nch * 128], BF16)
            qs = [0]
            left = nch
            for frac in (0.40, 0.30, 0.20, 0.10):
                qs.append(min(nch, qs[-1] + max(1, int(round(nch * frac)))))
            qs[-1] = nch
            for i, (k, k1) in enumerate(zip(qs[:-1], qs[1:])):
                if k1 > k:
                    nc.sync.dma_start(xc[:, k * 128:k1 * 128],
                                      xt[:, k * 128:k1 * 128])
                if i == 0:
                    nc.sync.dma_start(wsb[:], wext[:])
            hsb = cpool.tile([128, nch * 32], BF16)
            asd = cpool.tile([128, nch * 2], BF16)
            n_chunks = -(-nch // PSUM_CHUNK)
            marks = [(n_chunks * 4) // 8, (n_chunks * 6) // 8, n_chunks - 1, n_chunks]
            flush = [0] + sorted(set(min(m * PSUM_CHUNK, nch) for m in marks))
            b0 = 0
            while b0 < nch:
                cn = min(PSUM_CHUNK, nch - b0)
                ps = pspool.tile([128, PSUM_CHUNK * CW], F32, tag="ps")
                for j in range(b0, b0 + cn):
                    nc.tensor.matmul(
                        ps[:, (j - b0) * CW:(j - b0 + 1) * CW],
                        xc[:, j * 128:(j + 1) * 128],
                        wsb[:], start=True, stop=True)
                psv = ps[:, :cn * CW].rearrange("p (s f) -> p s f", f=CW)
                if (b0 // PSUM_CHUNK) % 2 == 0:
                    nc.scalar.activation(
                        hsb[:, b0 * 32:(b0 + cn) * 32]
                        .rearrange("p (s c) -> p s c", c=32),
                        psv[:, :, 0:32],
                        mybir.ActivationFunctionType.Copy)
                else:
                    nc.vector.tensor_copy(
                        out=hsb[:, b0 * 32:(b0 + cn) * 32]
                        .rearrange("p (s c) -> p s c", c=32),
                        in_=psv[:, :, 0:32])
                nc.vector.tensor_copy(
                    out=asd[:, b0 * 2:(b0 + cn) * 2]
                    .rearrange("p (s c) -> p s c", c=2),
                    in_=psv[:, :, 32:34])
                b1 = b0 + cn
                # flush h_out at staged boundaries (earlier pieces bigger)
                for lo, hi in zip(flush[:-1], flush[1:]):
                    if b0 < hi <= b1:
                        if hi == nch:
                            nc.sync.dma_start(a_out[:], asd[:])
                        nc.scalar.dma_start(h_out[:, lo * 32:hi * 32],
                                            hsb[:, lo * 32:hi * 32])
                b0 = b1
    nc.compile()
    return nc


# ---------------------------------------------------------------- phase 2
def _build_agg(nb):
    n_banks = -(-nb // BANK_WINS)
    ncols = nb * D
    ntp = -(-ncols // 128)                # transpose tiles
    nc = bacc.Bacc("TRN2", target_bir_lowering=False, debug=False,
                   num_devices=N_CORES)
    he = nc.dram_tensor("he", [128, nb * 32], BF16, kind="ExternalInput").ap()
    a_st = nc.dram_tensor("a_st", [128, 2 * nb], BF16,
                          kind="ExternalInput").ap()
    z_st = nc.dram_tensor("z_st", [128, nb], BF16, kind="ExternalInput").ap()
    consts = nc.dram_tensor("consts", [128, D + 33 + 32], BF16,
                            kind="ExternalInput").ap()
    out = nc.dram_tensor("out", [128, ntp * 32], BF16,
                         kind="ExternalOutput").ap()
    # epilogue round boundaries (tiles); last rounds smaller for the tail
    bounds = list(range(0, ntp, TP_TILES))
    if len(bounds) >= 2 and ntp - bounds[-1] > 6:
        bounds = bounds[:-1] + [ntp - 12, ntp - 6]
    elif ntp > 6:
        bounds = bounds[:-1] + [max(0, ntp - 6)]
    bounds = sorted(set(b for b in bounds if b < ntp))
    with tile.TileContext(nc) as tc:
        with (
            tc.tile_pool(name="const", bufs=1) as cpool,
            tc.tile_pool(name="hec", bufs=HEBUFS) as hepool,
            tc.tile_pool(name="acc", bufs=ACCB, space="PSUM") as accpool,
            tc.tile_pool(name="tp", bufs=8 - ACCB, space="PSUM") as tppool,
        ):
            # ---- constants + small streams (sync queue: ordered first)
            cst = cpool.tile([128, D + 33 + 32], BF16)
            nc.sync.dma_start(cst[:], consts[:])
            ident = cst[:, D:D + 33]      # identity in partitions 0..32
            bias_sb = cst[:, D + 33:D + 33 + 32]
            ones_sb = cpool.tile([128, 1], BF16)
            nc.gpsimd.memset(ones_sb[:], 1.0)
            ac = cpool.tile([128, 2 * nb], BF16)
            nc.sync.dma_start(ac[:], a_st[:])
            zc = cpool.tile([128, nb], BF16)
            nc.sync.dma_start(zc[:], z_st[:])
            # ---- num = exp(max(a, 0.2a))  [128, nb]
            num = cpool.tile([128, nb], BF16)
            wk = cpool.tile([128, nb], BF16)
            nc.vector.tensor_tensor(out=wk[:], in0=ac[:, 0:nb],
                                    in1=ac[:, nb:2 * nb],
                                    op=mybir.AluOpType.add)
            nc.vector.tensor_scalar(out=num[:], in0=wk[:], scalar1=0.2,
                                    scalar2=None, op0=mybir.AluOpType.mult)
            nc.vector.tensor_tensor(out=wk[:], in0=wk[:], in1=num[:],
                                    op=mybir.AluOpType.max)
            nc.scalar.activation(num[:], wk[:],
                                 mybir.ActivationFunctionType.Exp, scale=1.0)
            # ---- W[p, j, b] = num[p, b] * (z[p, b] == j), D-major
            wsel = cpool.tile([128, D * nb], BF16)
            w3 = wsel[:].rearrange("p (d b) -> p d b", b=nb)
            NCHUNK = 6
            cb = -(-nb // NCHUNK)
            wstate = {"done": 0, "warm": WARM}

            def emit_w_chunk():
                s0 = wstate["done"]
                if s0 >= nb:
                    return
                if wstate["warm"] > 0:
                    wstate["warm"] -= 1
                    s1 = min(s0 + BANK_WINS, nb)
                else:
                    s1 = min(s0 + cb, nb)
                for j in range(D):
                    if WSPLIT:
                        eng = nc.gpsimd if j >= D - WSPLIT else nc.vector
                        eng.tensor_scalar(
                            out=w3[:, j, s0:s1], in0=zc[:, s0:s1],
                            scalar1=float(j), scalar2=None,
                            op0=mybir.AluOpType.is_equal)
                        eng.tensor_tensor(
                            out=w3[:, j, s0:s1], in0=w3[:, j, s0:s1],
                            in1=num[:, s0:s1], op=mybir.AluOpType.mult)
                    else:
                        if EQPOOL:
                            eng = nc.gpsimd if j < EQPOOL else nc.vector
                        else:
                            eng = nc.vector if j % 2 == 0 else nc.gpsimd
                        eng.tensor_scalar(
                            out=w3[:, j, s0:s1], in0=zc[:, s0:s1],
                            scalar1=float(j), scalar2=None,
                            op0=mybir.AluOpType.is_equal)
                        nc.vector.tensor_tensor(
                            out=w3[:, j, s0:s1], in0=w3[:, j, s0:s1],
                            in1=num[:, s0:s1], op=mybir.AluOpType.mult)
                wstate["done"] = s1

            for _ in range(4):
                emit_w_chunk()
            # ---- streaming accumulate + interleaved epilogue
            acc_sb = cpool.tile([128, ntp * 128], BF16)
            if ntp * 128 > ncols:
                nc.gpsimd.memset(acc_sb[0:33, ncols:ntp * 128], 0.0)
            out_sb = cpool.tile([128, ntp * 32], BF16)
            rcp = cpool.tile([128, ntp], BF16)
            state = {"tile": 0, "round": 0, "odma": []}

            def emit_transposes(bank_done):
                """Emit transposes fully covered by merged banks <= bank_done."""
                max_t = min(ntp, ((bank_done + 1) * BANK_WINS * D) // 128)
                if bank_done >= n_banks - 1:
                    max_t = ntp
                while state["tile"] < max_t:
                    t = state["tile"]
                    r = state["round"]
                    t0 = bounds[r]
                    if r not in state["tps"]:
                        if BIASFOLD:
                            tp_r = tppool.tile([128, TP_TILES * 33], F32,
                                               tag="tp")
                        else:
                            tp_r = tppool.tile([128, TP_TILES * 34], BF16,
                                               tag="tp")
                        state["tps"][r] = tp_r
                    tp = state["tps"][r]
                    if BIASFOLD:
                        # regular matmul against [identity | bias row]:
                        # transposed acc with den*bias folded in
                        # ((acc + den*bias)*rcp == acc*rcp + bias)
                        nc.tensor.matmul(
                            tp[:, (t - t0) * 33:(t - t0) * 33 + 33],
                            acc_sb[0:33, t * 128:(t + 1) * 128],
                            ident[0:33, 0:33],
                            start=True, stop=True)
                    else:
                        nc.tensor.transpose(
                            tp[:, (t - t0) * 34:(t - t0) * 34 + 33],
                            acc_sb[0:33, t * 128:(t + 1) * 128],
                            ident[0:33, 0:33])
                    state["tile"] = t + 1
                    t1 = bounds[r + 1] if r + 1 < len(bounds) else ntp
                    if t + 1 == t1:
                        if not ROUNDEND:
                            emit_round(r, t0, t1, state["tps"][r])
                        state["round"] = r + 1

            def emit_round(r, t0, t1, tp):
                cw = 33 if BIASFOLD else 34
                tpv = tp[:, :(t1 - t0) * cw] \
                    .rearrange("p (t c) -> p t c", c=cw)
                with nc.allow_low_precision(reason="1/den bf16"):
                    nc.vector.reciprocal(rcp[:, t0:t1], tpv[:, :, 32])
                ov = out_sb[:, t0 * 32:t1 * 32] \
                    .rearrange("p (t c) -> p t c", c=32)
                nc.vector.tensor_tensor(
                    out=ov, in0=tpv[:, :, 0:32],
                    in1=rcp[:, t0:t1].rearrange("p (t o) -> p t o", o=1)
                    .to_broadcast([128, t1 - t0, 32]),
                    op=mybir.AluOpType.mult)
                if not BIASFOLD:
                    nc.vector.tensor_tensor(
                        out=ov, in0=ov,
                        in1=bias_sb.rearrange("p (o c) -> p o c", o=1)
                        .to_broadcast([128, t1 - t0, 32]),
                        op=mybir.AluOpType.add)
                nc.scalar.activation(out_sb[:, t0 * 32:t1 * 32],
                                     out_sb[:, t0 * 32:t1 * 32],
                                     mybir.ActivationFunctionType.Sigmoid)
                state["odma"].append((t0, t1))
                flush = (r % OUT_EVERY == OUT_EVERY - 1
                         or t1 >= ntp)
                if flush:
                    o0 = state["odma"][0][0]
                    o1 = state["odma"][-1][1]
                    state["odma"] = []
                    nc.scalar.dma_start(out[:, o0 * 32:o1 * 32],
                                        out_sb[:, o0 * 32:o1 * 32])

            # tp tiles must be allocated per round; pre-wire creation order
            state["tps"] = {}
            for s0 in range(0, nb, HE_CHUNK_BANKS * BANK_WINS):
                s1 = min(s0 + HE_CHUNK_BANKS * BANK_WINS, nb)
                hc = hepool.tile([128, HE_CHUNK_BANKS * BANK_WINS * 32], BF16,
                                 tag="hec")
                nc.sync.dma_start(hc[:, :(s1 - s0) * 32],
                                  he[:, s0 * 32:s1 * 32])
                # keep the on-device W build ~3 he-chunks ahead of the
                # matmul stream so merges queue promptly behind it
                if wstate["done"] < min(nb, s1 + 3 * HE_CHUNK_BANKS * BANK_WINS):
                    emit_w_chunk()
                for b in range(s0 // BANK_WINS,
                               s0 // BANK_WINS + HE_CHUNK_BANKS):
                    if b >= n_banks:
                        break
                    while wstate["done"] < min(nb, (b + 1) * BANK_WINS):
                        emit_w_chunk()
                    w0 = b * BANK_WINS
                    w1 = min(w0 + BANK_WINS, nb)
                    ap = accpool.tile([128, 512], F32, tag="acc")
                    for w in range(w0, min(w0 + BANK_WINS, nb)):
                        lw = w - s0
                        nc.tensor.matmul(
                            ap[0:32, (w - w0) * D:(w - w0 + 1) * D],
                            hc[:, lw * 32:(lw + 1) * 32],
                            w3[:, :, w],
                            start=True, stop=True)
                        nc.tensor.matmul(
                            ap[32:33, (w - w0) * D:(w - w0 + 1) * D],
                            ones_sb[:], w3[:, :, w],
                            start=True, stop=True)
                    # evacuate bank -> acc_sb (mostly ACT; Copy is in
                    # every act table set so no reloads)
                    if EVAC_DVE and b % EVAC_DVE == EVAC_DVE - 1:
                        nc.vector.tensor_copy(
                            out=acc_sb[0:33, w0 * D:w1 * D],
                            in_=ap[0:33, 0:(w1 - w0) * D])
                    else:
                        nc.scalar.activation(
                            acc_sb[0:33, w0 * D:w1 * D],
                            ap[0:33, 0:(w1 - w0) * D],
                            mybir.ActivationFunctionType.Copy)
                    # merge carries within the bank (Pool, sbuf only)
                    if w1 - w0 > 1:
                        a3o = acc_sb[0:33, w0 * D + D:w1 * D] \
                            .rearrange("p (b d) -> p b d", d=D)
                        a3i = acc_sb[0:33, w0 * D + D - 1:w1 * D - 1] \
                            .rearrange("p (b d) -> p b d", d=D)
                        nc.gpsimd.tensor_tensor(
                            out=a3o[:, :, 0:1], in0=a3o[:, :, 0:1],
                            in1=a3i[:, :, 0:1], op=mybir.AluOpType.add)
                    # interleave transposes/epilogue with a lag so their
                    # evac/merge deps are long resolved by the time in-order
                    # PE.SEQ reaches them
                    if LAG > 0 and b >= LAG:
                        emit_transposes(b - LAG)
            emit_transposes(n_banks - 1)
            assert state["tile"] == ntp and state["round"] == len(bounds)
            if ROUNDEND:
                for r, t0 in enumerate(bounds):
                    t1 = bounds[r + 1] if r + 1 < len(bounds) else ntp
                    emit_round(r, t0, t1, state["tps"][r])
    nc.compile()
    return nc


# ---------------------------------------------------------------- runners
def _run(nc, in_maps, trace):
    if os.environ.get("GAT_SIM"):
        from concourse.bass_interp import CoreSim
        results = []
        for m in in_maps:
            sim = CoreSim(nc, require_finite=False, require_nnan=False)
            for k, v in m.items():
                sim.tensor(k)[:] = v
            sim.simulate()
            outs = {}
            for alloc in nc.m.functions[0].allocations:
                if getattr(alloc, 'kind', None) == "ExternalOutput":
                    name = alloc.memorylocations[0].name
                    outs[name] = np.array(sim.tensor(name))
            results.append(outs)
        class R: pass
        r = R(); r.results = results; r.exec_time_ns = None
        return r
    return run_bass_kernel_spmd(nc, in_maps, core_ids=list(range(N_CORES)),
                                trace=trace)


def kernel(x, edge_index, W, att_src, att_dst, bias):
    global LAST_RESULTS, _LAST_NCS
    x = np.asarray(x, np.float32)
    edge_index = np.asarray(edge_index)
    W = np.asarray(W, np.float32)
    att_src = np.asarray(att_src, np.float32)
    att_dst = np.asarray(att_dst, np.float32)
    bias_np = np.asarray(bias, np.float32)

    N, C_in = x.shape
    C_out = W.shape[1]
    assert C_in == 128 and C_out == 32, (C_in, C_out)
    trace = bool(os.environ.get("GAT_TRACE"))

    loops = np.arange(N, dtype=np.int64)
    src = np.concatenate([edge_index[0].astype(np.int64), loops])
    dst = np.concatenate([edge_index[1].astype(np.int64), loops])

    Nc, prof, nb, rank_of_slot, col_of_slot, final_col, cores = \
        _plan(dst, N, N_CORES)
    nch = -(-Nc // 128)

    # ---- phase 1: project every node once (node-partitioned) ----
    ws = W @ att_src
    wd = W @ att_dst
    wext = np.concatenate([W, ws[:, None], wd[:, None]], 1).astype(NPBF16)
    in1 = []
    for c in range(N_CORES):
        xt = np.zeros((128, nch * 128), NPBF16)
        n0 = c * Nc
        xt[:, :Nc] = x[n0:n0 + Nc].T.astype(NPBF16)
        in1.append({"xt": xt, "wext": wext})

    key1 = ("proj", nch)
    if key1 not in _NC_CACHE:
        _NC_CACHE[key1] = _build_proj(nch)
    nc1 = _NC_CACHE[key1]
    res1 = _run(nc1, in1, trace)

    h_all = np.zeros((N, 32), NPBF16)
    as_all = np.zeros(N, NPBF16)
    ad_all = np.zeros(N, NPBF16)
    for c in range(N_CORES):
        o = res1.results[c]
        h = np.asarray(o["h_out"]).reshape(128, nch, 32) \
            .transpose(1, 0, 2).reshape(nch * 128, 32)[:Nc]
        a = np.asarray(o["a_out"]).reshape(128, nch, 2) \
            .transpose(1, 0, 2).reshape(nch * 128, 2)[:Nc]
        sl = slice(c * Nc, (c + 1) * Nc)
        h_all[sl] = h
        as_all[sl], ad_all[sl] = a[:, 0], a[:, 1]

    # ---- host gather: per-core streams (pure indexing) ----
    h_pad = np.concatenate([h_all, np.zeros((1, 32), NPBF16)], 0)
    as_pad = np.concatenate([as_all, [NPBF16(-300.0)]])
    ad_pad = np.concatenate([ad_all, [NPBF16(-300.0)]])

    z_np = np.broadcast_to(
        col_of_slot.reshape(nb, 128).T.astype(NPBF16), (128, nb)).copy()
    consts = np.zeros((128, D + 33 + 32), NPBF16)
    consts[:, 0:D] = np.arange(D, dtype=np.float32).astype(NPBF16)[None, :]
    idb = np.eye(33, dtype=np.float32)
    idb[32, 0:32] = bias_np          # transpose-matmul folds den*bias in
    consts[0:33, D:D + 33] = idb.astype(NPBF16)
    consts[:, D + 33:] = bias_np.astype(NPBF16)[None, :]

    ncols = nb * D
    ntp = -(-ncols // 128)

    in2 = []
    for c in range(N_CORES):
        src_of_slot, dst_of_slot = _core_slots(
            cores[c], prof, rank_of_slot, len(src), Nc, c)
        sid = np.where(src_of_slot >= 0, src[np.maximum(src_of_slot, 0)], N)
        did = np.where(dst_of_slot >= 0, dst_of_slot, N)
        he = np.empty((128, nb * 32), NPBF16)
        he3 = he.reshape(128, nb, 32)
        hrows = h_pad[sid].reshape(nb, 128, 32)
        he3[:, :, :] = hrows.transpose(1, 0, 2)
        a_st = np.empty((128, 2 * nb), NPBF16)
        a_st[:, 0:nb] = as_pad[sid].reshape(nb, 128).T
        a_st[:, nb:2 * nb] = ad_pad[did].reshape(nb, 128).T
        in2.append({"he": he, "a_st": a_st, "z_st": z_np, "consts": consts})

    key2 = ("agg", nb, LAG, WARM, ACCB, EVAC_DVE, WSPLIT, EQPOOL, ROUNDEND, BIASFOLD, OUT_EVERY, HEBUFS)
    if key2 not in _NC_CACHE:
        _NC_CACHE[key2] = _build_agg(nb)
    nc2 = _NC_CACHE[key2]
    res2 = _run(nc2, in2, trace)
    LAST_RESULTS = (res1, res2)
    _LAST_NCS = (nc1, nc2)

    out_full = np.zeros((N, C_out), np.float32)
    fc = final_col                       # rank -> global col id
    tpi = fc // 128                      # transpose tile
    tpp = fc % 128                       # partition within tile
    for c in range(N_CORES):
        o = np.asarray(res2.results[c]["out"]).astype(np.float32)
        o3 = o.reshape(128, ntp, 32)
        vals = o3[tpp, tpi, :]           # [n_ranks, 32]
        perm = cores[c][3]
        out_full[c * Nc + perm] = vals
    return out_full


# revision 12
# speedup vs baseline: 1.0483x; 1.0270x over previous
"""GAT encoder (PyG GATConv-style, single head) for Trainium2, 8 NeuronCores.

v3: segment-sum as PE matmuls with on-device selection masks.

  Phase 1 (proj): node-partitioned x @ [W | W@att_src | W@att_dst] -> per
  node h (32, bf16) and logits a_s, a_d (bf16).  No activations.

  Host (pure indexing): destinations degree-sorted per core against a
  COMMON degree profile (elementwise max across cores, +0.6% slots) so all
  8 cores share one program.  Edge slots laid dst-major into 128-slot
  windows; each window owns D=13 psum columns (col 0 = carry for a dst
  straddling from the previous window, straddler sits in col D-1; carries
  never cross a 39-window psum bank).

  Phase 2 (agg) per core, DMA-bound at ~17MB (h-stream 14.1MB @360B/ns):
    num  = exp(max(a, 0.2a)), a = a_s[src]+a_d[dst]      (DVE + ACT exp)
    W    = num * (z == j) built as D per-plane tensor_scalar/tensor_tensor
           ops (is_equal planes split DVE/Pool, mults DVE at 2x; D-major
           layout keeps every op densely packed -> 2x mode)
    per window w: two matmuls (stationary ldweights is free in the cost
    model; Matmult cost = out free size only):
      psum[0:32, wD:(w+1)D]  = he_w.T @ W_w    (h gathered per edge slot)
      psum[32:33, wD:(w+1)D] = ones.T @ W_w    (den row, aligned tile pos)
    per bank: ACT-copy psum -> acc_sb bf16; Pool merges window carries.
    PE re-matmul [33,128] acc tiles against [identity | bias-row] ->
    transposed dst-major psum tiles with den*bias folded in
    ((acc + den*bias)/den == acc/den + bias), then per-15-tile rounds:
    DVE rcp(den) + multiply, ACT sigmoid, batched out DMA.

  Host unshards via the rank->column map.  bf16 streams, f32 psum
  accumulate; rel err ~9e-3 vs the 2e-2 gate (fp8 h measured 2.9e-2).
"""
import os
import sys

for _p in ('/opt/trn_rl_repo',):
    if _p not in sys.path and os.path.isdir(_p):
        sys.path.insert(0, _p)

import numpy as np
import ml_dtypes

import concourse.mybir as mybir
import concourse.tile as tile
from concourse import bacc
from concourse.bass_utils import run_bass_kernel_spmd

F32 = mybir.dt.float32
BF16 = mybir.dt.bfloat16
NPBF16 = ml_dtypes.bfloat16

N_CORES = 8
PSUM_CHUNK = 15          # proj: 15*34 = 510 <= 512 f32 per PSUM bank
CW = 34                  # projected width: 32 h + a_s + a_d
D = 10                   # psum columns per 128-slot window
BANK_WINS = 51           # windows per psum bank (51*10 = 510 <= 512)
TP_TILES = 15            # [128,33] bf16 tiles per epilogue round, stride 34
HE_CHUNK_BANKS = 2       # he DMA granularity (banks per DMA)
LAG = int(os.environ.get("GAT_LAG", "0"))    # transpose lag (banks); 0 = all at end
ACCB = int(os.environ.get("GAT_ACCB", "6"))  # psum banks for accumulation
EVAC_DVE = int(os.environ.get("GAT_EVACDVE", "0"))  # every Nth evac on DVE (0=ACT only)
WSPLIT = int(os.environ.get("GAT_WSPLIT", "0"))  # Pool planes (0 = j%2 eq split, mults DVE)
EQPOOL = int(os.environ.get("GAT_EQPOOL", "0"))  # eq planes on Pool (only if WSPLIT=0; 0 = j%2)
ROUNDEND = int(os.environ.get("GAT_ROUNDEND", "0"))  # 1 = epilogue rounds after loop
BIASFOLD = int(os.environ.get("GAT_BIASFOLD", "1"))  # 1 = bias via transpose matmul
OUT_EVERY = int(os.environ.get("GAT_OUTEVERY", "4"))  # rounds per out DMA
WARM = int(os.environ.get("GAT_WARM", "0"))  # fine-grained W warmup chunks
HEBUFS = int(os.environ.get("GAT_HEBUFS", "6"))  # he stream buffers

LAST_RESULTS = None
_NC_CACHE = {}
_LAST_NCS = ()


def sim_exec_time_ns():
    """Sum of TimelineSim estimates for the programs run by kernel()."""
    from concourse.timeline_sim import TimelineSim
    return int(sum(TimelineSim(nc, trace=False).simulate()
                   for nc in _LAST_NCS))


# ---------------------------------------------------------------- planning
def _profile_plan(prof):
    """Pack the common degree profile into 128-slot windows.

    Protocol: per window, col 0 is reserved for a carry (continuation of the
    previous window's straddling dst), new dsts take cols 1..D-2, and a dst
    that straddles into the next window takes col D-1 (its continuation gets
    col 0 there).  The last window of each 42-window bank is padded so no
    carry crosses a psum bank.

    Returns (nb, rank_of_slot[nb*128], col_of_slot[nb*128],
    final_col[n_ranks] (global col id w*D+col)).
    """
    n = len(prof)
    rank_of_slot = []
    col_of_slot = []
    final_col = np.zeros(n, np.int64)
    w = 0          # current window index
    p = 0          # next free slot in window
    newd = 0       # new dsts started in this window

    def close_window():
        nonlocal w, p, newd
        pad = 128 - p
        rank_of_slot.extend([-1] * pad)
        col_of_slot.extend([0] * pad)
        w += 1
        p = 0
        newd = 0

    # visit ranks big/small interleaved so the new-dst cap (D-2 per
    # window) never closes a half-empty window in the small-degree tail
    lo, hi = 0, n - 1
    visit = []
    flip = True
    while lo <= hi:
        if flip:
            visit.append(lo); lo += 1
        else:
            visit.append(hi); hi -= 1
        flip = not flip
    for r in visit:
        d = int(prof[r])
        assert d >= 1
        while True:
            if p >= 128:
                close_window()
                continue
            if newd >= D - 2:
                close_window()
                continue
            space = 128 - p
            straddle = d > space
            if straddle and (w % BANK_WINS) == BANK_WINS - 1:
                # no carry across banks: pad and start in next bank
                close_window()
                continue
            break
        newd += 1
        if straddle:
            rank_of_slot.extend([r] * space)
            col_of_slot.extend([D - 1] * space)
            p = 128
            close_window()
            rest = d - space
            assert rest <= 128
            rank_of_slot.extend([r] * rest)
            col_of_slot.extend([0] * rest)
            p = rest
            final_col[r] = w * D + 0
        else:
            col = newd  # 1..D-2
            rank_of_slot.extend([r] * d)
            col_of_slot.extend([col] * d)
            p += d
            final_col[r] = w * D + col
    if p > 0:
        close_window()
    nb = w
    return (nb, np.array(rank_of_slot, np.int64),
            np.array(col_of_slot, np.int64), final_col)


def _plan(dst, N, n_cores):
    """Common profile + per-core degree-sorted dst orders and edge lists."""
    Nc = N // n_cores
    assert Nc * n_cores == N
    degs = np.zeros((n_cores, Nc), np.int64)
    cores = []
    for c in range(n_cores):
        sel = (dst >= c * Nc) & (dst < (c + 1) * Nc)
        idx = np.nonzero(sel)[0]
        d_c = dst[idx] - c * Nc
        order = np.argsort(d_c, kind='stable')
        eidx_sorted = idx[order]            # edge ids grouped by local dst
        counts = np.bincount(d_c, minlength=Nc).astype(np.int64)
        offsets = np.zeros(Nc + 1, np.int64)
        np.cumsum(counts, out=offsets[1:])
        perm = np.argsort(-counts, kind='stable')   # rank -> local dst
        degs[c] = counts[perm]
        cores.append((eidx_sorted, counts, offsets, perm))
    prof = degs.max(axis=0)
    assert prof[-1] >= 1 and prof[0] <= 128
    nb, rank_of_slot, col_of_slot, final_col = _profile_plan(prof)
    return Nc, prof, nb, rank_of_slot, col_of_slot, final_col, cores


def _core_slots(core_plan, prof, rank_of_slot, n_edges, Nc, c):
    """Per-core (src_of_slot, dst_of_slot) with -1 for padding slots."""
    eidx_sorted, counts, offsets, perm = core_plan
    nslots = len(rank_of_slot)
    src_of_slot = np.full(nslots, -1, np.int64)
    dst_of_slot = np.full(nslots, -1, np.int64)
    # slot positions per rank, in slot order
    pos = np.nonzero(rank_of_slot >= 0)[0]
    rk = rank_of_slot[pos]
    # index of each slot within its rank (0..prof[r]-1), slots of a rank
    # appear in increasing slot order
    order = np.argsort(rk, kind='stable')
    within = np.zeros(len(rk), np.int64)
    cum = np.zeros(len(prof) + 1, np.int64)
    np.cumsum(prof, out=cum[1:])
    within[order] = np.arange(len(rk)) - cum[rk[order]]
    ldst = perm[rk]                        # local dst of each real slot
    k = within
    valid = k < counts[ldst]
    epos = offsets[ldst[valid]] + k[valid]
    src_of_slot[pos[valid]] = -2           # placeholder, filled below
    sv = np.full(len(rk), -1, np.int64)
    sv[valid] = eidx_sorted[epos]
    src_of_slot[pos] = sv                  # edge id per slot (-1 pad)
    dst_of_slot[pos[valid]] = ldst[valid] + c * Nc
    return src_of_slot, dst_of_slot


# ---------------------------------------------------------------- phase 1
def _build_proj(nch):
    nc = bacc.Bacc("TRN2", target_bir_lowering=False, debug=False,
                   num_devices=N_CORES)
    xt = nc.dram_tensor("xt", [128, nch * 128], BF16, kind="ExternalInput").ap()
    wext = nc.dram_tensor("wext", [128, CW], BF16, kind="ExternalInput").ap()
    h_out = nc.dram_tensor("h_out", [128, nch * 32], BF16,
                           kind="ExternalOutput").ap()
    a_out = nc.dram_tensor("a_out", [128, nch * 2], BF16,
                           kind="ExternalOutput").ap()
    N_IN_DMA = 4
    with tile.TileContext(nc) as tc:
        with (
            tc.tile_pool(name="const", bufs=1) as cpool,
            tc.tile_pool(name="ps", bufs=8, space="PSUM") as pspool,
        ):
            wsb = cpool.tile([128, CW], BF16)
            xc = cpool.tile([128, nch * 128], BF16)
            qs = [0]
            left = nch
            for frac in (0.40, 0.30, 0.20, 0.10):
                qs.append(min(nch, qs[-1] + max(1, int(round(nch * frac)))))
            qs[-1] = nch
            for i, (k, k1) in enumerate(zip(qs[:-1], qs[1:])):
                if k1 > k:
                    nc.sync.dma_start(xc[:, k * 128:k1 * 128],
                                      xt[:, k * 128:k1 * 128])
                if i == 0:
                    nc.sync.dma_start(wsb[:], wext[:])
            hsb = cpool.tile([128, nch * 32], BF16)
            asd = cpool.tile([128, nch * 2], BF16)
            n_chunks = -(-nch // PSUM_CHUNK)
            marks = [(n_chunks * 4) // 8, (n_chunks * 6) // 8, n_chunks - 1, n_chunks]
            flush = [0] + sorted(set(min(m * PSUM_CHUNK, nch) for m in marks))
            b0 = 0
            while b0 < nch:
                cn = min(PSUM_CHUNK, nch - b0)
                ps = pspool.tile([128, PSUM_CHUNK * CW], F32, tag="ps")
                for j in range(b0, b0 + cn):
                    nc.tensor.matmul(
                        ps[:, (j - b0) * CW:(j - b0 + 1) * CW],
                        xc[:, j * 128:(j + 1) * 128],
                        wsb[:], start=True, stop=True)
                psv = ps[:, :cn * CW].rearrange("p (s f) -> p s f", f=CW)
                if (b0 // PSUM_CHUNK) % 2 == 0:
                    nc.scalar.activation(
                        hsb[:, b0 * 32:(b0 + cn) * 32]
                        .rearrange("p (s c) -> p s c", c=32),
                        psv[:, :, 0:32],
                        mybir.ActivationFunctionType.Copy)
                else:
                    nc.vector.tensor_copy(
                        out=hsb[:, b0 * 32:(b0 + cn) * 32]
                        .rearrange("p (s c) -> p s c", c=32),
                        in_=psv[:, :, 0:32])
                nc.vector.tensor_copy(
                    out=asd[:, b0 * 2:(b0 + cn) * 2]
                    .rearrange("p (s c) -> p s c", c=2),
                    in_=psv[:, :, 32:34])
                b1 = b0 + cn
                # flush h_out at staged boundaries (earlier pieces bigger)
                for lo, hi in zip(flush[:-1], flush[1:]):
                    if b0 < hi <= b1:
                        if hi == nch:
                            nc.sync.dma_start(a_out[:], asd[:])
                        nc.scalar.dma_start(h_out[:, lo * 32:hi * 32],
                                            hsb[:, lo * 32:hi * 32])
                b0 = b1
    nc.compile()
    return nc


# ---------------------------------------------------------------- phase 2
def _build_agg(nb):
    n_banks = -(-nb // BANK_WINS)
    ncols = nb * D
    ntp = -(-ncols // 128)                # transpose tiles
    nc = bacc.Bacc("TRN2", target_bir_lowering=False, debug=False,
                   num_devices=N_CORES)
    he = nc.dram_tensor("he", [128, nb * 32], BF16, kind="ExternalInput").ap()
    a_st = nc.dram_tensor("a_st", [128, 2 * nb], BF16,
                          kind="ExternalInput").ap()
    z_st = nc.dram_tensor("z_st", [128, nb], BF16, kind="ExternalInput").ap()
    consts = nc.dram_tensor("consts", [128, D + 33 + 32], BF16,
                            kind="ExternalInput").ap()
    out = nc.dram_tensor("out", [128, ntp * 32], BF16,
                         kind="ExternalOutput").ap()
    # epilogue round boundaries (tiles); last rounds smaller for the tail
    bounds = list(range(0, ntp, TP_TILES))
    if len(bounds) >= 2 and ntp - bounds[-1] > 6:
        bounds = bounds[:-1] + [ntp - 12, ntp - 6]
    elif ntp > 6:
        bounds = bounds[:-1] + [max(0, ntp - 6)]
    bounds = sorted(set(b for b in bounds if b < ntp))
    with tile.TileContext(nc) as tc:
        with (
            tc.tile_pool(name="const", bufs=1) as cpool,
            tc.tile_pool(name="hec", bufs=HEBUFS) as hepool,
            tc.tile_pool(name="acc", bufs=ACCB, space="PSUM") as accpool,
            tc.tile_pool(name="tp", bufs=8 - ACCB, space="PSUM") as tppool,
        ):
            # ---- constants + small streams (sync queue: ordered first)
            cst = cpool.tile([128, D + 33 + 32], BF16)
            nc.sync.dma_start(cst[:], consts[:])
            ident = cst[:, D:D + 33]      # identity in partitions 0..32
            bias_sb = cst[:, D + 33:D + 33 + 32]
            ones_sb = cpool.tile([128, 1], BF16)
            nc.gpsimd.memset(ones_sb[:], 1.0)
            ac = cpool.tile([128, 2 * nb], BF16)
            nc.sync.dma_start(ac[:], a_st[:])
            zc = cpool.tile([128, nb], BF16)
            nc.sync.dma_start(zc[:], z_st[:])
            # ---- num = exp(max(a, 0.2a))  [128, nb]
            num = cpool.tile([128, nb], BF16)
            wk = cpool.tile([128, nb], BF16)
            nc.vector.tensor_tensor(out=wk[:], in0=ac[:, 0:nb],
                                    in1=ac[:, nb:2 * nb],
                                    op=mybir.AluOpType.add)
            nc.vector.tensor_scalar(out=num[:], in0=wk[:], scalar1=0.2,
                                    scalar2=None, op0=mybir.AluOpType.mult)
            nc.vector.tensor_tensor(out=wk[:], in0=wk[:], in1=num[:],
                                    op=mybir.AluOpType.max)
            nc.scalar.activation(num[:], wk[:],
                                 mybir.ActivationFunctionType.Exp, scale=1.0)
            # ---- W[p, j, b] = num[p, b] * (z[p, b] == j), D-major
            wsel = cpool.tile([128, D * nb], BF16)
            w3 = wsel[:].rearrange("p (d b) -> p d b", b=nb)
            NCHUNK = 6
            cb = -(-nb // NCHUNK)
            wstate = {"done": 0, "warm": WARM}

            def emit_w_chunk():
                s0 = wstate["done"]
                if s0 >= nb:
                    return
                if wstate["warm"] > 0:
                    wstate["warm"] -= 1
                    s1 = min(s0 + BANK_WINS, nb)
                else:
                    s1 = min(s0 + cb, nb)
                for j in range(D):
                    if WSPLIT:
                        eng = nc.gpsimd if j >= D - WSPLIT else nc.vector
                        eng.tensor_scalar(
                            out=w3[:, j, s0:s1], in0=zc[:, s0:s1],
                            scalar1=float(j), scalar2=None,
                            op0=mybir.AluOpType.is_equal)
                        eng.tensor_tensor(
                            out=w3[:, j, s0:s1], in0=w3[:, j, s0:s1],
                            in1=num[:, s0:s1], op=mybir.AluOpType.mult)
                    else:
                        if EQPOOL:
                            eng = nc.gpsimd if j < EQPOOL else nc.vector
                        else:
                            eng = nc.vector if j % 2 == 0 else nc.gpsimd
                        eng.tensor_scalar(
                            out=w3[:, j, s0:s1], in0=zc[:, s0:s1],
                            scalar1=float(j), scalar2=None,
                            op0=mybir.AluOpType.is_equal)
                        nc.vector.tensor_tensor(
                            out=w3[:, j, s0:s1], in0=w3[:, j, s0:s1],
                            in1=num[:, s0:s1], op=mybir.AluOpType.mult)
                wstate["done"] = s1

            for _ in range(4):
                emit_w_chunk()
            # ---- streaming accumulate + interleaved epilogue
            acc_sb = cpool.tile([128, ntp * 128], BF16)
            if ntp * 128 > ncols:
                nc.gpsimd.memset(acc_sb[0:33, ncols:ntp * 128], 0.0)
            out_sb = cpool.tile([128, ntp * 32], BF16)
            rcp = cpool.tile([128, ntp], BF16)
            state = {"tile": 0, "round": 0, "odma": []}

            def emit_transposes(bank_done):
                """Emit transposes fully covered by merged banks <= bank_done."""
                max_t = min(ntp, ((bank_done + 1) * BANK_WINS * D) // 128)
                if bank_done >= n_banks - 1:
                    max_t = ntp
                while state["tile"] < max_t:
                    t = state["tile"]
                    r = state["round"]
                    t0 = bounds[r]
                    if r not in state["tps"]:
                        if BIASFOLD:
                            tp_r = tppool.tile([128, TP_TILES * 33], F32,
                                               tag="tp")
                        else:
                            tp_r = tppool.tile([128, TP_TILES * 34], BF16,
                                               tag="tp")
                        state["tps"][r] = tp_r
                    tp = state["tps"][r]
                    if BIASFOLD:
                        # regular matmul against [identity | bias row]:
                        # transposed acc with den*bias folded in
                        # ((acc + den*bias)*rcp == acc*rcp + bias)
                        nc.tensor.matmul(
                            tp[:, (t - t0) * 33:(t - t0) * 33 + 33],
                            acc_sb[0:33, t * 128:(t + 1) * 128],
                            ident[0:33, 0:33],
                            start=True, stop=True)
                    else:
                        nc.tensor.transpose(
                            tp[:, (t - t0) * 34:(t - t0) * 34 + 33],
                            acc_sb[0:33, t * 128:(t + 1) * 128],
                            ident[0:33, 0:33])
                    state["tile"] = t + 1
                    t1 = bounds[r + 1] if r + 1 < len(bounds) else ntp
                    if t + 1 == t1:
                        if not ROUNDEND:
                            emit_round(r, t0, t1, state["tps"][r])
                        state["round"] = r + 1

            def emit_round(r, t0, t1, tp):
                cw = 33 if BIASFOLD else 34
                tpv = tp[:, :(t1 - t0) * cw] \
                    .rearrange("p (t c) -> p t c", c=cw)
                with nc.allow_low_precision(reason="1/den bf16"):
                    nc.vector.reciprocal(rcp[:, t0:t1], tpv[:, :, 32])
                ov = out_sb[:, t0 * 32:t1 * 32] \
                    .rearrange("p (t c) -> p t c", c=32)
                nc.vector.tensor_tensor(
                    out=ov, in0=tpv[:, :, 0:32],
                    in1=rcp[:, t0:t1].rearrange("p (t o) -> p t o", o=1)
                    .to_broadcast([128, t1 - t0, 32]),
                    op=mybir.AluOpType.mult)
                if not BIASFOLD:
                    nc.vector.tensor_tensor(
                        out=ov, in0=ov,
                        in1=bias_sb.rearrange("p (o c) -> p o c", o=1)
                        .to_broadcast([128, t1 - t0, 32]),
                        op=mybir.AluOpType.add)
                nc.scalar.activation(out_sb[:, t0 * 32:t1 * 32],
                                     out_sb[:, t0 * 32:t1 * 32],
                                     mybir.ActivationFunctionType.Sigmoid)
                state["odma"].append((t0, t1))
                flush = (r % OUT_EVERY == OUT_EVERY - 1
                         or t1 >= ntp)
                if flush:
                    o0 = state["odma"][0][0]
                    o1 = state["odma"][-1][1]
                    state["odma"] = []
                    nc.scalar.dma_start(out[:, o0 * 32:o1 * 32],
                                        out_sb[:, o0 * 32:o1 * 32])

            # tp tiles must be allocated per round; pre-wire creation order
            state["tps"] = {}
            for s0 in range(0, nb, HE_CHUNK_BANKS * BANK_WINS):
                s1 = min(s0 + HE_CHUNK_BANKS * BANK_WINS, nb)
                hc = hepool.tile([128, HE_CHUNK_BANKS * BANK_WINS * 32], BF16,
                                 tag="hec")
                nc.sync.dma_start(hc[:, :(s1 - s0) * 32],
                                  he[:, s0 * 32:s1 * 32])
                # keep the on-device W build ~3 he-chunks ahead of the
                # matmul stream so merges queue promptly behind it
                if wstate["done"] < min(nb, s1 + 3 * HE_CHUNK_BANKS * BANK_WINS):
                    emit_w_chunk()
                for b in range(s0 // BANK_WINS,
                               s0 // BANK_WINS + HE_CHUNK_BANKS):
                    if b >= n_banks:
                        break
                    while wstate["done"] < min(nb, (b + 1) * BANK_WINS):
                        emit_w_chunk()
                    w0 = b * BANK_WINS
                    w1 = min(w0 + BANK_WINS, nb)
                    ap = accpool.tile([128, 512], F32, tag="acc")
                    for w in range(w0, min(w0 + BANK_WINS, nb)):
                        lw = w - s0
                        nc.tensor.matmul(
                            ap[0:32, (w - w0) * D:(w - w0 + 1) * D],
                            hc[:, lw * 32:(lw + 1) * 32],
                            w3[:, :, w],
                            start=True, stop=True)
                        nc.tensor.matmul(
                            ap[32:33, (w - w0) * D:(w - w0 + 1) * D],
                            ones_sb[:], w3[:, :, w],
                            start=True, stop=True)
                    # evacuate bank -> acc_sb (mostly ACT; Copy is in
                    # every act table set so no reloads)
                    if EVAC_DVE and b % EVAC_DVE == EVAC_DVE - 1:
                        nc.vector.tensor_copy(
                            out=acc_sb[0:33, w0 * D:w1 * D],
                            in_=ap[0:33, 0:(w1 - w0) * D])
                    else:
                        nc.scalar.activation(
                            acc_sb[0:33, w0 * D:w1 * D],
                            ap[0:33, 0:(w1 - w0) * D],
                            mybir.ActivationFunctionType.Copy)
                    # merge carries within the bank (Pool, sbuf only)
                    if w1 - w0 > 1:
                        a3o = acc_sb[0:33, w0 * D + D:w1 * D] \
                            .rearrange("p (b d) -> p b d", d=D)
                        a3i = acc_sb[0:33, w0 * D + D - 1:w1 * D - 1] \
                            .rearrange("p (b d) -> p b d", d=D)
                        nc.gpsimd.tensor_tensor(
                            out=a3o[:, :, 0:1], in0=a3o[:, :, 0:1],
                            in1=a3i[:, :, 0:1], op=mybir.AluOpType.add)
                    # interleave transposes/epilogue with a lag so their
                    # evac/merge deps are long resolved by the time in-order
                    # PE.SEQ reaches them
                    if LAG > 0 and b >= LAG:
                        emit_transposes(b - LAG)
            emit_transposes(n_banks - 1)
            assert state["tile"] == ntp and state["round"] == len(bounds)
            if ROUNDEND:
                for r, t0 in enumerate(bounds):
                    t1 = bounds[r + 1] if r + 1 < len(bounds) else ntp
                    emit_round(r, t0, t1, state["tps"][r])
    nc.compile()
    return nc


# ---------------------------------------------------------------- runners
def _run(nc, in_maps, trace):
    if os.environ.get("GAT_SIM"):
        from concourse.bass_interp import CoreSim
        results = []
        for m in in_maps:
            sim = CoreSim(nc, require_finite=False, require_nnan=False)
            for k, v in m.items():
                sim.tensor(k)[:] = v
            sim.simulate()
            outs = {}
            for alloc in nc.m.functions[0].allocations:
                if getattr(alloc, 'kind', None) == "ExternalOutput":
                    name = alloc.memorylocations[0].name
                    outs[name] = np.array(sim.tensor(name))
            results.append(outs)
        class R: pass
        r = R(); r.results = results; r.exec_time_ns = None
        return r
    return run_bass_kernel_spmd(nc, in_maps, core_ids=list(range(N_CORES)),
                                trace=trace)


def kernel(x, edge_index, W, att_src, att_dst, bias):
    global LAST_RESULTS, _LAST_NCS
    x = np.asarray(x, np.float32)
    edge_index = np.asarray(edge_index)
    W = np.asarray(W, np.float32)
    att_src = np.asarray(att_src, np.float32)
    att_dst = np.asarray(att_dst, np.float32)
    bias_np = np.asarray(bias, np.float32)

    N, C_in = x.shape
    C_out = W.shape[1]
    assert C_in == 128 and C_out == 32, (C_in, C_out)
    trace = bool(os.environ.get("GAT_TRACE"))

    loops = np.arange(N, dtype=np.int64)
    src = np.concatenate([edge_index[0].astype(np.int64), loops])
    dst = np.concatenate([edge_index[1].astype(np.int64), loops])

    Nc, prof, nb, rank_of_slot, col_of_slot, final_col, cores = \
        _plan(dst, N, N_CORES)
    nch = -(-Nc // 128)

    # ---- phase 1: project every node once (node-partitioned) ----
    ws = W @ att_src
    wd = W @ att_dst
    wext = np.concatenate([W, ws[:, None], wd[:, None]], 1).astype(NPBF16)
    in1 = []
    for c in range(N_CORES):
        xt = np.zeros((128, nch * 128), NPBF16)
        n0 = c * Nc
        xt[:, :Nc] = x[n0:n0 + Nc].T.astype(NPBF16)
        in1.append({"xt": xt, "wext": wext})

    key1 = ("proj", nch)
    if key1 not in _NC_CACHE:
        _NC_CACHE[key1] = _build_proj(nch)
    nc1 = _NC_CACHE[key1]
    res1 = _run(nc1, in1, trace)

    h_all = np.zeros((N, 32), NPBF16)
    as_all = np.zeros(N, NPBF16)
    ad_all = np.zeros(N, NPBF16)
    for c in range(N_CORES):
        o = res1.results[c]
        h = np.asarray(o["h_out"]).reshape(128, nch, 32) \
            .transpose(1, 0, 2).reshape(nch * 128, 32)[:Nc]
        a = np.asarray(o["a_out"]).reshape(128, nch, 2) \
            .transpose(1, 0, 2).reshape(nch * 128, 2)[:Nc]
        sl = slice(c * Nc, (c + 1) * Nc)
        h_all[sl] = h
        as_all[sl], ad_all[sl] = a[:, 0], a[:, 1]

    # ---- host gather: per-core streams (pure indexing) ----
    h_pad = np.concatenate([h_all, np.zeros((1, 32), NPBF16)], 0)
    as_pad = np.concatenate([as_all, [NPBF16(-300.0)]])
    ad_pad = np.concatenate([ad_all, [NPBF16(-300.0)]])

    z_np = np.broadcast_to(
        col_of_slot.reshape(nb, 128).T.astype(NPBF16), (128, nb)).copy()
    consts = np.zeros((128, D + 33 + 32), NPBF16)
    consts[:, 0:D] = np.arange(D, dtype=np.float32).astype(NPBF16)[None, :]
    idb = np.eye(33, dtype=np.float32)
    idb[32, 0:32] = bias_np          # transpose-matmul folds den*bias in
    consts[0:33, D:D + 33] = idb.astype(NPBF16)
    consts[:, D + 33:] = bias_np.astype(NPBF16)[None, :]

    ncols = nb * D
    ntp = -(-ncols // 128)

    in2 = []
    for c in range(N_CORES):
        src_of_slot, dst_of_slot = _core_slots(
            cores[c], prof, rank_of_slot, len(src), Nc, c)
        sid = np.where(src_of_slot >= 0, src[np.maximum(src_of_slot, 0)], N)
        did = np.where(dst_of_slot >= 0, dst_of_slot, N)
        he = np.empty((128, nb * 32), NPBF16)
        he3 = he.reshape(128, nb, 32)
        hrows = h_pad[sid].reshape(nb, 128, 32)
        he3[:, :, :] = hrows.transpose(1, 0, 2)
        a_st = np.empty((128, 2 * nb), NPBF16)
        a_st[:, 0:nb] = as_pad[sid].reshape(nb, 128).T
        a_st[:, nb:2 * nb] = ad_pad[did].reshape(nb, 128).T
        in2.append({"he": he, "a_st": a_st, "z_st": z_np, "consts": consts})

    key2 = ("agg", nb, LAG, WARM, ACCB, EVAC_DVE, WSPLIT, EQPOOL, ROUNDEND, BIASFOLD, OUT_EVERY, HEBUFS)
    if key2 not in _NC_CACHE:
        _NC_CACHE[key2] = _build_agg(nb)
    nc2 = _NC_CACHE[key2]
    res2 = _run(nc2, in2, trace)
    LAST_RESULTS = (res1, res2)
    _LAST_NCS = (nc1, nc2)

    out_full = np.zeros((N, C_out), np.float32)
    fc = final_col                       # rank -> global col id
    tpi = fc // 128                      # transpose tile
    tpp = fc % 128                       # partition within tile
    for c in range(N_CORES):
        o = np.asarray(res2.results[c]["out"]).astype(np.float32)
        o3 = o.reshape(128, ntp, 32)
        vals = o3[tpp, tpi, :]           # [n_ranks, 32]
        perm = cores[c][3]
        out_full[c * Nc + perm] = vals
    return out_full


# revision 13
# speedup vs baseline: 1.0558x; 1.0072x over previous
"""GAT encoder (PyG GATConv-style, single head) for Trainium2, 8 NeuronCores.

v3: segment-sum as PE matmuls with on-device selection masks.

  Phase 1 (proj): node-partitioned x @ [W | W@att_src | W@att_dst] -> per
  node h (32, bf16) and logits a_s, a_d (bf16).  No activations.

  Host (pure indexing): destinations degree-sorted per core against a
  COMMON degree profile (elementwise max across cores, +0.6% slots) so all
  8 cores share one program.  Edge slots laid dst-major into 128-slot
  windows; each window owns D=13 psum columns (col 0 = carry for a dst
  straddling from the previous window, straddler sits in col D-1; carries
  never cross a 39-window psum bank).

  Phase 2 (agg) per core, DMA-bound at ~17MB (h-stream 14.1MB @360B/ns):
    num  = exp(max(a, 0.2a)), a = a_s[src]+a_d[dst]      (DVE + ACT exp)
    W    = num * (z == j) built as D per-plane tensor_scalar/tensor_tensor
           ops (is_equal planes split DVE/Pool, mults DVE at 2x; D-major
           layout keeps every op densely packed -> 2x mode)
    per window w: two matmuls (stationary ldweights is free in the cost
    model; Matmult cost = out free size only):
      psum[0:32, wD:(w+1)D]  = he_w.T @ W_w    (h gathered per edge slot)
      psum[32:33, wD:(w+1)D] = ones.T @ W_w    (den row, aligned tile pos)
    per bank: ACT-copy psum -> acc_sb bf16; Pool merges window carries.
    PE re-matmul [33,128] acc tiles against [identity | bias-row] ->
    transposed dst-major psum tiles with den*bias folded in
    ((acc + den*bias)/den == acc/den + bias), then per-15-tile rounds:
    DVE rcp(den) + multiply, ACT sigmoid, batched out DMA.

  Host unshards via the rank->column map.  bf16 streams, f32 psum
  accumulate; rel err ~9e-3 vs the 2e-2 gate (fp8 h measured 2.9e-2).
"""
import os
import sys

for _p in ('/opt/trn_rl_repo',):
    if _p not in sys.path and os.path.isdir(_p):
        sys.path.insert(0, _p)

import numpy as np
import ml_dtypes

import concourse.mybir as mybir
import concourse.tile as tile
from concourse import bacc
from concourse.bass_utils import run_bass_kernel_spmd

F32 = mybir.dt.float32
BF16 = mybir.dt.bfloat16
NPBF16 = ml_dtypes.bfloat16

N_CORES = 8
PSUM_CHUNK = 15          # proj: 15*34 = 510 <= 512 f32 per PSUM bank
CW = 34                  # projected width: 32 h + a_s + a_d
D = 10                   # psum columns per 128-slot window
BANK_WINS = 51           # windows per psum bank (51*10 = 510 <= 512)
TP_TILES = 15            # [128,33] bf16 tiles per epilogue round, stride 34
HE_CHUNK_BANKS = 2       # he DMA granularity (banks per DMA)
LAG = int(os.environ.get("GAT_LAG", "0"))    # transpose lag (banks); 0 = all at end
ACCB = int(os.environ.get("GAT_ACCB", "6"))  # psum banks for accumulation
EVAC_DVE = int(os.environ.get("GAT_EVACDVE", "0"))  # every Nth evac on DVE (0=ACT only)
WSPLIT = int(os.environ.get("GAT_WSPLIT", "0"))  # Pool planes (0 = j%2 eq split, mults DVE)
EQPOOL = int(os.environ.get("GAT_EQPOOL", "0"))  # eq planes on Pool (only if WSPLIT=0; 0 = j%2)
ROUNDEND = int(os.environ.get("GAT_ROUNDEND", "0"))  # 1 = epilogue rounds after loop
BIASFOLD = int(os.environ.get("GAT_BIASFOLD", "1"))  # 1 = bias via transpose matmul
OUT_EVERY = int(os.environ.get("GAT_OUTEVERY", "4"))  # rounds per out DMA
WARM = int(os.environ.get("GAT_WARM", "0"))  # fine-grained W warmup chunks
HEBUFS = int(os.environ.get("GAT_HEBUFS", "6"))  # he stream buffers

LAST_RESULTS = None
_NC_CACHE = {}
_LAST_NCS = ()


def sim_exec_time_ns():
    """Sum of TimelineSim estimates for the programs run by kernel()."""
    from concourse.timeline_sim import TimelineSim
    return int(sum(TimelineSim(nc, trace=False).simulate()
                   for nc in _LAST_NCS))


# ---------------------------------------------------------------- planning
def _profile_plan(prof):
    """Pack the common degree profile into 128-slot windows.

    Protocol: per window, col 0 is reserved for a carry (continuation of the
    previous window's straddling dst), new dsts take cols 1..D-2, and a dst
    that straddles into the next window takes col D-1 (its continuation gets
    col 0 there).  The last window of each 42-window bank is padded so no
    carry crosses a psum bank.

    Returns (nb, rank_of_slot[nb*128], col_of_slot[nb*128],
    final_col[n_ranks] (global col id w*D+col)).
    """
    n = len(prof)
    rank_of_slot = []
    col_of_slot = []
    final_col = np.zeros(n, np.int64)
    w = 0          # current window index
    p = 0          # next free slot in window
    newd = 0       # new dsts started in this window

    def close_window():
        nonlocal w, p, newd
        pad = 128 - p
        rank_of_slot.extend([-1] * pad)
        col_of_slot.extend([0] * pad)
        w += 1
        p = 0
        newd = 0

    # visit ranks big/small interleaved so the new-dst cap (D-2 per
    # window) never closes a half-empty window in the small-degree tail
    lo, hi = 0, n - 1
    visit = []
    flip = True
    while lo <= hi:
        if flip:
            visit.append(lo); lo += 1
        else:
            visit.append(hi); hi -= 1
        flip = not flip
    for r in visit:
        d = int(prof[r])
        assert d >= 1
        while True:
            if p >= 128:
                close_window()
                continue
            if newd >= D - 2:
                close_window()
                continue
            space = 128 - p
            straddle = d > space
            if straddle and (w % BANK_WINS) == BANK_WINS - 1:
                # no carry across banks: pad and start in next bank
                close_window()
                continue
            break
        newd += 1
        if straddle:
            rank_of_slot.extend([r] * space)
            col_of_slot.extend([D - 1] * space)
            p = 128
            close_window()
            rest = d - space
            assert rest <= 128
            rank_of_slot.extend([r] * rest)
            col_of_slot.extend([0] * rest)
            p = rest
            final_col[r] = w * D + 0
        else:
            col = newd  # 1..D-2
            rank_of_slot.extend([r] * d)
            col_of_slot.extend([col] * d)
            p += d
            final_col[r] = w * D + col
    if p > 0:
        close_window()
    nb = w
    return (nb, np.array(rank_of_slot, np.int64),
            np.array(col_of_slot, np.int64), final_col)


def _plan(dst, N, n_cores):
    """Common profile + per-core degree-sorted dst orders and edge lists."""
    Nc = N // n_cores
    assert Nc * n_cores == N
    degs = np.zeros((n_cores, Nc), np.int64)
    cores = []
    for c in range(n_cores):
        sel = (dst >= c * Nc) & (dst < (c + 1) * Nc)
        idx = np.nonzero(sel)[0]
        d_c = dst[idx] - c * Nc
        order = np.argsort(d_c, kind='stable')
        eidx_sorted = idx[order]            # edge ids grouped by local dst
        counts = np.bincount(d_c, minlength=Nc).astype(np.int64)
        offsets = np.zeros(Nc + 1, np.int64)
        np.cumsum(counts, out=offsets[1:])
        perm = np.argsort(-counts, kind='stable')   # rank -> local dst
        degs[c] = counts[perm]
        cores.append((eidx_sorted, counts, offsets, perm))
    prof = degs.max(axis=0)
    assert prof[-1] >= 1 and prof[0] <= 128
    nb, rank_of_slot, col_of_slot, final_col = _profile_plan(prof)
    return Nc, prof, nb, rank_of_slot, col_of_slot, final_col, cores


def _core_slots(core_plan, prof, rank_of_slot, n_edges, Nc, c):
    """Per-core (src_of_slot, dst_of_slot) with -1 for padding slots."""
    eidx_sorted, counts, offsets, perm = core_plan
    nslots = len(rank_of_slot)
    src_of_slot = np.full(nslots, -1, np.int64)
    dst_of_slot = np.full(nslots, -1, np.int64)
    # slot positions per rank, in slot order
    pos = np.nonzero(rank_of_slot >= 0)[0]
    rk = rank_of_slot[pos]
    # index of each slot within its rank (0..prof[r]-1), slots of a rank
    # appear in increasing slot order
    order = np.argsort(rk, kind='stable')
    within = np.zeros(len(rk), np.int64)
    cum = np.zeros(len(prof) + 1, np.int64)
    np.cumsum(prof, out=cum[1:])
    within[order] = np.arange(len(rk)) - cum[rk[order]]
    ldst = perm[rk]                        # local dst of each real slot
    k = within
    valid = k < counts[ldst]
    epos = offsets[ldst[valid]] + k[valid]
    src_of_slot[pos[valid]] = -2           # placeholder, filled below
    sv = np.full(len(rk), -1, np.int64)
    sv[valid] = eidx_sorted[epos]
    src_of_slot[pos] = sv                  # edge id per slot (-1 pad)
    dst_of_slot[pos[valid]] = ldst[valid] + c * Nc
    return src_of_slot, dst_of_slot


# ---------------------------------------------------------------- phase 1
def _build_proj(nch):
    nc = bacc.Bacc("TRN2", target_bir_lowering=False, debug=False,
                   num_devices=N_CORES)
    xt = nc.dram_tensor("xt", [128, nch * 128], BF16, kind="ExternalInput").ap()
    wext = nc.dram_tensor("wext", [128, CW], BF16, kind="ExternalInput").ap()
    h_out = nc.dram_tensor("h_out", [128, nch * 32], BF16,
                           kind="ExternalOutput").ap()
    a_out = nc.dram_tensor("a_out", [128, nch * 2], BF16,
                           kind="ExternalOutput").ap()
    N_IN_DMA = 4
    with tile.TileContext(nc) as tc:
        with (
            tc.tile_pool(name="const", bufs=1) as cpool,
            tc.tile_pool(name="ps", bufs=8, space="PSUM") as pspool,
        ):
            wsb = cpool.tile([128, CW], BF16)
            xc = cpool.tile([128, nch * 128], BF16)
            qs = [0]
            left = nch
            for frac in (0.40, 0.30, 0.20, 0.10):
                qs.append(min(nch, qs[-1] + max(1, int(round(nch * frac)))))
            qs[-1] = nch
            for i, (k, k1) in enumerate(zip(qs[:-1], qs[1:])):
                if k1 > k:
                    nc.sync.dma_start(xc[:, k * 128:k1 * 128],
                                      xt[:, k * 128:k1 * 128])
                if i == 0:
                    nc.sync.dma_start(wsb[:], wext[:])
            hsb = cpool.tile([128, nch * 32], BF16)
            asd = cpool.tile([128, nch * 2], BF16)
            n_chunks = -(-nch // PSUM_CHUNK)
            marks = [(n_chunks * 4) // 8, (n_chunks * 6) // 8, n_chunks - 1, n_chunks]
            flush = [0] + sorted(set(min(m * PSUM_CHUNK, nch) for m in marks))
            b0 = 0
            while b0 < nch:
                cn = min(PSUM_CHUNK, nch - b0)
                ps = pspool.tile([128, PSUM_CHUNK * CW], F32, tag="ps")
                for j in range(b0, b0 + cn):
                    nc.tensor.matmul(
                        ps[:, (j - b0) * CW:(j - b0 + 1) * CW],
                        xc[:, j * 128:(j + 1) * 128],
                        wsb[:], start=True, stop=True)
                psv = ps[:, :cn * CW].rearrange("p (s f) -> p s f", f=CW)
                if (b0 // PSUM_CHUNK) % 2 == 0:
                    nc.scalar.activation(
                        hsb[:, b0 * 32:(b0 + cn) * 32]
                        .rearrange("p (s c) -> p s c", c=32),
                        psv[:, :, 0:32],
                        mybir.ActivationFunctionType.Copy)
                else:
                    nc.vector.tensor_copy(
                        out=hsb[:, b0 * 32:(b0 + cn) * 32]
                        .rearrange("p (s c) -> p s c", c=32),
                        in_=psv[:, :, 0:32])
                nc.vector.tensor_copy(
                    out=asd[:, b0 * 2:(b0 + cn) * 2]
                    .rearrange("p (s c) -> p s c", c=2),
                    in_=psv[:, :, 32:34])
                b1 = b0 + cn
                # flush h_out at staged boundaries (earlier pieces bigger)
                for lo, hi in zip(flush[:-1], flush[1:]):
                    if b0 < hi <= b1:
                        if hi == nch:
                            nc.sync.dma_start(a_out[:], asd[:])
                        nc.scalar.dma_start(h_out[:, lo * 32:hi * 32],
                                            hsb[:, lo * 32:hi * 32])
                b0 = b1
    nc.compile()
    return nc


# ---------------------------------------------------------------- phase 2
def _build_agg(nb):
    n_banks = -(-nb // BANK_WINS)
    ncols = nb * D
    ntp = -(-ncols // 128)                # transpose tiles
    nc = bacc.Bacc("TRN2", target_bir_lowering=False, debug=False,
                   num_devices=N_CORES)
    he = nc.dram_tensor("he", [128, nb * 32], BF16, kind="ExternalInput").ap()
    a_st = nc.dram_tensor("a_st", [128, 3 * nb], BF16,
                          kind="ExternalInput").ap()
    consts = nc.dram_tensor("consts", [128, D + 33 + 32], BF16,
                            kind="ExternalInput").ap()
    out = nc.dram_tensor("out", [128, ntp * 32], BF16,
                         kind="ExternalOutput").ap()
    # epilogue round boundaries (tiles); last rounds smaller for the tail
    bounds = list(range(0, ntp, TP_TILES))
    if len(bounds) >= 2 and ntp - bounds[-1] > 6:
        bounds = bounds[:-1] + [ntp - 12, ntp - 6]
    elif ntp > 6:
        bounds = bounds[:-1] + [max(0, ntp - 6)]
    bounds = sorted(set(b for b in bounds if b < ntp))
    with tile.TileContext(nc) as tc:
        with (
            tc.tile_pool(name="const", bufs=1) as cpool,
            tc.tile_pool(name="hec", bufs=HEBUFS) as hepool,
            tc.tile_pool(name="acc", bufs=ACCB, space="PSUM") as accpool,
            tc.tile_pool(name="tp", bufs=8 - ACCB, space="PSUM") as tppool,
        ):
            # ---- constants + small streams (sync queue: ordered first)
            cst = cpool.tile([128, D + 33 + 32], BF16)
            nc.sync.dma_start(cst[:], consts[:])
            ident = cst[:, D:D + 33]      # identity in partitions 0..32
            bias_sb = cst[:, D + 33:D + 33 + 32]
            ones_sb = cpool.tile([128, 1], BF16)
            nc.gpsimd.memset(ones_sb[:], 1.0)
            ac = cpool.tile([128, 3 * nb], BF16)
            nc.sync.dma_start(ac[:], a_st[:])
            zc = ac[:, 2 * nb:3 * nb]
            # ---- num = exp(max(a, 0.2a))  [128, nb]
            num = cpool.tile([128, nb], BF16)
            wk = cpool.tile([128, nb], BF16)
            nc.vector.tensor_tensor(out=wk[:], in0=ac[:, 0:nb],
                                    in1=ac[:, nb:2 * nb],
                                    op=mybir.AluOpType.add)
            nc.vector.tensor_scalar(out=num[:], in0=wk[:], scalar1=0.2,
                                    scalar2=None, op0=mybir.AluOpType.mult)
            nc.vector.tensor_tensor(out=wk[:], in0=wk[:], in1=num[:],
                                    op=mybir.AluOpType.max)
            nc.scalar.activation(num[:], wk[:],
                                 mybir.ActivationFunctionType.Exp, scale=1.0)
            # ---- W[p, j, b] = num[p, b] * (z[p, b] == j), D-major
            wsel = cpool.tile([128, D * nb], BF16)
            w3 = wsel[:].rearrange("p (d b) -> p d b", b=nb)
            NCHUNK = 6
            cb = -(-nb // NCHUNK)
            wstate = {"done": 0, "warm": WARM}

            def emit_w_chunk():
                s0 = wstate["done"]
                if s0 >= nb:
                    return
                if wstate["warm"] > 0:
                    wstate["warm"] -= 1
                    s1 = min(s0 + BANK_WINS, nb)
                else:
                    s1 = min(s0 + cb, nb)
                for j in range(D):
                    if WSPLIT:
                        eng = nc.gpsimd if j >= D - WSPLIT else nc.vector
                        eng.tensor_scalar(
                            out=w3[:, j, s0:s1], in0=zc[:, s0:s1],
                            scalar1=float(j), scalar2=None,
                            op0=mybir.AluOpType.is_equal)
                        eng.tensor_tensor(
                            out=w3[:, j, s0:s1], in0=w3[:, j, s0:s1],
                            in1=num[:, s0:s1], op=mybir.AluOpType.mult)
                    else:
                        if EQPOOL:
                            eng = nc.gpsimd if j < EQPOOL else nc.vector
                        else:
                            eng = nc.vector if j % 2 == 0 else nc.gpsimd
                        eng.tensor_scalar(
                            out=w3[:, j, s0:s1], in0=zc[:, s0:s1],
                            scalar1=float(j), scalar2=None,
                            op0=mybir.AluOpType.is_equal)
                        nc.vector.tensor_tensor(
                            out=w3[:, j, s0:s1], in0=w3[:, j, s0:s1],
                            in1=num[:, s0:s1], op=mybir.AluOpType.mult)
                wstate["done"] = s1

            for _ in range(4):
                emit_w_chunk()
            # ---- streaming accumulate + interleaved epilogue
            acc_sb = cpool.tile([128, ntp * 128], BF16)
            if ntp * 128 > ncols:
                nc.gpsimd.memset(acc_sb[0:33, ncols:ntp * 128], 0.0)
            out_sb = cpool.tile([128, ntp * 32], BF16)
            rcp = cpool.tile([128, ntp], BF16)
            state = {"tile": 0, "round": 0, "odma": []}

            def emit_transposes(bank_done):
                """Emit transposes fully covered by merged banks <= bank_done."""
                max_t = min(ntp, ((bank_done + 1) * BANK_WINS * D) // 128)
                if bank_done >= n_banks - 1:
                    max_t = ntp
                while state["tile"] < max_t:
                    t = state["tile"]
                    r = state["round"]
                    t0 = bounds[r]
                    if r not in state["tps"]:
                        if BIASFOLD:
                            tp_r = tppool.tile([128, TP_TILES * 33], F32,
                                               tag="tp")
                        else:
                            tp_r = tppool.tile([128, TP_TILES * 34], BF16,
                                               tag="tp")
                        state["tps"][r] = tp_r
                    tp = state["tps"][r]
                    if BIASFOLD:
                        # regular matmul against [identity | bias row]:
                        # transposed acc with den*bias folded in
                        # ((acc + den*bias)*rcp == acc*rcp + bias)
                        nc.tensor.matmul(
                            tp[:, (t - t0) * 33:(t - t0) * 33 + 33],
                            acc_sb[0:33, t * 128:(t + 1) * 128],
                            ident[0:33, 0:33],
                            start=True, stop=True)
                    else:
                        nc.tensor.transpose(
                            tp[:, (t - t0) * 34:(t - t0) * 34 + 33],
                            acc_sb[0:33, t * 128:(t + 1) * 128],
                            ident[0:33, 0:33])
                    state["tile"] = t + 1
                    t1 = bounds[r + 1] if r + 1 < len(bounds) else ntp
                    if t + 1 == t1:
                        if not ROUNDEND:
                            emit_round(r, t0, t1, state["tps"][r])
                        state["round"] = r + 1

            def emit_round(r, t0, t1, tp):
                cw = 33 if BIASFOLD else 34
                tpv = tp[:, :(t1 - t0) * cw] \
                    .rearrange("p (t c) -> p t c", c=cw)
                with nc.allow_low_precision(reason="1/den bf16"):
                    nc.vector.reciprocal(rcp[:, t0:t1], tpv[:, :, 32])
                ov = out_sb[:, t0 * 32:t1 * 32] \
                    .rearrange("p (t c) -> p t c", c=32)
                nc.vector.tensor_tensor(
                    out=ov, in0=tpv[:, :, 0:32],
                    in1=rcp[:, t0:t1].rearrange("p (t o) -> p t o", o=1)
                    .to_broadcast([128, t1 - t0, 32]),
                    op=mybir.AluOpType.mult)
                if not BIASFOLD:
                    nc.vector.tensor_tensor(
                        out=ov, in0=ov,
                        in1=bias_sb.rearrange("p (o c) -> p o c", o=1)
                        .to_broadcast([128, t1 - t0, 32]),
                        op=mybir.AluOpType.add)
                nc.scalar.activation(out_sb[:, t0 * 32:t1 * 32],
                                     out_sb[:, t0 * 32:t1 * 32],
                                     mybir.ActivationFunctionType.Sigmoid)
                state["odma"].append((t0, t1))
                flush = (r % OUT_EVERY == OUT_EVERY - 1
                         or t1 >= ntp)
                if flush:
                    o0 = state["odma"][0][0]
                    o1 = state["odma"][-1][1]
                    state["odma"] = []
                    nc.scalar.dma_start(out[:, o0 * 32:o1 * 32],
                                        out_sb[:, o0 * 32:o1 * 32])

            # tp tiles must be allocated per round; pre-wire creation order
            state["tps"] = {}
            for s0 in range(0, nb, HE_CHUNK_BANKS * BANK_WINS):
                s1 = min(s0 + HE_CHUNK_BANKS * BANK_WINS, nb)
                hc = hepool.tile([128, HE_CHUNK_BANKS * BANK_WINS * 32], BF16,
                                 tag="hec")
                nc.sync.dma_start(hc[:, :(s1 - s0) * 32],
                                  he[:, s0 * 32:s1 * 32])
                # keep the on-device W build ~3 he-chunks ahead of the
                # matmul stream so merges queue promptly behind it
                if wstate["done"] < min(nb, s1 + 3 * HE_CHUNK_BANKS * BANK_WINS):
                    emit_w_chunk()
                for b in range(s0 // BANK_WINS,
                               s0 // BANK_WINS + HE_CHUNK_BANKS):
                    if b >= n_banks:
                        break
                    while wstate["done"] < min(nb, (b + 1) * BANK_WINS):
                        emit_w_chunk()
                    w0 = b * BANK_WINS
                    w1 = min(w0 + BANK_WINS, nb)
                    ap = accpool.tile([128, 512], F32, tag="acc")
                    for w in range(w0, min(w0 + BANK_WINS, nb)):
                        lw = w - s0
                        nc.tensor.matmul(
                            ap[0:32, (w - w0) * D:(w - w0 + 1) * D],
                            hc[:, lw * 32:(lw + 1) * 32],
                            w3[:, :, w],
                            start=True, stop=True)
                        nc.tensor.matmul(
                            ap[32:33, (w - w0) * D:(w - w0 + 1) * D],
                            ones_sb[:], w3[:, :, w],
                            start=True, stop=True)
                    # evacuate bank -> acc_sb (mostly ACT; Copy is in
                    # every act table set so no reloads)
                    if EVAC_DVE and b % EVAC_DVE == EVAC_DVE - 1:
                        nc.vector.tensor_copy(
                            out=acc_sb[0:33, w0 * D:w1 * D],
                            in_=ap[0:33, 0:(w1 - w0) * D])
                    else:
                        nc.scalar.activation(
                            acc_sb[0:33, w0 * D:w1 * D],
                            ap[0:33, 0:(w1 - w0) * D],
                            mybir.ActivationFunctionType.Copy)
                    # merge carries within the bank (Pool, sbuf only)
                    if w1 - w0 > 1:
                        a3o = acc_sb[0:33, w0 * D + D:w1 * D] \
                            .rearrange("p (b d) -> p b d", d=D)
                        a3i = acc_sb[0:33, w0 * D + D - 1:w1 * D - 1] \
                            .rearrange("p (b d) -> p b d", d=D)
                        nc.gpsimd.tensor_tensor(
                            out=a3o[:, :, 0:1], in0=a3o[:, :, 0:1],
                            in1=a3i[:, :, 0:1], op=mybir.AluOpType.add)
                    # interleave transposes/epilogue with a lag so their
                    # evac/merge deps are long resolved by the time in-order
                    # PE.SEQ reaches them
                    if LAG > 0 and b >= LAG:
                        emit_transposes(b - LAG)
            emit_transposes(n_banks - 1)
            assert state["tile"] == ntp and state["round"] == len(bounds)
            if ROUNDEND:
                for r, t0 in enumerate(bounds):
                    t1 = bounds[r + 1] if r + 1 < len(bounds) else ntp
                    emit_round(r, t0, t1, state["tps"][r])
    nc.compile()
    return nc


# ---------------------------------------------------------------- runners
def _run(nc, in_maps, trace):
    if os.environ.get("GAT_SIM"):
        from concourse.bass_interp import CoreSim
        results = []
        for m in in_maps:
            sim = CoreSim(nc, require_finite=False, require_nnan=False)
            for k, v in m.items():
                sim.tensor(k)[:] = v
            sim.simulate()
            outs = {}
            for alloc in nc.m.functions[0].allocations:
                if getattr(alloc, 'kind', None) == "ExternalOutput":
                    name = alloc.memorylocations[0].name
                    outs[name] = np.array(sim.tensor(name))
            results.append(outs)
        class R: pass
        r = R(); r.results = results; r.exec_time_ns = None
        return r
    return run_bass_kernel_spmd(nc, in_maps, core_ids=list(range(N_CORES)),
                                trace=trace)


def kernel(x, edge_index, W, att_src, att_dst, bias):
    global LAST_RESULTS, _LAST_NCS
    x = np.asarray(x, np.float32)
    edge_index = np.asarray(edge_index)
    W = np.asarray(W, np.float32)
    att_src = np.asarray(att_src, np.float32)
    att_dst = np.asarray(att_dst, np.float32)
    bias_np = np.asarray(bias, np.float32)

    N, C_in = x.shape
    C_out = W.shape[1]
    assert C_in == 128 and C_out == 32, (C_in, C_out)
    trace = bool(os.environ.get("GAT_TRACE"))

    loops = np.arange(N, dtype=np.int64)
    src = np.concatenate([edge_index[0].astype(np.int64), loops])
    dst = np.concatenate([edge_index[1].astype(np.int64), loops])

    Nc, prof, nb, rank_of_slot, col_of_slot, final_col, cores = \
        _plan(dst, N, N_CORES)
    nch = -(-Nc // 128)

    # ---- phase 1: project every node once (node-partitioned) ----
    ws = W @ att_src
    wd = W @ att_dst
    wext = np.concatenate([W, ws[:, None], wd[:, None]], 1).astype(NPBF16)
    in1 = []
    for c in range(N_CORES):
        xt = np.zeros((128, nch * 128), NPBF16)
        n0 = c * Nc
        xt[:, :Nc] = x[n0:n0 + Nc].T.astype(NPBF16)
        in1.append({"xt": xt, "wext": wext})

    key1 = ("proj", nch)
    if key1 not in _NC_CACHE:
        _NC_CACHE[key1] = _build_proj(nch)
    nc1 = _NC_CACHE[key1]
    res1 = _run(nc1, in1, trace)

    h_all = np.zeros((N, 32), NPBF16)
    as_all = np.zeros(N, NPBF16)
    ad_all = np.zeros(N, NPBF16)
    for c in range(N_CORES):
        o = res1.results[c]
        h = np.asarray(o["h_out"]).reshape(128, nch, 32) \
            .transpose(1, 0, 2).reshape(nch * 128, 32)[:Nc]
        a = np.asarray(o["a_out"]).reshape(128, nch, 2) \
            .transpose(1, 0, 2).reshape(nch * 128, 2)[:Nc]
        sl = slice(c * Nc, (c + 1) * Nc)
        h_all[sl] = h
        as_all[sl], ad_all[sl] = a[:, 0], a[:, 1]

    # ---- host gather: per-core streams (pure indexing) ----
    h_pad = np.concatenate([h_all, np.zeros((1, 32), NPBF16)], 0)
    as_pad = np.concatenate([as_all, [NPBF16(-300.0)]])
    ad_pad = np.concatenate([ad_all, [NPBF16(-300.0)]])

    z_np = np.broadcast_to(
        col_of_slot.reshape(nb, 128).T.astype(NPBF16), (128, nb)).copy()
    consts = np.zeros((128, D + 33 + 32), NPBF16)
    consts[:, 0:D] = np.arange(D, dtype=np.float32).astype(NPBF16)[None, :]
    idb = np.eye(33, dtype=np.float32)
    idb[32, 0:32] = bias_np          # transpose-matmul folds den*bias in
    consts[0:33, D:D + 33] = idb.astype(NPBF16)
    consts[:, D + 33:] = bias_np.astype(NPBF16)[None, :]

    ncols = nb * D
    ntp = -(-ncols // 128)

    in2 = []
    for c in range(N_CORES):
        src_of_slot, dst_of_slot = _core_slots(
            cores[c], prof, rank_of_slot, len(src), Nc, c)
        sid = np.where(src_of_slot >= 0, src[np.maximum(src_of_slot, 0)], N)
        did = np.where(dst_of_slot >= 0, dst_of_slot, N)
        he = np.empty((128, nb * 32), NPBF16)
        he3 = he.reshape(128, nb, 32)
        hrows = h_pad[sid].reshape(nb, 128, 32)
        he3[:, :, :] = hrows.transpose(1, 0, 2)
        a_st = np.empty((128, 3 * nb), NPBF16)
        a_st[:, 0:nb] = as_pad[sid].reshape(nb, 128).T
        a_st[:, nb:2 * nb] = ad_pad[did].reshape(nb, 128).T
        a_st[:, 2 * nb:3 * nb] = z_np
        in2.append({"he": he, "a_st": a_st, "consts": consts})

    key2 = ("agg", nb, LAG, WARM, ACCB, EVAC_DVE, WSPLIT, EQPOOL, ROUNDEND, BIASFOLD, OUT_EVERY, HEBUFS)
    if key2 not in _NC_CACHE:
        _NC_CACHE[key2] = _build_agg(nb)
    nc2 = _NC_CACHE[key2]
    res2 = _run(nc2, in2, trace)
    LAST_RESULTS = (res1, res2)
    _LAST_NCS = (nc1, nc2)

    out_full = np.zeros((N, C_out), np.float32)
    fc = final_col                       # rank -> global col id
    tpi = fc // 128                      # transpose tile
    tpp = fc % 128                       # partition within tile
    for c in range(N_CORES):
        o = np.asarray(res2.results[c]["out"]).astype(np.float32)
        o3 = o.reshape(128, ntp, 32)
        vals = o3[tpp, tpi, :]           # [n_ranks, 32]
        perm = cores[c][3]
        out_full[c * Nc + perm] = vals
    return out_full


# revision 14
# speedup vs baseline: 1.0646x; 1.0083x over previous
"""GAT encoder (PyG GATConv-style, single head) for Trainium2, 8 NeuronCores.

v3: segment-sum as PE matmuls with on-device selection masks.

  Phase 1 (proj): node-partitioned x @ [W | W@att_src | W@att_dst] -> per
  node h (32, bf16) and logits a_s, a_d (bf16).  No activations.

  Host (pure indexing): destinations degree-sorted per core against a
  COMMON degree profile (elementwise max across cores, +0.6% slots) so all
  8 cores share one program.  Edge slots laid dst-major into 128-slot
  windows; each window owns D=13 psum columns (col 0 = carry for a dst
  straddling from the previous window, straddler sits in col D-1; carries
  never cross a 39-window psum bank).

  Phase 2 (agg) per core, DMA-bound at ~17MB (h-stream 14.1MB @360B/ns):
    num  = exp(max(a, 0.2a)), a = a_s[src]+a_d[dst]      (DVE + ACT exp)
    W    = num * (z == j) built as D per-plane tensor_scalar/tensor_tensor
           ops (is_equal planes split DVE/Pool, mults DVE at 2x; D-major
           layout keeps every op densely packed -> 2x mode)
    per window w: two matmuls (stationary ldweights is free in the cost
    model; Matmult cost = out free size only):
      psum[0:32, wD:(w+1)D]  = he_w.T @ W_w    (h gathered per edge slot)
      psum[32:33, wD:(w+1)D] = ones.T @ W_w    (den row, aligned tile pos)
    per bank: ACT-copy psum -> acc_sb bf16; Pool merges window carries.
    PE re-matmul [33,128] acc tiles against [identity | bias-row] ->
    transposed dst-major psum tiles with den*bias folded in
    ((acc + den*bias)/den == acc/den + bias), then per-15-tile rounds:
    DVE rcp(den) + multiply, ACT sigmoid, batched out DMA.

  Host unshards via the rank->column map.  bf16 streams, f32 psum
  accumulate; rel err ~9e-3 vs the 2e-2 gate (fp8 h measured 2.9e-2).
"""
import os
import sys

for _p in ('/opt/trn_rl_repo',):
    if _p not in sys.path and os.path.isdir(_p):
        sys.path.insert(0, _p)

import numpy as np
import ml_dtypes

import concourse.mybir as mybir
import concourse.tile as tile
from concourse import bacc
from concourse.bass_utils import run_bass_kernel_spmd

F32 = mybir.dt.float32
BF16 = mybir.dt.bfloat16
NPBF16 = ml_dtypes.bfloat16

N_CORES = 8
PSUM_CHUNK = 15          # proj: 15*34 = 510 <= 512 f32 per PSUM bank
CW = 34                  # projected width: 32 h + a_s + a_d
D = 10                   # psum columns per 128-slot window
BANK_WINS = 51           # windows per psum bank (51*10 = 510 <= 512)
TP_TILES = 15            # [128,33] bf16 tiles per epilogue round, stride 34
HE_CHUNK_BANKS = 2       # he DMA granularity (banks per DMA)
LAG = int(os.environ.get("GAT_LAG", "0"))    # transpose lag (banks); 0 = all at end
ACCB = int(os.environ.get("GAT_ACCB", "6"))  # psum banks for accumulation
EVAC_DVE = int(os.environ.get("GAT_EVACDVE", "0"))  # every Nth evac on DVE (0=ACT only)
WSPLIT = int(os.environ.get("GAT_WSPLIT", "0"))  # Pool planes (0 = j%2 eq split, mults DVE)
EQPOOL = int(os.environ.get("GAT_EQPOOL", "0"))  # eq planes on Pool (only if WSPLIT=0; 0 = j%2)
ROUNDEND = int(os.environ.get("GAT_ROUNDEND", "0"))  # 1 = epilogue rounds after loop
BIASFOLD = int(os.environ.get("GAT_BIASFOLD", "1"))  # 1 = bias via transpose matmul
OUT_EVERY = int(os.environ.get("GAT_OUTEVERY", "4"))  # rounds per out DMA
WARM = int(os.environ.get("GAT_WARM", "0"))  # fine-grained W warmup chunks
HEBUFS = int(os.environ.get("GAT_HEBUFS", "6"))  # he stream buffers

LAST_RESULTS = None
_NC_CACHE = {}
_LAST_NCS = ()


def sim_exec_time_ns():
    """Sum of TimelineSim estimates for the programs run by kernel()."""
    from concourse.timeline_sim import TimelineSim
    return int(sum(TimelineSim(nc, trace=False).simulate()
                   for nc in _LAST_NCS))


# ---------------------------------------------------------------- planning
def _profile_plan(prof):
    """Pack the common degree profile into 128-slot windows.

    Protocol: per window, col 0 is reserved for a carry (continuation of the
    previous window's straddling dst), new dsts take cols 1..D-2, and a dst
    that straddles into the next window takes col D-1 (its continuation gets
    col 0 there).  The last window of each 42-window bank is padded so no
    carry crosses a psum bank.

    Returns (nb, rank_of_slot[nb*128], col_of_slot[nb*128],
    final_col[n_ranks] (global col id w*D+col)).
    """
    n = len(prof)
    rank_of_slot = []
    col_of_slot = []
    final_col = np.zeros(n, np.int64)
    w = 0          # current window index
    p = 0          # next free slot in window
    newd = 0       # new dsts started in this window

    def close_window():
        nonlocal w, p, newd
        pad = 128 - p
        rank_of_slot.extend([-1] * pad)
        col_of_slot.extend([0] * pad)
        w += 1
        p = 0
        newd = 0

    # visit ranks big/small interleaved so the new-dst cap (D-2 per
    # window) never closes a half-empty window in the small-degree tail
    lo, hi = 0, n - 1
    visit = []
    flip = True
    while lo <= hi:
        if flip:
            visit.append(lo); lo += 1
        else:
            visit.append(hi); hi -= 1
        flip = not flip
    for r in visit:
        d = int(prof[r])
        assert d >= 1
        while True:
            if p >= 128:
                close_window()
                continue
            if newd >= D - 2:
                close_window()
                continue
            space = 128 - p
            straddle = d > space
            if straddle and (w % BANK_WINS) == BANK_WINS - 1:
                # no carry across banks: pad and start in next bank
                close_window()
                continue
            break
        newd += 1
        if straddle:
            rank_of_slot.extend([r] * space)
            col_of_slot.extend([D - 1] * space)
            p = 128
            close_window()
            rest = d - space
            assert rest <= 128
            rank_of_slot.extend([r] * rest)
            col_of_slot.extend([0] * rest)
            p = rest
            final_col[r] = w * D + 0
        else:
            col = newd  # 1..D-2
            rank_of_slot.extend([r] * d)
            col_of_slot.extend([col] * d)
            p += d
            final_col[r] = w * D + col
    if p > 0:
        close_window()
    nb = w
    return (nb, np.array(rank_of_slot, np.int64),
            np.array(col_of_slot, np.int64), final_col)


def _plan(dst, N, n_cores):
    """Common profile + per-core degree-sorted dst orders and edge lists."""
    Nc = N // n_cores
    assert Nc * n_cores == N
    degs = np.zeros((n_cores, Nc), np.int64)
    cores = []
    for c in range(n_cores):
        sel = (dst >= c * Nc) & (dst < (c + 1) * Nc)
        idx = np.nonzero(sel)[0]
        d_c = dst[idx] - c * Nc
        order = np.argsort(d_c, kind='stable')
        eidx_sorted = idx[order]            # edge ids grouped by local dst
        counts = np.bincount(d_c, minlength=Nc).astype(np.int64)
        offsets = np.zeros(Nc + 1, np.int64)
        np.cumsum(counts, out=offsets[1:])
        perm = np.argsort(-counts, kind='stable')   # rank -> local dst
        degs[c] = counts[perm]
        cores.append((eidx_sorted, counts, offsets, perm))
    prof = degs.max(axis=0)
    assert prof[-1] >= 1 and prof[0] <= 128
    nb, rank_of_slot, col_of_slot, final_col = _profile_plan(prof)
    return Nc, prof, nb, rank_of_slot, col_of_slot, final_col, cores


def _core_slots(core_plan, prof, rank_of_slot, n_edges, Nc, c):
    """Per-core (src_of_slot, dst_of_slot) with -1 for padding slots."""
    eidx_sorted, counts, offsets, perm = core_plan
    nslots = len(rank_of_slot)
    src_of_slot = np.full(nslots, -1, np.int64)
    dst_of_slot = np.full(nslots, -1, np.int64)
    # slot positions per rank, in slot order
    pos = np.nonzero(rank_of_slot >= 0)[0]
    rk = rank_of_slot[pos]
    # index of each slot within its rank (0..prof[r]-1), slots of a rank
    # appear in increasing slot order
    order = np.argsort(rk, kind='stable')
    within = np.zeros(len(rk), np.int64)
    cum = np.zeros(len(prof) + 1, np.int64)
    np.cumsum(prof, out=cum[1:])
    within[order] = np.arange(len(rk)) - cum[rk[order]]
    ldst = perm[rk]                        # local dst of each real slot
    k = within
    valid = k < counts[ldst]
    epos = offsets[ldst[valid]] + k[valid]
    src_of_slot[pos[valid]] = -2           # placeholder, filled below
    sv = np.full(len(rk), -1, np.int64)
    sv[valid] = eidx_sorted[epos]
    src_of_slot[pos] = sv                  # edge id per slot (-1 pad)
    dst_of_slot[pos[valid]] = ldst[valid] + c * Nc
    return src_of_slot, dst_of_slot


# ---------------------------------------------------------------- phase 1
def _build_proj(nch):
    nc = bacc.Bacc("TRN2", target_bir_lowering=False, debug=False,
                   num_devices=N_CORES)
    xt = nc.dram_tensor("xt", [128, CW + nch * 128], BF16,
                        kind="ExternalInput").ap()
    h_out = nc.dram_tensor("h_out", [128, nch * 32], BF16,
                           kind="ExternalOutput").ap()
    a_out = nc.dram_tensor("a_out", [128, nch * 2], BF16,
                           kind="ExternalOutput").ap()
    N_IN_DMA = 4
    with tile.TileContext(nc) as tc:
        with (
            tc.tile_pool(name="const", bufs=1) as cpool,
            tc.tile_pool(name="ps", bufs=8, space="PSUM") as pspool,
        ):
            xc = cpool.tile([128, CW + nch * 128], BF16)
            wsb = xc[:, 0:CW]
            qs = [0]
            left = nch
            for frac in (0.40, 0.30, 0.20, 0.10):
                qs.append(min(nch, qs[-1] + max(1, int(round(nch * frac)))))
            qs[-1] = nch
            for i, (k, k1) in enumerate(zip(qs[:-1], qs[1:])):
                if k1 > k:
                    lo = 0 if i == 0 else CW + k * 128
                    nc.sync.dma_start(xc[:, lo:CW + k1 * 128],
                                      xt[:, lo:CW + k1 * 128])
            hsb = cpool.tile([128, nch * 32], BF16)
            asd = cpool.tile([128, nch * 2], BF16)
            n_chunks = -(-nch // PSUM_CHUNK)
            marks = [(n_chunks * 4) // 8, (n_chunks * 6) // 8, n_chunks - 1, n_chunks]
            flush = [0] + sorted(set(min(m * PSUM_CHUNK, nch) for m in marks))
            b0 = 0
            while b0 < nch:
                cn = min(PSUM_CHUNK, nch - b0)
                ps = pspool.tile([128, PSUM_CHUNK * CW], F32, tag="ps")
                for j in range(b0, b0 + cn):
                    nc.tensor.matmul(
                        ps[:, (j - b0) * CW:(j - b0 + 1) * CW],
                        xc[:, CW + j * 128:CW + (j + 1) * 128],
                        wsb, start=True, stop=True)
                psv = ps[:, :cn * CW].rearrange("p (s f) -> p s f", f=CW)
                if (b0 // PSUM_CHUNK) % 2 == 0:
                    nc.scalar.activation(
                        hsb[:, b0 * 32:(b0 + cn) * 32]
                        .rearrange("p (s c) -> p s c", c=32),
                        psv[:, :, 0:32],
                        mybir.ActivationFunctionType.Copy)
                else:
                    nc.vector.tensor_copy(
                        out=hsb[:, b0 * 32:(b0 + cn) * 32]
                        .rearrange("p (s c) -> p s c", c=32),
                        in_=psv[:, :, 0:32])
                nc.vector.tensor_copy(
                    out=asd[:, b0 * 2:(b0 + cn) * 2]
                    .rearrange("p (s c) -> p s c", c=2),
                    in_=psv[:, :, 32:34])
                b1 = b0 + cn
                # flush h_out at staged boundaries (earlier pieces bigger)
                for lo, hi in zip(flush[:-1], flush[1:]):
                    if b0 < hi <= b1:
                        if hi == nch:
                            nc.sync.dma_start(a_out[:], asd[:])
                        nc.scalar.dma_start(h_out[:, lo * 32:hi * 32],
                                            hsb[:, lo * 32:hi * 32])
                b0 = b1
    nc.compile()
    return nc


# ---------------------------------------------------------------- phase 2
def _build_agg(nb):
    n_banks = -(-nb // BANK_WINS)
    ncols = nb * D
    ntp = -(-ncols // 128)                # transpose tiles
    nc = bacc.Bacc("TRN2", target_bir_lowering=False, debug=False,
                   num_devices=N_CORES)
    he = nc.dram_tensor("he", [128, nb * 32], BF16, kind="ExternalInput").ap()
    a_st = nc.dram_tensor("a_st", [128, 3 * nb + D + 33 + 32], BF16,
                          kind="ExternalInput").ap()
    out = nc.dram_tensor("out", [128, ntp * 32], BF16,
                         kind="ExternalOutput").ap()
    # epilogue round boundaries (tiles); last rounds smaller for the tail
    bounds = list(range(0, ntp, TP_TILES))
    if len(bounds) >= 2 and ntp - bounds[-1] > 6:
        bounds = bounds[:-1] + [ntp - 12, ntp - 6]
    elif ntp > 6:
        bounds = bounds[:-1] + [max(0, ntp - 6)]
    bounds = sorted(set(b for b in bounds if b < ntp))
    with tile.TileContext(nc) as tc:
        with (
            tc.tile_pool(name="const", bufs=1) as cpool,
            tc.tile_pool(name="hec", bufs=HEBUFS) as hepool,
            tc.tile_pool(name="acc", bufs=ACCB, space="PSUM") as accpool,
            tc.tile_pool(name="tp", bufs=8 - ACCB, space="PSUM") as tppool,
        ):
            # ---- one merged input transfer: a_s | a_d | z | consts
            ones_sb = cpool.tile([128, 1], BF16)
            nc.gpsimd.memset(ones_sb[:], 1.0)
            ac = cpool.tile([128, 3 * nb + D + 33 + 32], BF16)
            nc.sync.dma_start(ac[:], a_st[:])
            zc = ac[:, 2 * nb:3 * nb]
            cst = ac[:, 3 * nb:]
            ident = cst[:, D:D + 33]      # identity in partitions 0..32
            bias_sb = cst[:, D + 33:D + 33 + 32]
            # ---- num = exp(max(a, 0.2a))  [128, nb]
            num = cpool.tile([128, nb], BF16)
            wk = cpool.tile([128, nb], BF16)
            nc.vector.tensor_tensor(out=wk[:], in0=ac[:, 0:nb],
                                    in1=ac[:, nb:2 * nb],
                                    op=mybir.AluOpType.add)
            nc.vector.tensor_scalar(out=num[:], in0=wk[:], scalar1=0.2,
                                    scalar2=None, op0=mybir.AluOpType.mult)
            nc.vector.tensor_tensor(out=wk[:], in0=wk[:], in1=num[:],
                                    op=mybir.AluOpType.max)
            nc.scalar.activation(num[:], wk[:],
                                 mybir.ActivationFunctionType.Exp, scale=1.0)
            # ---- W[p, j, b] = num[p, b] * (z[p, b] == j), D-major
            wsel = cpool.tile([128, D * nb], BF16)
            w3 = wsel[:].rearrange("p (d b) -> p d b", b=nb)
            NCHUNK = 6
            cb = -(-nb // NCHUNK)
            wstate = {"done": 0, "warm": WARM}

            def emit_w_chunk():
                s0 = wstate["done"]
                if s0 >= nb:
                    return
                if wstate["warm"] > 0:
                    wstate["warm"] -= 1
                    s1 = min(s0 + BANK_WINS, nb)
                else:
                    s1 = min(s0 + cb, nb)
                for j in range(D):
                    if WSPLIT:
                        eng = nc.gpsimd if j >= D - WSPLIT else nc.vector
                        eng.tensor_scalar(
                            out=w3[:, j, s0:s1], in0=zc[:, s0:s1],
                            scalar1=float(j), scalar2=None,
                            op0=mybir.AluOpType.is_equal)
                        eng.tensor_tensor(
                            out=w3[:, j, s0:s1], in0=w3[:, j, s0:s1],
                            in1=num[:, s0:s1], op=mybir.AluOpType.mult)
                    else:
                        if EQPOOL:
                            eng = nc.gpsimd if j < EQPOOL else nc.vector
                        else:
                            eng = nc.vector if j % 2 == 0 else nc.gpsimd
                        eng.tensor_scalar(
                            out=w3[:, j, s0:s1], in0=zc[:, s0:s1],
                            scalar1=float(j), scalar2=None,
                            op0=mybir.AluOpType.is_equal)
                        nc.vector.tensor_tensor(
                            out=w3[:, j, s0:s1], in0=w3[:, j, s0:s1],
                            in1=num[:, s0:s1], op=mybir.AluOpType.mult)
                wstate["done"] = s1

            for _ in range(4):
                emit_w_chunk()
            # ---- streaming accumulate + interleaved epilogue
            acc_sb = cpool.tile([128, ntp * 128], BF16)
            if ntp * 128 > ncols:
                nc.gpsimd.memset(acc_sb[0:33, ncols:ntp * 128], 0.0)
            out_sb = cpool.tile([128, ntp * 32], BF16)
            rcp = cpool.tile([128, ntp], BF16)
            state = {"tile": 0, "round": 0, "odma": []}

            def emit_transposes(bank_done):
                """Emit transposes fully covered by merged banks <= bank_done."""
                max_t = min(ntp, ((bank_done + 1) * BANK_WINS * D) // 128)
                if bank_done >= n_banks - 1:
                    max_t = ntp
                while state["tile"] < max_t:
                    t = state["tile"]
                    r = state["round"]
                    t0 = bounds[r]
                    if r not in state["tps"]:
                        if BIASFOLD:
                            tp_r = tppool.tile([128, TP_TILES * 33], F32,
                                               tag="tp")
                        else:
                            tp_r = tppool.tile([128, TP_TILES * 34], BF16,
                                               tag="tp")
                        state["tps"][r] = tp_r
                    tp = state["tps"][r]
                    if BIASFOLD:
                        # regular matmul against [identity | bias row]:
                        # transposed acc with den*bias folded in
                        # ((acc + den*bias)*rcp == acc*rcp + bias)
                        nc.tensor.matmul(
                            tp[:, (t - t0) * 33:(t - t0) * 33 + 33],
                            acc_sb[0:33, t * 128:(t + 1) * 128],
                            ident[0:33, 0:33],
                            start=True, stop=True)
                    else:
                        nc.tensor.transpose(
                            tp[:, (t - t0) * 34:(t - t0) * 34 + 33],
                            acc_sb[0:33, t * 128:(t + 1) * 128],
                            ident[0:33, 0:33])
                    state["tile"] = t + 1
                    t1 = bounds[r + 1] if r + 1 < len(bounds) else ntp
                    if t + 1 == t1:
                        if not ROUNDEND:
                            emit_round(r, t0, t1, state["tps"][r])
                        state["round"] = r + 1

            def emit_round(r, t0, t1, tp):
                cw = 33 if BIASFOLD else 34
                tpv = tp[:, :(t1 - t0) * cw] \
                    .rearrange("p (t c) -> p t c", c=cw)
                with nc.allow_low_precision(reason="1/den bf16"):
                    nc.vector.reciprocal(rcp[:, t0:t1], tpv[:, :, 32])
                ov = out_sb[:, t0 * 32:t1 * 32] \
                    .rearrange("p (t c) -> p t c", c=32)
                nc.vector.tensor_tensor(
                    out=ov, in0=tpv[:, :, 0:32],
                    in1=rcp[:, t0:t1].rearrange("p (t o) -> p t o", o=1)
                    .to_broadcast([128, t1 - t0, 32]),
                    op=mybir.AluOpType.mult)
                if not BIASFOLD:
                    nc.vector.tensor_tensor(
                        out=ov, in0=ov,
                        in1=bias_sb.rearrange("p (o c) -> p o c", o=1)
                        .to_broadcast([128, t1 - t0, 32]),
                        op=mybir.AluOpType.add)
                nc.scalar.activation(out_sb[:, t0 * 32:t1 * 32],
                                     out_sb[:, t0 * 32:t1 * 32],
                                     mybir.ActivationFunctionType.Sigmoid)
                state["odma"].append((t0, t1))
                flush = (r % OUT_EVERY == OUT_EVERY - 1
                         or t1 >= ntp)
                if flush:
                    o0 = state["odma"][0][0]
                    o1 = state["odma"][-1][1]
                    state["odma"] = []
                    nc.scalar.dma_start(out[:, o0 * 32:o1 * 32],
                                        out_sb[:, o0 * 32:o1 * 32])

            # tp tiles must be allocated per round; pre-wire creation order
            state["tps"] = {}
            for s0 in range(0, nb, HE_CHUNK_BANKS * BANK_WINS):
                s1 = min(s0 + HE_CHUNK_BANKS * BANK_WINS, nb)
                hc = hepool.tile([128, HE_CHUNK_BANKS * BANK_WINS * 32], BF16,
                                 tag="hec")
                nc.sync.dma_start(hc[:, :(s1 - s0) * 32],
                                  he[:, s0 * 32:s1 * 32])
                # keep the on-device W build ~3 he-chunks ahead of the
                # matmul stream so merges queue promptly behind it
                if wstate["done"] < min(nb, s1 + 3 * HE_CHUNK_BANKS * BANK_WINS):
                    emit_w_chunk()
                for b in range(s0 // BANK_WINS,
                               s0 // BANK_WINS + HE_CHUNK_BANKS):
                    if b >= n_banks:
                        break
                    while wstate["done"] < min(nb, (b + 1) * BANK_WINS):
                        emit_w_chunk()
                    w0 = b * BANK_WINS
                    w1 = min(w0 + BANK_WINS, nb)
                    ap = accpool.tile([128, 512], F32, tag="acc")
                    for w in range(w0, min(w0 + BANK_WINS, nb)):
                        lw = w - s0
                        nc.tensor.matmul(
                            ap[0:32, (w - w0) * D:(w - w0 + 1) * D],
                            hc[:, lw * 32:(lw + 1) * 32],
                            w3[:, :, w],
                            start=True, stop=True)
                        nc.tensor.matmul(
                            ap[32:33, (w - w0) * D:(w - w0 + 1) * D],
                            ones_sb[:], w3[:, :, w],
                            start=True, stop=True)
                    # evacuate bank -> acc_sb (mostly ACT; Copy is in
                    # every act table set so no reloads)
                    if EVAC_DVE and b % EVAC_DVE == EVAC_DVE - 1:
                        nc.vector.tensor_copy(
                            out=acc_sb[0:33, w0 * D:w1 * D],
                            in_=ap[0:33, 0:(w1 - w0) * D])
                    else:
                        nc.scalar.activation(
                            acc_sb[0:33, w0 * D:w1 * D],
                            ap[0:33, 0:(w1 - w0) * D],
                            mybir.ActivationFunctionType.Copy)
                    # merge carries within the bank (Pool, sbuf only)
                    if w1 - w0 > 1:
                        a3o = acc_sb[0:33, w0 * D + D:w1 * D] \
                            .rearrange("p (b d) -> p b d", d=D)
                        a3i = acc_sb[0:33, w0 * D + D - 1:w1 * D - 1] \
                            .rearrange("p (b d) -> p b d", d=D)
                        nc.gpsimd.tensor_tensor(
                            out=a3o[:, :, 0:1], in0=a3o[:, :, 0:1],
                            in1=a3i[:, :, 0:1], op=mybir.AluOpType.add)
                    # interleave transposes/epilogue with a lag so their
                    # evac/merge deps are long resolved by the time in-order
                    # PE.SEQ reaches them
                    if LAG > 0 and b >= LAG:
                        emit_transposes(b - LAG)
            emit_transposes(n_banks - 1)
            assert state["tile"] == ntp and state["round"] == len(bounds)
            if ROUNDEND:
                for r, t0 in enumerate(bounds):
                    t1 = bounds[r + 1] if r + 1 < len(bounds) else ntp
                    emit_round(r, t0, t1, state["tps"][r])
    nc.compile()
    return nc


# ---------------------------------------------------------------- runners
def _run(nc, in_maps, trace):
    if os.environ.get("GAT_SIM"):
        from concourse.bass_interp import CoreSim
        results = []
        for m in in_maps:
            sim = CoreSim(nc, require_finite=False, require_nnan=False)
            for k, v in m.items():
                sim.tensor(k)[:] = v
            sim.simulate()
            outs = {}
            for alloc in nc.m.functions[0].allocations:
                if getattr(alloc, 'kind', None) == "ExternalOutput":
                    name = alloc.memorylocations[0].name
                    outs[name] = np.array(sim.tensor(name))
            results.append(outs)
        class R: pass
        r = R(); r.results = results; r.exec_time_ns = None
        return r
    return run_bass_kernel_spmd(nc, in_maps, core_ids=list(range(N_CORES)),
                                trace=trace)


def kernel(x, edge_index, W, att_src, att_dst, bias):
    global LAST_RESULTS, _LAST_NCS
    x = np.asarray(x, np.float32)
    edge_index = np.asarray(edge_index)
    W = np.asarray(W, np.float32)
    att_src = np.asarray(att_src, np.float32)
    att_dst = np.asarray(att_dst, np.float32)
    bias_np = np.asarray(bias, np.float32)

    N, C_in = x.shape
    C_out = W.shape[1]
    assert C_in == 128 and C_out == 32, (C_in, C_out)
    trace = bool(os.environ.get("GAT_TRACE"))

    loops = np.arange(N, dtype=np.int64)
    src = np.concatenate([edge_index[0].astype(np.int64), loops])
    dst = np.concatenate([edge_index[1].astype(np.int64), loops])

    Nc, prof, nb, rank_of_slot, col_of_slot, final_col, cores = \
        _plan(dst, N, N_CORES)
    nch = -(-Nc // 128)

    # ---- phase 1: project every node once (node-partitioned) ----
    ws = W @ att_src
    wd = W @ att_dst
    wext = np.concatenate([W, ws[:, None], wd[:, None]], 1).astype(NPBF16)
    in1 = []
    for c in range(N_CORES):
        xt = np.zeros((128, CW + nch * 128), NPBF16)
        n0 = c * Nc
        xt[:, 0:CW] = wext
        xt[:, CW:CW + Nc] = x[n0:n0 + Nc].T.astype(NPBF16)
        in1.append({"xt": xt})

    key1 = ("proj", nch)
    if key1 not in _NC_CACHE:
        _NC_CACHE[key1] = _build_proj(nch)
    nc1 = _NC_CACHE[key1]
    res1 = _run(nc1, in1, trace)

    h_all = np.zeros((N, 32), NPBF16)
    as_all = np.zeros(N, NPBF16)
    ad_all = np.zeros(N, NPBF16)
    for c in range(N_CORES):
        o = res1.results[c]
        h = np.asarray(o["h_out"]).reshape(128, nch, 32) \
            .transpose(1, 0, 2).reshape(nch * 128, 32)[:Nc]
        a = np.asarray(o["a_out"]).reshape(128, nch, 2) \
            .transpose(1, 0, 2).reshape(nch * 128, 2)[:Nc]
        sl = slice(c * Nc, (c + 1) * Nc)
        h_all[sl] = h
        as_all[sl], ad_all[sl] = a[:, 0], a[:, 1]

    # ---- host gather: per-core streams (pure indexing) ----
    h_pad = np.concatenate([h_all, np.zeros((1, 32), NPBF16)], 0)
    as_pad = np.concatenate([as_all, [NPBF16(-300.0)]])
    ad_pad = np.concatenate([ad_all, [NPBF16(-300.0)]])

    z_np = np.broadcast_to(
        col_of_slot.reshape(nb, 128).T.astype(NPBF16), (128, nb)).copy()
    consts = np.zeros((128, D + 33 + 32), NPBF16)
    consts[:, 0:D] = np.arange(D, dtype=np.float32).astype(NPBF16)[None, :]
    idb = np.eye(33, dtype=np.float32)
    idb[32, 0:32] = bias_np          # transpose-matmul folds den*bias in
    consts[0:33, D:D + 33] = idb.astype(NPBF16)
    consts[:, D + 33:] = bias_np.astype(NPBF16)[None, :]

    ncols = nb * D
    ntp = -(-ncols // 128)

    in2 = []
    for c in range(N_CORES):
        src_of_slot, dst_of_slot = _core_slots(
            cores[c], prof, rank_of_slot, len(src), Nc, c)
        sid = np.where(src_of_slot >= 0, src[np.maximum(src_of_slot, 0)], N)
        did = np.where(dst_of_slot >= 0, dst_of_slot, N)
        he = np.empty((128, nb * 32), NPBF16)
        he3 = he.reshape(128, nb, 32)
        hrows = h_pad[sid].reshape(nb, 128, 32)
        he3[:, :, :] = hrows.transpose(1, 0, 2)
        a_st = np.empty((128, 3 * nb + D + 33 + 32), NPBF16)
        a_st[:, 0:nb] = as_pad[sid].reshape(nb, 128).T
        a_st[:, nb:2 * nb] = ad_pad[did].reshape(nb, 128).T
        a_st[:, 2 * nb:3 * nb] = z_np
        a_st[:, 3 * nb:] = consts
        in2.append({"he": he, "a_st": a_st})

    key2 = ("agg", nb, LAG, WARM, ACCB, EVAC_DVE, WSPLIT, EQPOOL, ROUNDEND, BIASFOLD, OUT_EVERY, HEBUFS)
    if key2 not in _NC_CACHE:
        _NC_CACHE[key2] = _build_agg(nb)
    nc2 = _NC_CACHE[key2]
    res2 = _run(nc2, in2, trace)
    LAST_RESULTS = (res1, res2)
    _LAST_NCS = (nc1, nc2)

    out_full = np.zeros((N, C_out), np.float32)
    fc = final_col                       # rank -> global col id
    tpi = fc // 128                      # transpose tile
    tpp = fc % 128                       # partition within tile
    for c in range(N_CORES):
        o = np.asarray(res2.results[c]["out"]).astype(np.float32)
        o3 = o.reshape(128, ntp, 32)
        vals = o3[tpp, tpi, :]           # [n_ranks, 32]
        perm = cores[c][3]
        out_full[c * Nc + perm] = vals
    return out_full


# revision 15
# speedup vs baseline: 1.0650x; 1.0004x over previous
"""GAT encoder (PyG GATConv-style, single head) for Trainium2, 8 NeuronCores.

v3: segment-sum as PE matmuls with on-device selection masks.

  Phase 1 (proj): node-partitioned x @ [W | W@att_src | W@att_dst] -> per
  node h (32, bf16) and logits a_s, a_d (bf16).  No activations.

  Host (pure indexing): destinations degree-sorted per core against a
  COMMON degree profile (elementwise max across cores, +0.6% slots) so all
  8 cores share one program.  Edge slots laid dst-major into 128-slot
  windows; each window owns D=13 psum columns (col 0 = carry for a dst
  straddling from the previous window, straddler sits in col D-1; carries
  never cross a 39-window psum bank).

  Phase 2 (agg) per core, DMA-bound at ~17MB (h-stream 14.1MB @360B/ns):
    num  = exp(max(a, 0.2a)), a = a_s[src]+a_d[dst]      (DVE + ACT exp)
    W    = num * (z == j) built as D per-plane tensor_scalar/tensor_tensor
           ops (is_equal planes split DVE/Pool, mults DVE at 2x; D-major
           layout keeps every op densely packed -> 2x mode)
    per window w: two matmuls (stationary ldweights is free in the cost
    model; Matmult cost = out free size only):
      psum[0:32, wD:(w+1)D]  = he_w.T @ W_w    (h gathered per edge slot)
      psum[32:33, wD:(w+1)D] = ones.T @ W_w    (den row, aligned tile pos)
    per bank: ACT-copy psum -> acc_sb bf16; Pool merges window carries.
    PE re-matmul [33,128] acc tiles against [identity | bias-row] ->
    transposed dst-major psum tiles with den*bias folded in
    ((acc + den*bias)/den == acc/den + bias), then per-15-tile rounds:
    DVE rcp(den) + multiply, ACT sigmoid, batched out DMA.

  Host unshards via the rank->column map.  bf16 streams, f32 psum
  accumulate; rel err ~9e-3 vs the 2e-2 gate (fp8 h measured 2.9e-2).
"""
import os
import sys

for _p in ('/opt/trn_rl_repo',):
    if _p not in sys.path and os.path.isdir(_p):
        sys.path.insert(0, _p)

import numpy as np
import ml_dtypes

import concourse.mybir as mybir
import concourse.tile as tile
from concourse import bacc
from concourse.bass_utils import run_bass_kernel_spmd

F32 = mybir.dt.float32
BF16 = mybir.dt.bfloat16
NPBF16 = ml_dtypes.bfloat16

N_CORES = 8
PSUM_CHUNK = 15          # proj: 15*34 = 510 <= 512 f32 per PSUM bank
CW = 34                  # projected width: 32 h + a_s + a_d
D = 10                   # psum columns per 128-slot window
BANK_WINS = 51           # windows per psum bank (51*10 = 510 <= 512)
TP_TILES = 15            # [128,33] bf16 tiles per epilogue round, stride 34
HE_CHUNK_BANKS = 2       # he DMA granularity (banks per DMA)
LAG = int(os.environ.get("GAT_LAG", "0"))    # transpose lag (banks); 0 = all at end
ACCB = int(os.environ.get("GAT_ACCB", "6"))  # psum banks for accumulation
EVAC_DVE = int(os.environ.get("GAT_EVACDVE", "0"))  # every Nth evac on DVE (0=ACT only)
WSPLIT = int(os.environ.get("GAT_WSPLIT", "0"))  # Pool planes (0 = j%2 eq split, mults DVE)
EQPOOL = int(os.environ.get("GAT_EQPOOL", "0"))  # eq planes on Pool (only if WSPLIT=0; 0 = j%2)
ROUNDEND = int(os.environ.get("GAT_ROUNDEND", "0"))  # 1 = epilogue rounds after loop
BIASFOLD = int(os.environ.get("GAT_BIASFOLD", "1"))  # 1 = bias via transpose matmul
OUT_EVERY = int(os.environ.get("GAT_OUTEVERY", "4"))  # rounds per out DMA
WARM = int(os.environ.get("GAT_WARM", "0"))  # fine-grained W warmup chunks
HEBUFS = int(os.environ.get("GAT_HEBUFS", "6"))  # he stream buffers

LAST_RESULTS = None
_NC_CACHE = {}
_LAST_NCS = ()


def sim_exec_time_ns():
    """Sum of TimelineSim estimates for the programs run by kernel()."""
    from concourse.timeline_sim import TimelineSim
    return int(sum(TimelineSim(nc, trace=False).simulate()
                   for nc in _LAST_NCS))


# ---------------------------------------------------------------- planning
def _profile_plan(prof):
    """Pack the common degree profile into 128-slot windows.

    Protocol: per window, col 0 is reserved for a carry (continuation of the
    previous window's straddling dst), new dsts take cols 1..D-2, and a dst
    that straddles into the next window takes col D-1 (its continuation gets
    col 0 there).  The last window of each 42-window bank is padded so no
    carry crosses a psum bank.

    Returns (nb, rank_of_slot[nb*128], col_of_slot[nb*128],
    final_col[n_ranks] (global col id w*D+col)).
    """
    n = len(prof)
    rank_of_slot = []
    col_of_slot = []
    final_col = np.zeros(n, np.int64)
    w = 0          # current window index
    p = 0          # next free slot in window
    newd = 0       # new dsts started in this window

    def close_window():
        nonlocal w, p, newd
        pad = 128 - p
        rank_of_slot.extend([-1] * pad)
        col_of_slot.extend([0] * pad)
        w += 1
        p = 0
        newd = 0

    # visit ranks big/small interleaved so the new-dst cap (D-2 per
    # window) never closes a half-empty window in the small-degree tail
    lo, hi = 0, n - 1
    visit = []
    flip = True
    while lo <= hi:
        if flip:
            visit.append(lo); lo += 1
        else:
            visit.append(hi); hi -= 1
        flip = not flip
    for r in visit:
        d = int(prof[r])
        assert d >= 1
        while True:
            if p >= 128:
                close_window()
                continue
            if newd >= D - 2:
                close_window()
                continue
            space = 128 - p
            straddle = d > space
            if straddle and (w % BANK_WINS) == BANK_WINS - 1:
                # no carry across banks: pad and start in next bank
                close_window()
                continue
            break
        newd += 1
        if straddle:
            rank_of_slot.extend([r] * space)
            col_of_slot.extend([D - 1] * space)
            p = 128
            close_window()
            rest = d - space
            assert rest <= 128
            rank_of_slot.extend([r] * rest)
            col_of_slot.extend([0] * rest)
            p = rest
            final_col[r] = w * D + 0
        else:
            col = newd  # 1..D-2
            rank_of_slot.extend([r] * d)
            col_of_slot.extend([col] * d)
            p += d
            final_col[r] = w * D + col
    if p > 0:
        close_window()
    nb = w
    return (nb, np.array(rank_of_slot, np.int64),
            np.array(col_of_slot, np.int64), final_col)


def _plan(dst, N, n_cores):
    """Common profile + per-core degree-sorted dst orders and edge lists."""
    Nc = N // n_cores
    assert Nc * n_cores == N
    degs = np.zeros((n_cores, Nc), np.int64)
    cores = []
    for c in range(n_cores):
        sel = (dst >= c * Nc) & (dst < (c + 1) * Nc)
        idx = np.nonzero(sel)[0]
        d_c = dst[idx] - c * Nc
        order = np.argsort(d_c, kind='stable')
        eidx_sorted = idx[order]            # edge ids grouped by local dst
        counts = np.bincount(d_c, minlength=Nc).astype(np.int64)
        offsets = np.zeros(Nc + 1, np.int64)
        np.cumsum(counts, out=offsets[1:])
        perm = np.argsort(-counts, kind='stable')   # rank -> local dst
        degs[c] = counts[perm]
        cores.append((eidx_sorted, counts, offsets, perm))
    prof = degs.max(axis=0)
    assert prof[-1] >= 1 and prof[0] <= 128
    nb, rank_of_slot, col_of_slot, final_col = _profile_plan(prof)
    return Nc, prof, nb, rank_of_slot, col_of_slot, final_col, cores


def _core_slots(core_plan, prof, rank_of_slot, n_edges, Nc, c):
    """Per-core (src_of_slot, dst_of_slot) with -1 for padding slots."""
    eidx_sorted, counts, offsets, perm = core_plan
    nslots = len(rank_of_slot)
    src_of_slot = np.full(nslots, -1, np.int64)
    dst_of_slot = np.full(nslots, -1, np.int64)
    # slot positions per rank, in slot order
    pos = np.nonzero(rank_of_slot >= 0)[0]
    rk = rank_of_slot[pos]
    # index of each slot within its rank (0..prof[r]-1), slots of a rank
    # appear in increasing slot order
    order = np.argsort(rk, kind='stable')
    within = np.zeros(len(rk), np.int64)
    cum = np.zeros(len(prof) + 1, np.int64)
    np.cumsum(prof, out=cum[1:])
    within[order] = np.arange(len(rk)) - cum[rk[order]]
    ldst = perm[rk]                        # local dst of each real slot
    k = within
    valid = k < counts[ldst]
    epos = offsets[ldst[valid]] + k[valid]
    src_of_slot[pos[valid]] = -2           # placeholder, filled below
    sv = np.full(len(rk), -1, np.int64)
    sv[valid] = eidx_sorted[epos]
    src_of_slot[pos] = sv                  # edge id per slot (-1 pad)
    dst_of_slot[pos[valid]] = ldst[valid] + c * Nc
    return src_of_slot, dst_of_slot


# ---------------------------------------------------------------- phase 1
def _build_proj(nch):
    nc = bacc.Bacc("TRN2", target_bir_lowering=False, debug=False,
                   num_devices=N_CORES)
    xt = nc.dram_tensor("xt", [128, CW + nch * 128], BF16,
                        kind="ExternalInput").ap()
    h_out = nc.dram_tensor("h_out", [128, nch * 32 + nch * 2], BF16,
                           kind="ExternalOutput").ap()
    N_IN_DMA = 4
    with tile.TileContext(nc) as tc:
        with (
            tc.tile_pool(name="const", bufs=1) as cpool,
            tc.tile_pool(name="ps", bufs=8, space="PSUM") as pspool,
        ):
            xc = cpool.tile([128, CW + nch * 128], BF16)
            wsb = xc[:, 0:CW]
            qs = [0]
            left = nch
            for frac in (0.40, 0.30, 0.20, 0.10):
                qs.append(min(nch, qs[-1] + max(1, int(round(nch * frac)))))
            qs[-1] = nch
            for i, (k, k1) in enumerate(zip(qs[:-1], qs[1:])):
                if k1 > k:
                    lo = 0 if i == 0 else CW + k * 128
                    nc.sync.dma_start(xc[:, lo:CW + k1 * 128],
                                      xt[:, lo:CW + k1 * 128])
            hsb = cpool.tile([128, nch * 32 + nch * 2], BF16)
            asd = hsb[:, nch * 32:]
            n_chunks = -(-nch // PSUM_CHUNK)
            marks = [(n_chunks * 4) // 8, (n_chunks * 6) // 8, n_chunks - 1, n_chunks]
            flush = [0] + sorted(set(min(m * PSUM_CHUNK, nch) for m in marks))
            b0 = 0
            while b0 < nch:
                cn = min(PSUM_CHUNK, nch - b0)
                ps = pspool.tile([128, PSUM_CHUNK * CW], F32, tag="ps")
                for j in range(b0, b0 + cn):
                    nc.tensor.matmul(
                        ps[:, (j - b0) * CW:(j - b0 + 1) * CW],
                        xc[:, CW + j * 128:CW + (j + 1) * 128],
                        wsb, start=True, stop=True)
                psv = ps[:, :cn * CW].rearrange("p (s f) -> p s f", f=CW)
                if (b0 // PSUM_CHUNK) % 2 == 0:
                    nc.scalar.activation(
                        hsb[:, b0 * 32:(b0 + cn) * 32]
                        .rearrange("p (s c) -> p s c", c=32),
                        psv[:, :, 0:32],
                        mybir.ActivationFunctionType.Copy)
                else:
                    nc.vector.tensor_copy(
                        out=hsb[:, b0 * 32:(b0 + cn) * 32]
                        .rearrange("p (s c) -> p s c", c=32),
                        in_=psv[:, :, 0:32])
                nc.vector.tensor_copy(
                    out=asd[:, b0 * 2:(b0 + cn) * 2]
                    .rearrange("p (s c) -> p s c", c=2),
                    in_=psv[:, :, 32:34])
                b1 = b0 + cn
                # flush h_out at staged boundaries (earlier pieces bigger)
                for lo, hi in zip(flush[:-1], flush[1:]):
                    if b0 < hi <= b1:
                        # last piece also carries the appended a columns
                        end = nch * 32 + nch * 2 if hi == nch else hi * 32
                        nc.scalar.dma_start(h_out[:, lo * 32:end],
                                            hsb[:, lo * 32:end])
                b0 = b1
    nc.compile()
    return nc


# ---------------------------------------------------------------- phase 2
def _build_agg(nb):
    n_banks = -(-nb // BANK_WINS)
    ncols = nb * D
    ntp = -(-ncols // 128)                # transpose tiles
    nc = bacc.Bacc("TRN2", target_bir_lowering=False, debug=False,
                   num_devices=N_CORES)
    he = nc.dram_tensor("he", [128, nb * 32], BF16, kind="ExternalInput").ap()
    a_st = nc.dram_tensor("a_st", [128, 3 * nb + D + 33 + 32], BF16,
                          kind="ExternalInput").ap()
    out = nc.dram_tensor("out", [128, ntp * 32], BF16,
                         kind="ExternalOutput").ap()
    # epilogue round boundaries (tiles); last rounds smaller for the tail
    bounds = list(range(0, ntp, TP_TILES))
    if len(bounds) >= 2 and ntp - bounds[-1] > 6:
        bounds = bounds[:-1] + [ntp - 12, ntp - 6]
    elif ntp > 6:
        bounds = bounds[:-1] + [max(0, ntp - 6)]
    bounds = sorted(set(b for b in bounds if b < ntp))
    with tile.TileContext(nc) as tc:
        with (
            tc.tile_pool(name="const", bufs=1) as cpool,
            tc.tile_pool(name="hec", bufs=HEBUFS) as hepool,
            tc.tile_pool(name="acc", bufs=ACCB, space="PSUM") as accpool,
            tc.tile_pool(name="tp", bufs=8 - ACCB, space="PSUM") as tppool,
        ):
            # ---- one merged input transfer: a_s | a_d | z | consts
            ones_sb = cpool.tile([128, 1], BF16)
            nc.gpsimd.memset(ones_sb[:], 1.0)
            ac = cpool.tile([128, 3 * nb + D + 33 + 32], BF16)
            nc.sync.dma_start(ac[:], a_st[:])
            zc = ac[:, 2 * nb:3 * nb]
            cst = ac[:, 3 * nb:]
            ident = cst[:, D:D + 33]      # identity in partitions 0..32
            bias_sb = cst[:, D + 33:D + 33 + 32]
            # ---- num = exp(max(a, 0.2a))  [128, nb]
            num = cpool.tile([128, nb], BF16)
            wk = cpool.tile([128, nb], BF16)
            nc.vector.tensor_tensor(out=wk[:], in0=ac[:, 0:nb],
                                    in1=ac[:, nb:2 * nb],
                                    op=mybir.AluOpType.add)
            nc.vector.tensor_scalar(out=num[:], in0=wk[:], scalar1=0.2,
                                    scalar2=None, op0=mybir.AluOpType.mult)
            nc.vector.tensor_tensor(out=wk[:], in0=wk[:], in1=num[:],
                                    op=mybir.AluOpType.max)
            nc.scalar.activation(num[:], wk[:],
                                 mybir.ActivationFunctionType.Exp, scale=1.0)
            # ---- W[p, j, b] = num[p, b] * (z[p, b] == j), D-major
            wsel = cpool.tile([128, D * nb], BF16)
            w3 = wsel[:].rearrange("p (d b) -> p d b", b=nb)
            NCHUNK = 6
            cb = -(-nb // NCHUNK)
            wstate = {"done": 0, "warm": WARM}

            def emit_w_chunk():
                s0 = wstate["done"]
                if s0 >= nb:
                    return
                if wstate["warm"] > 0:
                    wstate["warm"] -= 1
                    s1 = min(s0 + BANK_WINS, nb)
                else:
                    s1 = min(s0 + cb, nb)
                for j in range(D):
                    if WSPLIT:
                        eng = nc.gpsimd if j >= D - WSPLIT else nc.vector
                        eng.tensor_scalar(
                            out=w3[:, j, s0:s1], in0=zc[:, s0:s1],
                            scalar1=float(j), scalar2=None,
                            op0=mybir.AluOpType.is_equal)
                        eng.tensor_tensor(
                            out=w3[:, j, s0:s1], in0=w3[:, j, s0:s1],
                            in1=num[:, s0:s1], op=mybir.AluOpType.mult)
                    else:
                        if EQPOOL:
                            eng = nc.gpsimd if j < EQPOOL else nc.vector
                        else:
                            eng = nc.vector if j % 2 == 0 else nc.gpsimd
                        eng.tensor_scalar(
                            out=w3[:, j, s0:s1], in0=zc[:, s0:s1],
                            scalar1=float(j), scalar2=None,
                            op0=mybir.AluOpType.is_equal)
                        nc.vector.tensor_tensor(
                            out=w3[:, j, s0:s1], in0=w3[:, j, s0:s1],
                            in1=num[:, s0:s1], op=mybir.AluOpType.mult)
                wstate["done"] = s1

            for _ in range(4):
                emit_w_chunk()
            # ---- streaming accumulate + interleaved epilogue
            acc_sb = cpool.tile([128, ntp * 128], BF16)
            if ntp * 128 > ncols:
                nc.gpsimd.memset(acc_sb[0:33, ncols:ntp * 128], 0.0)
            out_sb = cpool.tile([128, ntp * 32], BF16)
            rcp = cpool.tile([128, ntp], BF16)
            state = {"tile": 0, "round": 0, "odma": []}

            def emit_transposes(bank_done):
                """Emit transposes fully covered by merged banks <= bank_done."""
                max_t = min(ntp, ((bank_done + 1) * BANK_WINS * D) // 128)
                if bank_done >= n_banks - 1:
                    max_t = ntp
                while state["tile"] < max_t:
                    t = state["tile"]
                    r = state["round"]
                    t0 = bounds[r]
                    if r not in state["tps"]:
                        if BIASFOLD:
                            tp_r = tppool.tile([128, TP_TILES * 33], F32,
                                               tag="tp")
                        else:
                            tp_r = tppool.tile([128, TP_TILES * 34], BF16,
                                               tag="tp")
                        state["tps"][r] = tp_r
                    tp = state["tps"][r]
                    if BIASFOLD:
                        # regular matmul against [identity | bias row]:
                        # transposed acc with den*bias folded in
                        # ((acc + den*bias)*rcp == acc*rcp + bias)
                        nc.tensor.matmul(
                            tp[:, (t - t0) * 33:(t - t0) * 33 + 33],
                            acc_sb[0:33, t * 128:(t + 1) * 128],
                            ident[0:33, 0:33],
                            start=True, stop=True)
                    else:
                        nc.tensor.transpose(
                            tp[:, (t - t0) * 34:(t - t0) * 34 + 33],
                            acc_sb[0:33, t * 128:(t + 1) * 128],
                            ident[0:33, 0:33])
                    state["tile"] = t + 1
                    t1 = bounds[r + 1] if r + 1 < len(bounds) else ntp
                    if t + 1 == t1:
                        if not ROUNDEND:
                            emit_round(r, t0, t1, state["tps"][r])
                        state["round"] = r + 1

            def emit_round(r, t0, t1, tp):
                cw = 33 if BIASFOLD else 34
                tpv = tp[:, :(t1 - t0) * cw] \
                    .rearrange("p (t c) -> p t c", c=cw)
                with nc.allow_low_precision(reason="1/den bf16"):
                    nc.vector.reciprocal(rcp[:, t0:t1], tpv[:, :, 32])
                ov = out_sb[:, t0 * 32:t1 * 32] \
                    .rearrange("p (t c) -> p t c", c=32)
                nc.vector.tensor_tensor(
                    out=ov, in0=tpv[:, :, 0:32],
                    in1=rcp[:, t0:t1].rearrange("p (t o) -> p t o", o=1)
                    .to_broadcast([128, t1 - t0, 32]),
                    op=mybir.AluOpType.mult)
                if not BIASFOLD:
                    nc.vector.tensor_tensor(
                        out=ov, in0=ov,
                        in1=bias_sb.rearrange("p (o c) -> p o c", o=1)
                        .to_broadcast([128, t1 - t0, 32]),
                        op=mybir.AluOpType.add)
                nc.scalar.activation(out_sb[:, t0 * 32:t1 * 32],
                                     out_sb[:, t0 * 32:t1 * 32],
                                     mybir.ActivationFunctionType.Sigmoid)
                state["odma"].append((t0, t1))
                flush = (r % OUT_EVERY == OUT_EVERY - 1
                         or t1 >= ntp)
                if flush:
                    o0 = state["odma"][0][0]
                    o1 = state["odma"][-1][1]
                    state["odma"] = []
                    nc.scalar.dma_start(out[:, o0 * 32:o1 * 32],
                                        out_sb[:, o0 * 32:o1 * 32])

            # tp tiles must be allocated per round; pre-wire creation order
            state["tps"] = {}
            for s0 in range(0, nb, HE_CHUNK_BANKS * BANK_WINS):
                s1 = min(s0 + HE_CHUNK_BANKS * BANK_WINS, nb)
                hc = hepool.tile([128, HE_CHUNK_BANKS * BANK_WINS * 32], BF16,
                                 tag="hec")
                nc.sync.dma_start(hc[:, :(s1 - s0) * 32],
                                  he[:, s0 * 32:s1 * 32])
                # keep the on-device W build ~3 he-chunks ahead of the
                # matmul stream so merges queue promptly behind it
                if wstate["done"] < min(nb, s1 + 3 * HE_CHUNK_BANKS * BANK_WINS):
                    emit_w_chunk()
                for b in range(s0 // BANK_WINS,
                               s0 // BANK_WINS + HE_CHUNK_BANKS):
                    if b >= n_banks:
                        break
                    while wstate["done"] < min(nb, (b + 1) * BANK_WINS):
                        emit_w_chunk()
                    w0 = b * BANK_WINS
                    w1 = min(w0 + BANK_WINS, nb)
                    ap = accpool.tile([128, 512], F32, tag="acc")
                    for w in range(w0, min(w0 + BANK_WINS, nb)):
                        lw = w - s0
                        nc.tensor.matmul(
                            ap[0:32, (w - w0) * D:(w - w0 + 1) * D],
                            hc[:, lw * 32:(lw + 1) * 32],
                            w3[:, :, w],
                            start=True, stop=True)
                        nc.tensor.matmul(
                            ap[32:33, (w - w0) * D:(w - w0 + 1) * D],
                            ones_sb[:], w3[:, :, w],
                            start=True, stop=True)
                    # evacuate bank -> acc_sb (mostly ACT; Copy is in
                    # every act table set so no reloads)
                    if EVAC_DVE and b % EVAC_DVE == EVAC_DVE - 1:
                        nc.vector.tensor_copy(
                            out=acc_sb[0:33, w0 * D:w1 * D],
                            in_=ap[0:33, 0:(w1 - w0) * D])
                    else:
                        nc.scalar.activation(
                            acc_sb[0:33, w0 * D:w1 * D],
                            ap[0:33, 0:(w1 - w0) * D],
                            mybir.ActivationFunctionType.Copy)
                    # merge carries within the bank (Pool, sbuf only)
                    if w1 - w0 > 1:
                        a3o = acc_sb[0:33, w0 * D + D:w1 * D] \
                            .rearrange("p (b d) -> p b d", d=D)
                        a3i = acc_sb[0:33, w0 * D + D - 1:w1 * D - 1] \
                            .rearrange("p (b d) -> p b d", d=D)
                        nc.gpsimd.tensor_tensor(
                            out=a3o[:, :, 0:1], in0=a3o[:, :, 0:1],
                            in1=a3i[:, :, 0:1], op=mybir.AluOpType.add)
                    # interleave transposes/epilogue with a lag so their
                    # evac/merge deps are long resolved by the time in-order
                    # PE.SEQ reaches them
                    if LAG > 0 and b >= LAG:
                        emit_transposes(b - LAG)
            emit_transposes(n_banks - 1)
            assert state["tile"] == ntp and state["round"] == len(bounds)
            if ROUNDEND:
                for r, t0 in enumerate(bounds):
                    t1 = bounds[r + 1] if r + 1 < len(bounds) else ntp
                    emit_round(r, t0, t1, state["tps"][r])
    nc.compile()
    return nc


# ---------------------------------------------------------------- runners
def _run(nc, in_maps, trace):
    if os.environ.get("GAT_SIM"):
        from concourse.bass_interp import CoreSim
        results = []
        for m in in_maps:
            sim = CoreSim(nc, require_finite=False, require_nnan=False)
            for k, v in m.items():
                sim.tensor(k)[:] = v
            sim.simulate()
            outs = {}
            for alloc in nc.m.functions[0].allocations:
                if getattr(alloc, 'kind', None) == "ExternalOutput":
                    name = alloc.memorylocations[0].name
                    outs[name] = np.array(sim.tensor(name))
            results.append(outs)
        class R: pass
        r = R(); r.results = results; r.exec_time_ns = None
        return r
    return run_bass_kernel_spmd(nc, in_maps, core_ids=list(range(N_CORES)),
                                trace=trace)


def kernel(x, edge_index, W, att_src, att_dst, bias):
    global LAST_RESULTS, _LAST_NCS
    x = np.asarray(x, np.float32)
    edge_index = np.asarray(edge_index)
    W = np.asarray(W, np.float32)
    att_src = np.asarray(att_src, np.float32)
    att_dst = np.asarray(att_dst, np.float32)
    bias_np = np.asarray(bias, np.float32)

    N, C_in = x.shape
    C_out = W.shape[1]
    assert C_in == 128 and C_out == 32, (C_in, C_out)
    trace = bool(os.environ.get("GAT_TRACE"))

    loops = np.arange(N, dtype=np.int64)
    src = np.concatenate([edge_index[0].astype(np.int64), loops])
    dst = np.concatenate([edge_index[1].astype(np.int64), loops])

    Nc, prof, nb, rank_of_slot, col_of_slot, final_col, cores = \
        _plan(dst, N, N_CORES)
    nch = -(-Nc // 128)

    # ---- phase 1: project every node once (node-partitioned) ----
    ws = W @ att_src
    wd = W @ att_dst
    wext = np.concatenate([W, ws[:, None], wd[:, None]], 1).astype(NPBF16)
    in1 = []
    for c in range(N_CORES):
        xt = np.zeros((128, CW + nch * 128), NPBF16)
        n0 = c * Nc
        xt[:, 0:CW] = wext
        xt[:, CW:CW + Nc] = x[n0:n0 + Nc].T.astype(NPBF16)
        in1.append({"xt": xt})

    key1 = ("proj", nch)
    if key1 not in _NC_CACHE:
        _NC_CACHE[key1] = _build_proj(nch)
    nc1 = _NC_CACHE[key1]
    res1 = _run(nc1, in1, trace)

    h_all = np.zeros((N, 32), NPBF16)
    as_all = np.zeros(N, NPBF16)
    ad_all = np.zeros(N, NPBF16)
    for c in range(N_CORES):
        o = res1.results[c]
        ho = np.asarray(o["h_out"])
        h = ho[:, :nch * 32].reshape(128, nch, 32) \
            .transpose(1, 0, 2).reshape(nch * 128, 32)[:Nc]
        a = ho[:, nch * 32:].reshape(128, nch, 2) \
            .transpose(1, 0, 2).reshape(nch * 128, 2)[:Nc]
        sl = slice(c * Nc, (c + 1) * Nc)
        h_all[sl] = h
        as_all[sl], ad_all[sl] = a[:, 0], a[:, 1]

    # ---- host gather: per-core streams (pure indexing) ----
    h_pad = np.concatenate([h_all, np.zeros((1, 32), NPBF16)], 0)
    as_pad = np.concatenate([as_all, [NPBF16(-300.0)]])
    ad_pad = np.concatenate([ad_all, [NPBF16(-300.0)]])

    z_np = np.broadcast_to(
        col_of_slot.reshape(nb, 128).T.astype(NPBF16), (128, nb)).copy()
    consts = np.zeros((128, D + 33 + 32), NPBF16)
    consts[:, 0:D] = np.arange(D, dtype=np.float32).astype(NPBF16)[None, :]
    idb = np.eye(33, dtype=np.float32)
    idb[32, 0:32] = bias_np          # transpose-matmul folds den*bias in
    consts[0:33, D:D + 33] = idb.astype(NPBF16)
    consts[:, D + 33:] = bias_np.astype(NPBF16)[None, :]

    ncols = nb * D
    ntp = -(-ncols // 128)

    in2 = []
    for c in range(N_CORES):
        src_of_slot, dst_of_slot = _core_slots(
            cores[c], prof, rank_of_slot, len(src), Nc, c)
        sid = np.where(src_of_slot >= 0, src[np.maximum(src_of_slot, 0)], N)
        did = np.where(dst_of_slot >= 0, dst_of_slot, N)
        he = np.empty((128, nb * 32), NPBF16)
        he3 = he.reshape(128, nb, 32)
        hrows = h_pad[sid].reshape(nb, 128, 32)
        he3[:, :, :] = hrows.transpose(1, 0, 2)
        a_st = np.empty((128, 3 * nb + D + 33 + 32), NPBF16)
        a_st[:, 0:nb] = as_pad[sid].reshape(nb, 128).T
        a_st[:, nb:2 * nb] = ad_pad[did].reshape(nb, 128).T
        a_st[:, 2 * nb:3 * nb] = z_np
        a_st[:, 3 * nb:] = consts
        in2.append({"he": he, "a_st": a_st})

    key2 = ("agg", nb, LAG, WARM, ACCB, EVAC_DVE, WSPLIT, EQPOOL, ROUNDEND, BIASFOLD, OUT_EVERY, HEBUFS)
    if key2 not in _NC_CACHE:
        _NC_CACHE[key2] = _build_agg(nb)
    nc2 = _NC_CACHE[key2]
    res2 = _run(nc2, in2, trace)
    LAST_RESULTS = (res1, res2)
    _LAST_NCS = (nc1, nc2)

    out_full = np.zeros((N, C_out), np.float32)
    fc = final_col                       # rank -> global col id
    tpi = fc // 128                      # transpose tile
    tpp = fc % 128                       # partition within tile
    for c in range(N_CORES):
        o = np.asarray(res2.results[c]["out"]).astype(np.float32)
        o3 = o.reshape(128, ntp, 32)
        vals = o3[tpp, tpi, :]           # [n_ranks, 32]
        perm = cores[c][3]
        out_full[c * Nc + perm] = vals
    return out_full
